# revision 1
# baseline (speedup 1.0000x reference)
"""Trainium2 Bass kernel for the STU (spectral transform unit) dense-transformer block.

Algorithm (validated against the jax reference in fp64 numpy):
  The FFT causal conv is rewritten as a block-Toeplitz matmul. For each of the
  K=16 filters and each sign branch (the alternating-sign branch folds into the
  filter taps: T^-[s,s'] = phi[s-s'] * (-1)^(s-s')), the causal conv is
    U_br = T_br @ u,  T_br block-Toeplitz with 16 distinct 128x128 blocks.
  sigma^(1/4) folds into the taps. The (k,i)->d projection contracts U with
  M_phi_{plus,minus}; the KU=3 autoregressive taps are shifted-u projections
  with M_u. MLP is a standard gated MLP.

Sharding (8 cores, no cross-core communication, host-side reduce between two
uniform SPMD programs):
  Phase 1: filter-branch-parallel. Core c computes conv + projection for its 4
           of the 32 (k, sign) branches over the full (B, SL): partial spectral.
  Host:    x1 = x + sum_c partial_c
  Phase 2: row-parallel. Core c owns 512 of the 4096 (b, s) rows: adds the AR
           term and computes the gated MLP + residual for its rows.

Precision: the conv runs in fp8 (output magnitude ~0.05 -> noise negligible);
fc1 and fc2 run as compensated hi+lo fp8 splits (h = yh@wh + DoubleRow-paired
cross terms wl@yh + wh@yl, dropping only the second-order yl@wl product),
which gets fp8 DoubleRow matmul rates at bf16-class accuracy. The AR term
stays bf16: its shifted u^T windows would have odd byte offsets in fp8,
which the Ldweights ISA rejects. Measured end-to-end error: 3.9e-3
scale-relative vs the 2e-2 harness gate. rn1/rn2 rmsnorm weights are folded
into the downstream contraction weights host-side (they commute through the
seq-dim conv / shifts).

Schedule notes (all targets are the InstructionCostModel timeline):
 - DMA is a serial ~360GB/s resource; transfers are emitted in the order
   compute needs them (x row-blocks and tw delta-chunks interleaved, weights
   after first-use rows), which removes the 24us/30us startup stalls the
   v1 kernel had.
 - PSUM->SBUF drain copies alternate across DVE/Act (GPSIMD cannot read
   PSUM); Pool takes the SBUF->SBUF rmsnorm multiplies.
 - Phase-1 software-pipelines the next block's conv between conv(I) and
   proj(I) (depth 2-3 for the short early blocks) so the PE covers the
   psum-drain latency; projection iterates cp-outer so its first matmuls
   depend only on the first conv psum drains.
 - Phase-1 warms the PE p-state with dummy matmuls while the first input
   blocks stream in; phase-2 finalizes fc2 m-outer and d-half-outer so each
   row block's residual add and output DMA overlap later matmuls.
 - Both phases issue dummy Square/Sqrt activations at the head of the
   Activation queue so the act-table loads finish before the first rmsnorm
   needs them.
"""

import numpy as np
import ml_dtypes

import concourse.bacc as bacc
import concourse.tile as tile
from concourse import mybir
from concourse.bass_utils import run_bass_kernel_spmd  # noqa: F401 (debug path)
from concourse.masks import make_identity


class _SpmdRunner:
    """Cached-jit SPMD executor: trace/compile once, then repeat calls only
    pay input upload + execution (mirrors bass2jax.run_bass_via_pjrt).

    ``shared`` names inputs that are identical on every core: they are fed
    replicated (host uploads one copy) instead of 8x-concatenated."""

    def __init__(self, nc, shared=(), volatile=()):
        import jax
        import concourse.mybir as _mb
        from concourse.bass2jax import (
            install_neuronx_cc_hook, _bass_exec_p, partition_id_tensor,
        )
        from jax.experimental.shard_map import shard_map
        from jax.sharding import Mesh, PartitionSpec

        install_neuronx_cc_hook()
        self.nc = nc
        assert nc.dbg_addr is None
        pid_name = (nc.partition_id_tensor.name
                    if nc.partition_id_tensor is not None else None)
        in_names, out_names, out_avals = [], [], []
        for alloc in nc.m.functions[0].allocations:
            if not isinstance(alloc, mybir.MemoryLocationSet):
                continue
            name = alloc.memorylocations[0].name
            if alloc.kind == "ExternalInput":
                if name != pid_name:
                    in_names.append(name)
            elif alloc.kind == "ExternalOutput":
                out_names.append(name)
                out_avals.append(jax.core.ShapedArray(
                    tuple(alloc.tensor_shape), mybir.dt.np(alloc.dtype)))
        self.in_names, self.out_names, self.out_avals = in_names, out_names, out_avals
        self.shared = frozenset(shared)
        self.volatile = frozenset(volatile)
        self._dev_cache = {}
        n_params = len(in_names)
        all_names = tuple(in_names + out_names)
        if pid_name is not None:
            all_names = all_names + (pid_name,)

        def _body(*args):
            args = list(args)
            if pid_name is not None:
                args.append(partition_id_tensor())
            return tuple(_bass_exec_p.bind(
                *args,
                out_avals=tuple(out_avals),
                in_names=all_names,
                out_names=tuple(out_names),
                lowering_input_output_aliases=(),
                sim_require_finite=True,
                sim_require_nnan=True,
                nc=nc,
            ))

        import jax.numpy as jnp
        from jax.sharding import NamedSharding
        devices = jax.devices()[:NCORES]
        mesh = Mesh(np.asarray(devices), ("core",))
        rep = PartitionSpec()
        core = PartitionSpec("core")
        in_specs = tuple(
            rep if nm in self.shared else core for nm in in_names
        ) + (core,) * len(out_names)
        out_specs = (core,) * len(out_names)
        donate = tuple(range(n_params, n_params + len(out_names)))
        self._fn = jax.jit(
            shard_map(_body, mesh=mesh, in_specs=in_specs, out_specs=out_specs,
                      check_rep=False),
            donate_argnums=donate, keep_unused=True,
        )
        self._zeros_fn = jax.jit(
            lambda: tuple(
                jnp.zeros((NCORES * a.shape[0], *a.shape[1:]), a.dtype)
                for a in out_avals
            ),
            out_shardings=tuple(
                NamedSharding(mesh, core) for _ in out_avals
            ),
        )
        self._shardings = {
            nm: NamedSharding(mesh, rep if nm in self.shared else core)
            for nm in in_names
        }

    def prep(self, in_maps):
        import hashlib
        import jax
        ins = []
        for nm in self.in_names:
            if nm in self.shared:
                arr = np.ascontiguousarray(in_maps[0][nm])
            else:
                arr = np.concatenate(
                    [np.asarray(in_maps[c][nm]) for c in range(NCORES)], axis=0)
            if nm in self.volatile:
                ins.append(arr)
                continue
            key = (nm, hashlib.md5(arr.tobytes()).hexdigest())
            dev = self._dev_cache.get(key)
            if dev is None:
                self._dev_cache.clear() if len(self._dev_cache) > 32 else None
                dev = jax.device_put(arr, self._shardings[nm])
                self._dev_cache[key] = dev
            ins.append(dev)
        return ins

    def run_prepped(self, ins):
        return self._fn(*ins, *self._zeros_fn())

    def __call__(self, in_maps):
        out_arrs = self.run_prepped(self.prep(in_maps))
        return [
            {nm: np.asarray(out_arrs[i]).reshape(NCORES, *self.out_avals[i].shape)[c]
             for i, nm in enumerate(self.out_names)}
            for c in range(NCORES)
        ]

BF16NP = ml_dtypes.bfloat16
FP8NP = ml_dtypes.float8_e4m3
TAP_SCALE = 1024.0
UT_SCALE = 32.0      # psum (TAP_SCALE*U) -> fp8 ut tiles scale factor: 32/1024
W_SCALE = 16.0       # projection weights scaled by 16 for fp8 range
SP_SCALE = UT_SCALE * W_SCALE  # spectral psum carries 32*16 = 512x
MLP_SCALE = 16.0     # fc1 hi/lo fp8 weights carry 16x for fp8 range
F32 = mybir.dt.float32
F32R = mybir.dt.float32r
F16 = mybir.dt.float16
BF = mybir.dt.bfloat16
FP8 = mybir.dt.float8e4

B, SL, D, K, KU = 2, 2048, 768, 16, 3
NFFT, EPS, P, H = 4096, 1e-5, 128, 3072
NB = SL // P            # 16 seq blocks
DC = D // P             # 6 d-chunks
NBR = 2 * K             # 32 conv branches
NCORES = 8
BPC = NBR // NCORES     # 4 branches per core
RPC = (B * SL) // NCORES  # 512 rows per core
MB = RPC // P           # 4 row blocks per core in phase 2
JC = H // P             # 24 hidden chunks
F1 = 512                # free-dim split of D=768 into 512+256
DR = mybir.MatmulPerfMode.DoubleRow

_cache: dict = {}


def _build_phase1():
    nc = bacc.Bacc("TRN2", target_bir_lowering=False, debug=False, num_devices=NCORES)
    xb = nc.dram_tensor("xb", (B, SL, D), BF, kind="ExternalInput").ap()
    tw = nc.dram_tensor("tw", (NB, P, 2, BPC * P), FP8, kind="ExternalInput").ap()
    wt = nc.dram_tensor("wt", (BPC, DC // 2, P, 2, D), FP8, kind="ExternalInput").ap()
    sp = nc.dram_tensor("sp", (B, SL, D), F16, kind="ExternalOutput").ap()

    with tile.TileContext(nc) as tc:
        with (
            tc.tile_pool(name="const", bufs=1) as const_pool,
            tc.tile_pool(name="ubuf", bufs=1) as ubuf_pool,
            tc.tile_pool(name="work", bufs=3) as work,
            tc.tile_pool(name="drain", bufs=4) as drain_pool,
            tc.tile_pool(name="spill", bufs=3) as spill_pool,
            tc.tile_pool(name="psum_u", bufs=4, space="PSUM") as psum_u_pool,
            tc.tile_pool(name="psum_sp", bufs=2, space="PSUM") as psum_sp_pool,
        ):
            eps_sb = const_pool.tile([P, 1], F32)
            nc.vector.memset(eps_sb, float(EPS))
            dummy = const_pool.tile([P, 1], F32, name="dummy")
            nc.scalar.activation(
                dummy, eps_sb, mybir.ActivationFunctionType.Square)
            nc.scalar.activation(
                dummy, dummy, mybir.ActivationFunctionType.Sqrt)
            tw_sb = const_pool.tile([P, NB, 2, BPC * P], FP8)
            wt_sb = const_pool.tile([P, BPC, DC // 2, 2, D], FP8)

            # one persistent fp8 u tile per (b, J-pair): keeps the conv's
            # dependencies fine-grained (conv block I waits only on the pairs
            # it reads, not on all of u)
            u_t = [[ubuf_pool.tile([P, 2, D], FP8, name=f"u{b}_{jp}")
                    for jp in range(NB // 2)] for b in range(B)]

            def jprep(b, J):
                """x row-block DMA -> rmsnorm -> fp8 u pair-half.
                (rn1_w is folded into the projection weights host-side.)"""
                xt = work.tile([P, D], BF, name="xt")
                nc.sync.dma_start(xt, xb[b, J * P:(J + 1) * P, :])
                sq = work.tile([P, D], F32, name="sq")
                ms = work.tile([P, 1], F32, name="ms")
                if b == 0 and J < 4:
                    nc.vector.scalar_tensor_tensor(
                        sq, xt, 1.0, xt, mybir.AluOpType.mult,
                        mybir.AluOpType.mult, accum_out=ms,
                    )
                else:
                    nc.scalar.activation(
                        sq, xt, mybir.ActivationFunctionType.Square,
                        accum_out=ms,
                    )
                nc.scalar.activation(
                    ms, ms, mybir.ActivationFunctionType.Sqrt,
                    bias=eps_sb, scale=1.0 / D,
                )
                nc.vector.reciprocal(ms, ms)
                nc.gpsimd.tensor_scalar_mul(
                    u_t[b][J // 2][:, J % 2, :], xt, ms
                )

            # PE warmup: dummy matmuls on a zero tile ramp the tensor
            # engine p-state while the first input blocks stream in, so the
            # first real conv matmuls run at full clock
            wz = const_pool.tile([P, 2, BPC * P], FP8, name="wz")
            nc.vector.memset(wz, 0.0)
            wps = psum_u_pool.tile([P, BPC * P], F32, name="psu")
            NW = 24
            for i in range(NW):
                nc.tensor.matmul(wps, lhsT=wz[:, :, 0:P], rhs=wz,
                                 start=i == 0, stop=i == NW - 1, perf_mode=DR)

            # prologue: just enough input for conv block I=0, weights after
            # the first row blocks they trail in the serial DMA queue
            nc.sync.dma_start(tw_sb[:, 0, :, :], tw[0])
            jprep(0, 0)
            jprep(0, 1)
            for j in range(2, 6):
                jprep(0, j)
            nc.sync.dma_start(
                wt_sb[:, :, 0, :, :], wt[:, 0].rearrange("b p k f -> p b k f"))
            next_j = [6, 0]

            drain_engines = (nc.vector, nc.scalar)

            def conv_block(b, I):
                ut_sb = drain_pool.tile([P, DC, BPC * P], FP8, name="ut")
                npair = I // 2 + 1
                for c in range(DC):
                    ps = psum_u_pool.tile([P, BPC * P], F32, name="psu")
                    for Jp in range(npair):
                        nc.tensor.matmul(
                            ps,
                            lhsT=u_t[b][Jp][:, :, c * P:(c + 1) * P],
                            rhs=tw_sb[:, I - 2 * Jp, :, :],
                            start=(Jp == 0),
                            stop=(Jp == npair - 1),
                            perf_mode=DR,
                        )
                    eng = drain_engines[c % 2]
                    if eng is nc.scalar:
                        nc.scalar.activation(
                            ut_sb[:, c, :], ps,
                            mybir.ActivationFunctionType.Copy,
                            scale=float(UT_SCALE / TAP_SCALE),
                        )
                    else:
                        eng.tensor_scalar_mul(
                            ut_sb[:, c, :], ps, float(UT_SCALE / TAP_SCALE)
                        )
                return ut_sb

            def proj_block(b, I, ut_sb):
                psp = psum_sp_pool.tile([P, D], F32, name="psp")
                n_mm = BPC * (DC // 2)
                i_mm = 0
                for cp in range(DC // 2):
                    for br in range(BPC):
                        st = i_mm == 0
                        fin = i_mm == n_mm - 1
                        lh = ut_sb[:, 2 * cp:2 * cp + 2, br * P:(br + 1) * P]
                        nc.tensor.matmul(
                            psp[:, 0:F1], lhsT=lh,
                            rhs=wt_sb[:, br, cp, :, 0:F1],
                            start=st, stop=fin, perf_mode=DR,
                        )
                        nc.tensor.matmul(
                            psp[:, F1:D], lhsT=lh,
                            rhs=wt_sb[:, br, cp, :, F1:D],
                            start=st, stop=fin, perf_mode=DR,
                        )
                        i_mm += 1
                sp_t = spill_pool.tile([P, D], F16, name="spt")
                last = b == B - 1 and I == NB - 1
                if last:
                    nc.scalar.activation(
                        sp_t[:, 0:F1], psp[:, 0:F1],
                        mybir.ActivationFunctionType.Copy,
                        scale=float(1.0 / SP_SCALE),
                    )
                    nc.sync.dma_start(
                        sp[b, I * P:(I + 1) * P, 0:F1], sp_t[:, 0:F1])
                    nc.vector.tensor_scalar_mul(
                        sp_t[:, F1:D], psp[:, F1:D], float(1.0 / SP_SCALE))
                    nc.sync.dma_start(
                        sp[b, I * P:(I + 1) * P, F1:D], sp_t[:, F1:D])
                elif I % 2 == 0:
                    nc.scalar.activation(
                        sp_t, psp, mybir.ActivationFunctionType.Copy,
                        scale=float(1.0 / SP_SCALE),
                    )
                    nc.sync.dma_start(sp[b, I * P:(I + 1) * P, :], sp_t)
                else:
                    nc.vector.tensor_scalar_mul(sp_t, psp, float(1.0 / SP_SCALE))
                    nc.sync.dma_start(sp[b, I * P:(I + 1) * P, :], sp_t)

            # software pipeline: emit the next block's conv before proj(I)
            # so the PE covers the psum-drain latency of block I; the first
            # (short) blocks keep two convs in flight
            from collections import deque
            pend = deque()
            for b in range(B):
                for I in range(NB):
                    # pace the DMA queue: tw delta-chunk I+1, the remaining
                    # projection-weight chunks, and the u row blocks the next
                    # conv iterations will read
                    if b == 0 and I + 1 < NB:
                        nc.sync.dma_start(tw_sb[:, I + 1, :, :], tw[I + 1])
                    if b == 0 and 0 <= I < 2:
                        cp = I + 1
                        nc.sync.dma_start(
                            wt_sb[:, :, cp, :, :],
                            wt[:, cp].rearrange("b p k f -> p b k f"))
                    while next_j[b] <= min(I + 3, NB - 1):
                        jprep(b, next_j[b])
                        next_j[b] += 1
                    if b == 0 and I >= 8:
                        while next_j[1] <= min(2 * (I - 8) + 1, NB - 1):
                            jprep(1, next_j[1])
                            next_j[1] += 1

                    ut_sb = conv_block(b, I)
                    pend.append((b, I, ut_sb))
                    depth = 3 if I < 4 else (2 if I < 7 else 1)
                    while len(pend) > depth:
                        proj_block(*pend.popleft())
            while pend:
                proj_block(*pend.popleft())
    nc.compile()
    return nc


def _build_phase2():
    nc = bacc.Bacc("TRN2", target_bir_lowering=False, debug=False, num_devices=NCORES)
    xr = nc.dram_tensor("xr", (RPC + 2, D), BF, kind="ExternalInput").ap()
    x1r = nc.dram_tensor("x1r", (RPC, D), F32, kind="ExternalInput").ap()
    mut = nc.dram_tensor("mut", (KU, DC, P, D), BF, kind="ExternalInput").ap()
    fc1 = nc.dram_tensor("fc1", (JC, DC, P, 2, 2, P), FP8, kind="ExternalInput").ap()
    fc2 = nc.dram_tensor("fc2", (JC, P, 2, D), FP8, kind="ExternalInput").ap()
    o = nc.dram_tensor("o", (RPC, D), F32, kind="ExternalOutput").ap()


    with tile.TileContext(nc) as tc:
        with (
            tc.tile_pool(name="const", bufs=1) as const_pool,
            tc.tile_pool(name="persist", bufs=1) as persist,
            tc.tile_pool(name="work", bufs=3) as work,
            tc.tile_pool(name="wstream", bufs=3) as wstream,
            tc.tile_pool(name="psum", bufs=4, space="PSUM") as psum_pool,
        ):
            ident = const_pool.tile([P, P], F32)
            make_identity(nc, ident)
            eps_sb = const_pool.tile([P, 1], F32)
            nc.vector.memset(eps_sb, float(EPS))
            dummy = const_pool.tile([P, 1], F32, name="dummy")
            nc.scalar.activation(
                dummy, eps_sb, mybir.ActivationFunctionType.Square)
            nc.scalar.activation(
                dummy, dummy, mybir.ActivationFunctionType.Sqrt)

            ut_ext = persist.tile([P, DC, MB, P + 2], BF)
            x1p = persist.tile([P, MB, D], F32)
            xrows = persist.tile([P, MB, D], BF)
            x1rows = persist.tile([P, MB, D], F32)
            yt = persist.tile([P, DC, 2, MB * P], FP8)
            gt = persist.tile([P, JC, 2, MB * P], FP8)
            mut_sb = persist.tile([P, KU, DC, D], BF)
            fc2_sb = persist.tile([P, JC, 2, D], FP8)

            def rmsnorm_to(dst, src, rows, dve_sq=False):
                """dst = src / rms(src); the rmsnorm weight is folded into
                the downstream contraction weights host-side."""
                sq = work.tile([P, D], F32, name="sq")
                ms = work.tile([P, 1], F32, name="ms")
                if dve_sq:
                    nc.vector.scalar_tensor_tensor(
                        sq[:rows], src[:rows], 1.0, src[:rows],
                        mybir.AluOpType.mult, mybir.AluOpType.mult,
                        accum_out=ms[:rows],
                    )
                else:
                    nc.scalar.activation(
                        sq[:rows], src[:rows],
                        mybir.ActivationFunctionType.Square,
                        accum_out=ms[:rows],
                    )
                nc.scalar.activation(
                    ms[:rows], ms[:rows], mybir.ActivationFunctionType.Sqrt,
                    bias=eps_sb[:rows], scale=1.0 / D,
                )
                nc.vector.reciprocal(ms[:rows], ms[:rows])
                nc.gpsimd.tensor_scalar_mul(dst, src[:rows], ms[:rows])

            # DMA queue front: prefix rows, rn1, the 4 u row blocks, then the
            # mut taps (in per-tap-half chunks so AR starts on the first),
            # then x1 rows; fc1/fc2 stream later in the fws loop
            u_pre = persist.tile([2, D], F32)
            xp = work.tile([P, D], BF, name="xt")[:2]
            nc.sync.dma_start(xp, xr[0:2, :])
            for m in range(MB):
                nc.sync.dma_start(
                    xrows[:, m, :], xr[2 + m * P: 2 + (m + 1) * P, :])
            HC = DC // 2
            for t in range(KU):
                for h in range(2):
                    nc.sync.dma_start(
                        mut_sb[:, t, h * HC:(h + 1) * HC, :],
                        mut[t, h * HC:(h + 1) * HC].rearrange("c p d -> p c d"),
                    )
            for m in range(MB):
                nc.sync.dma_start(x1rows[:, m, :], x1r[m * P:(m + 1) * P, :])

            def psum_copy(dst, src_ps, idx):
                if idx % 2 == 0:
                    nc.vector.tensor_copy(dst, src_ps)
                else:
                    nc.scalar.activation(
                        dst, src_ps, mybir.ActivationFunctionType.Copy
                    )

            # ---- u^T tiles for the AR term (rmsnorm1 + PE transpose);
            # the 2-row prefix runs after the m blocks so it stays off the
            # critical path ----
            for m in range(MB):
                uo = work.tile([P, D], F32, name="uo")
                rmsnorm_to(uo, xrows[:, m, :], P, dve_sq=m < 2)
                for c in range(DC):
                    pst = psum_pool.tile([P, D], F32, name="ps")[:, 0:P]
                    nc.tensor.transpose(pst, uo[:, c * P:(c + 1) * P], ident)
                    psum_copy(ut_ext[:, c, m, 2:P + 2], pst, c + 1)
            rmsnorm_to(u_pre, xp, 2)
            for c in range(DC):
                pst2 = psum_pool.tile([P, D], F32, name="ps")[:, 0:P]
                nc.tensor.transpose(
                    pst2[:, 0:2], u_pre[:, c * P:(c + 1) * P], ident[0:2, 0:2]
                )
                nc.vector.tensor_copy(ut_ext[:, c, 0, 0:2], pst2[:, 0:2])
            for m in range(1, MB):
                for c in range(DC):
                    nc.gpsimd.tensor_copy(
                        ut_ext[:, c, m, 0:2], ut_ext[:, c, m - 1, P:P + 2]
                    )

            # ---- AR term: all 4 row-blocks accumulate per-(tap, d-half) in
            # mut arrival order so the psum groups start on the first chunk ----
            psa = [psum_pool.tile([P, D], F32, name="ps") for _ in range(MB)]
            for t in range(KU):
                for h in range(2):
                    for m in range(MB):
                        for c in range(h * HC, (h + 1) * HC):
                            st = t == 0 and c == 0
                            fin = t == KU - 1 and c == DC - 1
                            lh = ut_ext[:, c, m, 2 - t:P + 2 - t]
                            nc.tensor.matmul(
                                psa[m][:, 0:F1], lhsT=lh,
                                rhs=mut_sb[:, t, c, 0:F1], start=st, stop=fin,
                            )
                            nc.tensor.matmul(
                                psa[m][:, F1:D], lhsT=lh,
                                rhs=mut_sb[:, t, c, F1:D], start=st, stop=fin,
                            )
            for m in range(MB):
                nc.vector.tensor_tensor(
                    x1p[:, m, :], x1rows[:, m, :], psa[m], mybir.AluOpType.add
                )

            # fc1 weight chunks + fc2/mut resident weights, in first-use order
            fws = []
            for jc in range(JC):
                fw = wstream.tile([P, DC, 2, 2, P], FP8, name="fw")
                nc.sync.dma_start(fw, fc1[jc].rearrange("c p q k f -> p c q k f"))
                fws.append(fw)
                if jc == 8:
                    nc.sync.dma_start(fc2_sb, fc2.rearrange("c p q d -> p c q d"))

            # ---- y = rmsnorm2(x1) transposed ----
            for m in range(MB):
                yf = work.tile([P, D], F32, name="uo")
                rmsnorm_to(yf, x1p[:, m, :], P)
                for c in range(DC):
                    pst = psum_pool.tile([P, D], F32, name="ps")[:, 0:P]
                    nc.tensor.transpose(pst, yf[:, c * P:(c + 1) * P], ident)
                    sl = slice(m * P, (m + 1) * P)
                    psum_copy(yt[:, c, 0, sl], pst, 1)
                    nc.vector.scalar_tensor_tensor(
                        yt[:, c, 1, sl], pst, 1.0, yt[:, c, 0, sl],
                        mybir.AluOpType.mult, mybir.AluOpType.subtract,
                    )

            # ---- fc1 + silu gate ----
            for jc in range(JC):
                ph1 = psum_pool.tile([P, D], F32, name="ps")[:, 0:F1]
                ph2 = psum_pool.tile([P, D], F32, name="ps")[:, 0:F1]
                for m in range(MB):
                    sl = slice(m * P, (m + 1) * P)
                    for hh, ph in ((0, ph1), (1, ph2)):
                        # y_hi @ w_hi, c-pairs packed in DoubleRow
                        for cp in range(DC // 2):
                            nc.tensor.matmul(
                                ph[:, sl],
                                lhsT=fws[jc][:, 2 * cp:2 * cp + 2, 1, hh, :],
                                rhs=yt[:, 2 * cp:2 * cp + 2, 0, sl],
                                start=cp == 0, stop=False, perf_mode=DR,
                            )
                        # cross terms w_lo@y_hi + w_hi@y_lo, DR-paired per c
                        for c in range(DC):
                            nc.tensor.matmul(
                                ph[:, sl],
                                lhsT=fws[jc][:, c, :, hh, :],
                                rhs=yt[:, c, :, sl],
                                start=False, stop=c == DC - 1, perf_mode=DR,
                            )
                sact = work.tile([P, F1], F32, name="sact")
                nc.scalar.activation(sact, ph2,
                                     mybir.ActivationFunctionType.Silu,
                                     scale=float(1.0 / MLP_SCALE))
                g32 = work.tile([P, F1], F32, name="g32")
                nc.vector.scalar_tensor_tensor(
                    g32, ph1, float(1.0 / MLP_SCALE), sact,
                    mybir.AluOpType.mult, mybir.AluOpType.mult,
                )
                nc.scalar.activation(
                    gt[:, jc, 0, :], g32, mybir.ActivationFunctionType.Copy
                )
                nc.vector.scalar_tensor_tensor(
                    gt[:, jc, 1, :], g32, 1.0, gt[:, jc, 0, :],
                    mybir.AluOpType.mult, mybir.AluOpType.subtract,
                )

            # ---- fc2 + residual, m-outer so each row block's residual add
            # and output DMA overlap the next block's matmuls ----
            for m in range(MB):
                po = psum_pool.tile([P, D], F32, name="ps")
                msl = slice(m * P, (m + 1) * P)
                ot = work.tile([P, D], F32, name="ot")
                for d0, d1 in ((0, F1), (F1, D)):
                    for jp in range(JC // 2):
                        lh = gt[:, 2 * jp:2 * jp + 2, 0, msl]
                        rh = fc2_sb[:, 2 * jp:2 * jp + 2, 1, d0:d1]
                        nc.tensor.matmul(po[:, d0:d1], lhsT=lh, rhs=rh,
                                         start=jp == 0, stop=False,
                                         perf_mode=DR)
                    for jc in range(JC):
                        lh = gt[:, jc, :, msl]
                        rh = fc2_sb[:, jc, :, d0:d1]
                        nc.tensor.matmul(po[:, d0:d1], lhsT=lh, rhs=rh,
                                         start=False, stop=jc == JC - 1,
                                         perf_mode=DR)
                    nc.vector.scalar_tensor_tensor(
                        ot[:, d0:d1], po[:, d0:d1], float(1.0 / MLP_SCALE),
                        x1p[:, m, d0:d1],
                        mybir.AluOpType.mult, mybir.AluOpType.add)
                    nc.sync.dma_start(o[m * P:(m + 1) * P, d0:d1],
                                      ot[:, d0:d1])
    nc.compile()
    return nc


def _host_prep(V, sigma, M_u, M_phi_plus, M_phi_minus, rn1):
    """Per-core weight tensors: Toeplitz tap blocks + projection matrices.
    rn1_w is folded into the projection's contraction axis (the rmsnorm
    weight commutes through the seq-dim conv)."""
    phi = np.fft.irfft(V.astype(np.complex128), n=NFFT, axis=0)[:SL]
    s4 = sigma.astype(np.float64) ** 0.25
    alt = (-1.0) ** np.arange(SL)

    taps = np.zeros((NBR, SL))
    Wb = np.zeros((NBR, D, D), np.float32)
    for k in range(K):
        taps[2 * k] = s4[k] * phi[:, k]
        taps[2 * k + 1] = s4[k] * phi[:, k] * alt
        Wb[2 * k] = M_phi_plus[k] * rn1[None, :]
        Wb[2 * k + 1] = M_phi_minus[k] * rn1[None, :]

    idx = np.arange(P)
    cmr = idx[None, :] - idx[:, None]       # [r, c] = c - r
    tw_cores = []
    wt_cores = []
    for core in range(NCORES):
        brs = range(core * BPC, (core + 1) * BPC)
        # tw[d0, :, ko, :] = T-block pair (delta=d0 for ko=0, delta=d0-1 for
        # ko=1, zeros for delta<0), taps scaled by TAP_SCALE for fp8 range
        tw = np.zeros((NB, P, 2, BPC * P), np.float32)
        wt = np.zeros((BPC, DC // 2, P, 2, D), np.float32)
        for bi, br in enumerate(brs):
            tsc = taps[br] * TAP_SCALE
            for d0 in range(NB):
                for ko in range(2):
                    d = d0 - ko
                    if d < 0:
                        continue
                    ii = d * P + cmr
                    blk = np.where(ii >= 0, tsc[np.clip(ii, 0, SL - 1)], 0.0)
                    tw[d0, :, ko, bi * P:(bi + 1) * P] = blk
            for cp in range(DC // 2):
                for ko in range(2):
                    c = 2 * cp + ko
                    # wt[bi, cp, i, ko, d] = Wb[br][d, c*P + i] * W_SCALE
                    wt[bi, cp, :, ko, :] = Wb[br][:, c * P:(c + 1) * P].T * W_SCALE
        tw_cores.append(tw.astype(FP8NP))
        wt_cores.append(wt.astype(FP8NP))
    return tw_cores, wt_cores


def kernel(x, V, sigma, M_u, M_phi_plus, M_phi_minus, rn1_w, rn2_w, fc1_w, fc2_w):
    x = np.ascontiguousarray(x, np.float32)
    if "p1" not in _cache:
        _cache["p1"] = _SpmdRunner(_build_phase1(), shared=("xb",), volatile=("xb",))
    if "p2" not in _cache:
        _cache["p2"] = _SpmdRunner(
            _build_phase2(), shared=("mut", "fc1", "fc2"),
            volatile=("xr", "x1r"))

    rn1 = np.ascontiguousarray(rn1_w, np.float32)
    rn2 = np.ascontiguousarray(rn2_w, np.float32)
    tw_cores, wt_cores = _host_prep(V, sigma, M_u, M_phi_plus, M_phi_minus, rn1)
    xb = x.astype(BF16NP)

    in_maps1 = [
        {"xb": xb, "tw": tw_cores[c], "wt": wt_cores[c]}
        for c in range(NCORES)
    ]
    r1 = _cache["p1"]
    sp_cat = r1.run_prepped(r1.prep(in_maps1))[0]
    if "reduce" not in _cache:
        import jax
        import jax.numpy as jnp
        from jax.sharding import NamedSharding, PartitionSpec
        sh = NamedSharding(r1._shardings["xb"].mesh, PartitionSpec())
        _cache["reduce"] = jax.jit(
            lambda spc, xx: xx + spc.reshape(NCORES, B, SL, D)
            .astype(jnp.float32).sum(0),
            out_shardings=sh,
        )
    x1 = np.asarray(_cache["reduce"](sp_cat, np.asarray(x)))

    # phase 2 inputs; rn1 folds into mut's contraction axis, rn2 into fc1's
    mut = np.zeros((KU, DC, P, D), np.float32)
    for t in range(KU):
        for c in range(DC):
            mut[t, c] = (M_u[t] * rn1[None, :])[:, c * P:(c + 1) * P].T
    mut = mut.astype(BF16NP)
    # fc1 pre-paired layout (JC, DC, P, 2, P): [..., 0, :] = y half column
    # block jc, [..., 1, :] = gate half column block jc
    f1s = np.ascontiguousarray(fc1_w, np.float32) * rn2[:, None] * MLP_SCALE
    hi8 = f1s.astype(FP8NP)
    lo8 = (f1s - hi8.astype(np.float32)).astype(FP8NP)

    def _lay(a):
        return np.transpose(a.reshape(DC, P, 2, JC, P), (3, 0, 1, 2, 4))

    # q axis: 0 = lo, 1 = hi (cross-product DR pairs lo/hi against yt's hi/lo)
    fc1p = np.ascontiguousarray(np.stack([_lay(lo8), _lay(hi8)], axis=3))
    f2s = np.ascontiguousarray(fc2_w, np.float32).reshape(JC, P, D) * MLP_SCALE
    f2hi = f2s.astype(FP8NP)
    f2lo = (f2s - f2hi.astype(np.float32)).astype(FP8NP)
    fc2 = np.ascontiguousarray(np.stack([f2lo, f2hi], axis=2))

    x_rows = x.reshape(B * SL, D)
    x1_rows = x1.reshape(B * SL, D)
    in_maps2 = []
    for c in range(NCORES):
        r0 = c * RPC
        xr = np.zeros((RPC + 2, D), np.float32)
        xr[2:] = x_rows[r0:r0 + RPC]
        if r0 % SL != 0:
            xr[0:2] = x_rows[r0 - 2:r0]
        in_maps2.append({
            "xr": xr.astype(BF16NP),
            "x1r": np.ascontiguousarray(x1_rows[r0:r0 + RPC]),
            "mut": mut, "fc1": fc1p, "fc2": fc2,
        })
    res2 = _cache["p2"](in_maps2)
    out = np.concatenate(
        [res2[c]["o"] for c in range(NCORES)], axis=0
    ).reshape(B, SL, D)
    return out



# revision 7
# speedup vs baseline: 1.0928x; 1.0928x over previous
"""Trainium2 Bass kernel for the STU (spectral transform unit) dense-transformer block.

Algorithm (validated against the jax reference in fp64 numpy):
  The FFT causal conv is rewritten as a block-Toeplitz matmul. For each of the
  K=16 filters and each sign branch (the alternating-sign branch folds into the
  filter taps: T^-[s,s'] = phi[s-s'] * (-1)^(s-s')), the causal conv is
    U_br = T_br @ u,  T_br block-Toeplitz with 16 distinct 128x128 blocks.
  sigma^(1/4) folds into the taps. The (k,i)->d projection contracts U with
  M_phi_{plus,minus}; the KU=3 autoregressive taps are shifted-u projections
  with M_u. MLP is a standard gated MLP.

Sharding (8 cores, no cross-core communication, host-side reduce between two
uniform SPMD programs):
  Phase 1: filter-branch-parallel. Core c computes conv + projection for its 4
           of the 32 (k, sign) branches over the full (B, SL): partial spectral.
  Host:    x1 = x + sum_c partial_c
  Phase 2: row-parallel. Core c owns 512 of the 4096 (b, s) rows: adds the AR
           term and computes the gated MLP + residual for its rows.

Precision: the conv runs in fp8 (output magnitude ~0.05 -> noise negligible);
fc1 and fc2 run as compensated hi+lo fp8 splits (h = yh@wh + DoubleRow-paired
cross terms wl@yh + wh@yl, dropping only the second-order yl@wl product),
which gets fp8 DoubleRow matmul rates at bf16-class accuracy. The AR term
stays bf16: its shifted u^T windows would have odd byte offsets in fp8,
which the Ldweights ISA rejects. Measured end-to-end error: 3.9e-3
scale-relative vs the 2e-2 harness gate. rn1/rn2 rmsnorm weights are folded
into the downstream contraction weights host-side (they commute through the
seq-dim conv / shifts).

Schedule notes (all targets are the InstructionCostModel timeline):
 - DMA is a serial ~360GB/s resource; transfers are emitted in the order
   compute needs them (x row-blocks and tw delta-chunks interleaved, weights
   after first-use rows), which removes the 24us/30us startup stalls the
   v1 kernel had.
 - PSUM->SBUF drain copies alternate across DVE/Act (GPSIMD cannot read
   PSUM); Pool takes the SBUF->SBUF rmsnorm multiplies.
 - Phase-1 software-pipelines the next block's conv between conv(I) and
   proj(I) (depth 2-3 for the short early blocks) so the PE covers the
   psum-drain latency; projection iterates cp-outer so its first matmuls
   depend only on the first conv psum drains.
 - Phase-1 warms the PE p-state with dummy matmuls while the first input
   blocks stream in; phase-2 finalizes fc2 m-outer and d-half-outer so each
   row block's residual add and output DMA overlap later matmuls.
 - Both phases issue dummy Square/Sqrt activations at the head of the
   Activation queue so the act-table loads finish before the first rmsnorm
   needs them.
"""

import numpy as np
import ml_dtypes

import concourse.bacc as bacc
import concourse.tile as tile
from concourse import mybir
from concourse.bass_utils import run_bass_kernel_spmd  # noqa: F401 (debug path)
from concourse.masks import make_identity


class _SpmdRunner:
    """Cached-jit SPMD executor: trace/compile once, then repeat calls only
    pay input upload + execution (mirrors bass2jax.run_bass_via_pjrt).

    ``shared`` names inputs that are identical on every core: they are fed
    replicated (host uploads one copy) instead of 8x-concatenated."""

    def __init__(self, nc, shared=(), volatile=()):
        import jax
        import concourse.mybir as _mb
        from concourse.bass2jax import (
            install_neuronx_cc_hook, _bass_exec_p, partition_id_tensor,
        )
        from jax.experimental.shard_map import shard_map
        from jax.sharding import Mesh, PartitionSpec

        install_neuronx_cc_hook()
        self.nc = nc
        assert nc.dbg_addr is None
        pid_name = (nc.partition_id_tensor.name
                    if nc.partition_id_tensor is not None else None)
        in_names, out_names, out_avals = [], [], []
        for alloc in nc.m.functions[0].allocations:
            if not isinstance(alloc, mybir.MemoryLocationSet):
                continue
            name = alloc.memorylocations[0].name
            if alloc.kind == "ExternalInput":
                if name != pid_name:
                    in_names.append(name)
            elif alloc.kind == "ExternalOutput":
                out_names.append(name)
                out_avals.append(jax.core.ShapedArray(
                    tuple(alloc.tensor_shape), mybir.dt.np(alloc.dtype)))
        self.in_names, self.out_names, self.out_avals = in_names, out_names, out_avals
        self.shared = frozenset(shared)
        self.volatile = frozenset(volatile)
        self._dev_cache = {}
        n_params = len(in_names)
        all_names = tuple(in_names + out_names)
        if pid_name is not None:
            all_names = all_names + (pid_name,)

        def _body(*args):
            args = list(args)
            if pid_name is not None:
                args.append(partition_id_tensor())
            return tuple(_bass_exec_p.bind(
                *args,
                out_avals=tuple(out_avals),
                in_names=all_names,
                out_names=tuple(out_names),
                lowering_input_output_aliases=(),
                sim_require_finite=True,
                sim_require_nnan=True,
                nc=nc,
            ))

        import jax.numpy as jnp
        from jax.sharding import NamedSharding
        devices = jax.devices()[:NCORES]
        mesh = Mesh(np.asarray(devices), ("core",))
        rep = PartitionSpec()
        core = PartitionSpec("core")
        in_specs = tuple(
            rep if nm in self.shared else core for nm in in_names
        ) + (core,) * len(out_names)
        out_specs = (core,) * len(out_names)
        donate = tuple(range(n_params, n_params + len(out_names)))
        self._fn = jax.jit(
            shard_map(_body, mesh=mesh, in_specs=in_specs, out_specs=out_specs,
                      check_rep=False),
            donate_argnums=donate, keep_unused=True,
        )
        self._zeros_fn = jax.jit(
            lambda: tuple(
                jnp.zeros((NCORES * a.shape[0], *a.shape[1:]), a.dtype)
                for a in out_avals
            ),
            out_shardings=tuple(
                NamedSharding(mesh, core) for _ in out_avals
            ),
        )
        self._shardings = {
            nm: NamedSharding(mesh, rep if nm in self.shared else core)
            for nm in in_names
        }

    def prep(self, in_maps):
        import hashlib
        import jax
        ins = []
        for nm in self.in_names:
            if nm in self.shared:
                arr = np.ascontiguousarray(in_maps[0][nm])
            else:
                arr = np.concatenate(
                    [np.asarray(in_maps[c][nm]) for c in range(NCORES)], axis=0)
            if nm in self.volatile:
                ins.append(arr)
                continue
            key = (nm, hashlib.md5(arr.tobytes()).hexdigest())
            dev = self._dev_cache.get(key)
            if dev is None:
                self._dev_cache.clear() if len(self._dev_cache) > 32 else None
                dev = jax.device_put(arr, self._shardings[nm])
                self._dev_cache[key] = dev
            ins.append(dev)
        return ins

    def run_prepped(self, ins):
        return self._fn(*ins, *self._zeros_fn())

    def __call__(self, in_maps):
        out_arrs = self.run_prepped(self.prep(in_maps))
        return [
            {nm: np.asarray(out_arrs[i]).reshape(NCORES, *self.out_avals[i].shape)[c]
             for i, nm in enumerate(self.out_names)}
            for c in range(NCORES)
        ]

BF16NP = ml_dtypes.bfloat16
FP8NP = ml_dtypes.float8_e4m3
TAP_SCALE = 1024.0
UT_SCALE = 32.0      # psum (TAP_SCALE*U) -> fp8 ut tiles scale factor: 32/1024
W_SCALE = 16.0       # projection weights scaled by 16 for fp8 range
SP_SCALE = UT_SCALE * W_SCALE  # spectral psum carries 32*16 = 512x
MLP_SCALE = 16.0     # fc1 hi/lo fp8 weights carry 16x for fp8 range
F32 = mybir.dt.float32
F32R = mybir.dt.float32r
F16 = mybir.dt.float16
BF = mybir.dt.bfloat16
FP8 = mybir.dt.float8e4

B, SL, D, K, KU = 2, 2048, 768, 16, 3
NFFT, EPS, P, H = 4096, 1e-5, 128, 3072
NB = SL // P            # 16 seq blocks
DC = D // P             # 6 d-chunks
NBR = 2 * K             # 32 conv branches
NCORES = 8
BPC = NBR // NCORES     # 4 branches per core
FPC = K // NCORES       # 2 filters per core (parity-fused conv)
NSB = SL // (2 * P)     # 8 superblocks (256 rows) per batch
RPC = (B * SL) // NCORES  # 512 rows per core
MB = RPC // P           # 4 row blocks per core in phase 2
JC = H // P             # 24 hidden chunks
F1 = 512                # free-dim split of D=768 into 512+256
DR = mybir.MatmulPerfMode.DoubleRow

_cache: dict = {}


def _build_phase1():
    """Parity-fused spectral conv: since T^- = D T D (D = alt signs), the
    even/odd half-convs C_e, C_o of each filter determine both sign branches:
      spectral[even s] = C_e Ws + C_o Wd,  spectral[odd s] = C_e Wd + C_o Ws
    with Ws = W+ + W-, Wd = W+ - W-. The conv FLOPs halve (each half-conv
    reads only half the input rows); the +- reconstruction is absorbed into
    the projection weights at no extra cost. Projection psums pack the
    same-parity rows of two consecutive 128-row blocks (conv output columns
    are emitted parity-major so the proj lhsT stays a contiguous 128-slice).
    Each core owns FPC=2 of the 16 filters."""
    nc = bacc.Bacc("TRN2", target_bir_lowering=False, debug=False, num_devices=NCORES)
    xb = nc.dram_tensor("xb", (B, SL, D), BF, kind="ExternalInput").ap()
    # tw[d0, r, ko, chain, f*128+col]: tap block pair (delta=d0-2*ko), chain
    # 0=even-input 1=odd-input, col parity-major within each filter's 128
    tw = nc.dram_tensor("tw", (NB, P, 2, 2, FPC * P), FP8, kind="ExternalInput").ap()
    # wt[f, sd(0=Ws,1=Wd), cp, r, ko, d_out]
    wt = nc.dram_tensor("wt", (FPC, 2, DC // 2, P, 2, D), FP8, kind="ExternalInput").ap()
    sp = nc.dram_tensor("sp", (B, SL, D), F16, kind="ExternalOutput").ap()

    with tile.TileContext(nc) as tc:
        with (
            tc.tile_pool(name="const", bufs=1) as const_pool,
            tc.tile_pool(name="ubuf", bufs=1) as ubuf_pool,
            tc.tile_pool(name="work", bufs=3) as work,
            tc.tile_pool(name="drain", bufs=4) as drain_pool,
            tc.tile_pool(name="spill", bufs=3) as spill_pool,
            tc.tile_pool(name="psum_u", bufs=4, space="PSUM") as psum_u_pool,
            tc.tile_pool(name="psum_sp", bufs=2, space="PSUM") as psum_sp_pool,
        ):
            eps_sb = const_pool.tile([P, 1], F32)
            nc.vector.memset(eps_sb, float(EPS))
            dummy = const_pool.tile([P, 1], F32, name="dummy")
            nc.scalar.activation(
                dummy, eps_sb, mybir.ActivationFunctionType.Square)
            nc.scalar.activation(
                dummy, dummy, mybir.ActivationFunctionType.Sqrt)
            tw_sb = const_pool.tile([P, NB, 2, 2, FPC * P], FP8)
            wt_sb = const_pool.tile([P, FPC, 2, DC // 2, 2, D], FP8)

            # one persistent fp8 u tile per (b, parity, even/odd-block pair):
            # u_t[b][par][jp][:, ko, :] holds rmsnormed rows
            # 256*(2*jp+ko) + 2r + par of batch b
            u_t = [[[ubuf_pool.tile([P, 2, D], FP8, name=f"u{b}_{par}_{jp}")
                     for jp in range(NB // 4)] for par in range(2)]
                   for b in range(B)]

            def jprep(b, par, blk, dve_sq=False):
                """strided x row DMA (one parity class) -> rmsnorm -> fp8.
                (rn1_w is folded into the projection weights host-side.)"""
                xt = work.tile([P, D], BF, name="xt")
                r0 = 256 * blk + par
                nc.sync.dma_start(xt, xb[b, r0:r0 + 255:2, :])
                sq = work.tile([P, D], F32, name="sq")
                ms = work.tile([P, 1], F32, name="ms")
                if dve_sq:
                    nc.vector.scalar_tensor_tensor(
                        sq, xt, 1.0, xt, mybir.AluOpType.mult,
                        mybir.AluOpType.mult, accum_out=ms,
                    )
                else:
                    nc.scalar.activation(
                        sq, xt, mybir.ActivationFunctionType.Square,
                        accum_out=ms,
                    )
                nc.scalar.activation(
                    ms, ms, mybir.ActivationFunctionType.Sqrt,
                    bias=eps_sb, scale=1.0 / D,
                )
                nc.vector.reciprocal(ms, ms)
                nc.gpsimd.tensor_scalar_mul(
                    u_t[b][par][blk // 2][:, blk % 2, :], xt, ms
                )

            # PE warmup: dummy matmuls on a zero tile ramp the tensor
            # engine p-state while the first input blocks stream in
            wz = const_pool.tile([P, 2, 2 * FPC * P], FP8, name="wz")
            nc.vector.memset(wz, 0.0)
            wps = psum_u_pool.tile([P, 2 * FPC * P], F32, name="psu")
            NW = 24
            for i in range(NW):
                nc.tensor.matmul(wps, lhsT=wz[:, :, 0:P], rhs=wz,
                                 start=i == 0, stop=i == NW - 1, perf_mode=DR)

            # prologue: conv block I=0 needs x rows 0..511 of both parities
            nc.sync.dma_start(tw_sb[:, 0], tw[0])
            nc.sync.dma_start(tw_sb[:, 1], tw[1])
            for blk in range(2):
                jprep(0, 0, blk, dve_sq=blk == 0)
                jprep(0, 1, blk, dve_sq=blk == 0)
            nc.sync.dma_start(
                wt_sb[:, :, :, 0, :, :],
                wt[:, :, 0].rearrange("f s p k d -> p f s k d"))
            next_blk = [2, 0]

            def conv_block(b, I, ut_sb):
                """both half-conv chains for seq block I into one psum; drain
                into the superblock ut tile (cols parity-major per filter)."""
                npair = I // 4 + 1
                half = I % 2
                for c in range(DC):
                    ps = psum_u_pool.tile([P, 2 * FPC * P], F32, name="psu")
                    for chain in range(2):
                        dst = ps[:, chain * FPC * P:(chain + 1) * FPC * P]
                        for Jp in range(npair):
                            nc.tensor.matmul(
                                dst,
                                lhsT=u_t[b][chain][Jp][:, :, c * P:(c + 1) * P],
                                rhs=tw_sb[:, I - 4 * Jp, :, chain, :],
                                start=(Jp == 0),
                                stop=(Jp == npair - 1),
                                perf_mode=DR,
                            )
                    # psum free dim = (chain, f, par, 64); ut free dim per
                    # chunk = (chain*FPC+f, par, 128=(half,64))
                    pv = ps.rearrange("p (s q j) -> p s q j", s=2 * FPC, q=2)
                    dst = ut_sb[:, c, :, :, half * 64:half * 64 + 64]
                    if c % 2 == 0:
                        nc.vector.tensor_scalar_mul(
                            dst, pv, float(UT_SCALE / TAP_SCALE))
                    else:
                        nc.scalar.activation(
                            dst, pv, mybir.ActivationFunctionType.Copy,
                            scale=float(UT_SCALE / TAP_SCALE),
                        )

            def proj_block(b, Ip, par, ut_sb, last=False):
                """one parity's projection for superblock Ip: 128 same-parity
                rows of blocks (2Ip, 2Ip+1); sd picks Ws for the matching
                parity chain, Wd for the crossed one."""
                psp = psum_sp_pool.tile([P, D], F32, name="psp")
                i_mm = 0
                for cp in range(DC // 2):
                    for st_i in range(2 * FPC):
                        chain, f = divmod(st_i, FPC)
                        sd = par if chain == 0 else 1 - par
                        st = i_mm == 0
                        fin = i_mm == 2 * FPC * (DC // 2) - 1
                        lh = ut_sb[:, 2 * cp:2 * cp + 2, st_i, par, :]
                        nc.tensor.matmul(
                            psp[:, 0:F1], lhsT=lh,
                            rhs=wt_sb[:, f, sd, cp, :, 0:F1],
                            start=st, stop=fin, perf_mode=DR,
                        )
                        nc.tensor.matmul(
                            psp[:, F1:D], lhsT=lh,
                            rhs=wt_sb[:, f, sd, cp, :, F1:D],
                            start=st, stop=fin, perf_mode=DR,
                        )
                        i_mm += 1
                sp_t = spill_pool.tile([P, D], F16, name="spt")
                r0 = 256 * Ip + par
                if last:
                    nc.scalar.activation(
                        sp_t[:, 0:F1], psp[:, 0:F1],
                        mybir.ActivationFunctionType.Copy,
                        scale=float(1.0 / SP_SCALE),
                    )
                    nc.sync.dma_start(
                        sp[b, r0:r0 + 255:2, 0:F1], sp_t[:, 0:F1])
                    nc.vector.tensor_scalar_mul(
                        sp_t[:, F1:D], psp[:, F1:D], float(1.0 / SP_SCALE))
                    nc.sync.dma_start(
                        sp[b, r0:r0 + 255:2, F1:D], sp_t[:, F1:D])
                elif par == 0:
                    nc.scalar.activation(
                        sp_t, psp, mybir.ActivationFunctionType.Copy,
                        scale=float(1.0 / SP_SCALE),
                    )
                    nc.sync.dma_start(sp[b, r0:r0 + 255:2, :], sp_t)
                else:
                    nc.vector.tensor_scalar_mul(sp_t, psp, float(1.0 / SP_SCALE))
                    nc.sync.dma_start(sp[b, r0:r0 + 255:2, :], sp_t)

            # software pipeline: emit convs ahead of projections so the PE
            # covers psum-drain latency; a superblock's proj needs both its
            # conv blocks drained
            from collections import deque
            pend = deque()
            for b in range(B):
                for Ip in range(NSB):
                    ut_sb = drain_pool.tile([P, DC, 2 * FPC, 2, P], FP8,
                                            name="ut")
                    for half in range(2):
                        I = 2 * Ip + half
                        # pace the DMA queue: tw chunk I+2, remaining wt
                        # chunks, upcoming u row blocks
                        if b == 0 and I + 2 < NB:
                            nc.sync.dma_start(tw_sb[:, I + 2], tw[I + 2])
                        if b == 0 and 1 <= I < 3:
                            cp = I
                            nc.sync.dma_start(
                                wt_sb[:, :, :, cp, :, :],
                                wt[:, :, cp].rearrange(
                                    "f s p k d -> p f s k d"))
                        need = min(2 * (((I + 4) // 4)) + 1, NB // 2 - 1)
                        while next_blk[b] <= need:
                            jprep(b, 0, next_blk[b])
                            jprep(b, 1, next_blk[b])
                            next_blk[b] += 1
                        if b == 0 and I >= 10:
                            while next_blk[1] <= min(I - 10, NB // 2 - 1):
                                jprep(1, 0, next_blk[1])
                                jprep(1, 1, next_blk[1])
                                next_blk[1] += 1
                        conv_block(b, I, ut_sb)
                    pend.append((b, Ip, ut_sb))
                    depth = 2 if Ip < 3 else 1
                    while len(pend) > depth:
                        pb, pIp, put = pend.popleft()
                        proj_block(pb, pIp, 0, put)
                        proj_block(pb, pIp, 1, put)
            while pend:
                pb, pIp, put = pend.popleft()
                proj_block(pb, pIp, 0, put)
                proj_block(pb, pIp, 1, put,
                           last=pb == B - 1 and pIp == NSB - 1)
    nc.compile()
    return nc


def _build_phase2():
    nc = bacc.Bacc("TRN2", target_bir_lowering=False, debug=False, num_devices=NCORES)
    xr = nc.dram_tensor("xr", (RPC + 2, D), BF, kind="ExternalInput").ap()
    x1r = nc.dram_tensor("x1r", (RPC, D), F32, kind="ExternalInput").ap()
    mut = nc.dram_tensor("mut", (KU, DC, P, D), BF, kind="ExternalInput").ap()
    fc1 = nc.dram_tensor("fc1", (JC, DC, P, 2, 2, P), FP8, kind="ExternalInput").ap()
    fc2 = nc.dram_tensor("fc2", (JC, P, 2, D), FP8, kind="ExternalInput").ap()
    o = nc.dram_tensor("o", (RPC, D), F32, kind="ExternalOutput").ap()


    with tile.TileContext(nc) as tc:
        with (
            tc.tile_pool(name="const", bufs=1) as const_pool,
            tc.tile_pool(name="persist", bufs=1) as persist,
            tc.tile_pool(name="work", bufs=3) as work,
            tc.tile_pool(name="wstream", bufs=3) as wstream,
            tc.tile_pool(name="psum", bufs=4, space="PSUM") as psum_pool,
        ):
            ident = const_pool.tile([P, P], F32)
            make_identity(nc, ident)
            eps_sb = const_pool.tile([P, 1], F32)
            nc.vector.memset(eps_sb, float(EPS))
            dummy = const_pool.tile([P, 1], F32, name="dummy")
            nc.scalar.activation(
                dummy, eps_sb, mybir.ActivationFunctionType.Square)
            nc.scalar.activation(
                dummy, dummy, mybir.ActivationFunctionType.Sqrt)

            ut_ext = persist.tile([P, DC, MB, P + 2], BF)
            x1p = persist.tile([P, MB, D], F32)
            xrows = persist.tile([P, MB, D], BF)
            x1rows = persist.tile([P, MB, D], F32)
            yt = persist.tile([P, DC, 2, MB * P], FP8)
            gt = persist.tile([P, JC, 2, MB * P], FP8)
            mut_sb = persist.tile([P, KU, DC, D], BF)
            fc2_sb = persist.tile([P, JC, 2, D], FP8)

            def rmsnorm_to(dst, src, rows, dve_sq=False):
                """dst = src / rms(src); the rmsnorm weight is folded into
                the downstream contraction weights host-side."""
                sq = work.tile([P, D], F32, name="sq")
                ms = work.tile([P, 1], F32, name="ms")
                if dve_sq:
                    nc.vector.scalar_tensor_tensor(
                        sq[:rows], src[:rows], 1.0, src[:rows],
                        mybir.AluOpType.mult, mybir.AluOpType.mult,
                        accum_out=ms[:rows],
                    )
                else:
                    nc.scalar.activation(
                        sq[:rows], src[:rows],
                        mybir.ActivationFunctionType.Square,
                        accum_out=ms[:rows],
                    )
                nc.scalar.activation(
                    ms[:rows], ms[:rows], mybir.ActivationFunctionType.Sqrt,
                    bias=eps_sb[:rows], scale=1.0 / D,
                )
                nc.vector.reciprocal(ms[:rows], ms[:rows])
                nc.gpsimd.tensor_scalar_mul(dst, src[:rows], ms[:rows])

            # DMA queue front: prefix rows, rn1, the 4 u row blocks, then the
            # mut taps (in per-tap-half chunks so AR starts on the first),
            # then x1 rows; fc1/fc2 stream later in the fws loop
            u_pre = persist.tile([2, D], F32)
            xp = work.tile([P, D], BF, name="xt")[:2]
            nc.sync.dma_start(xp, xr[0:2, :])
            for m in range(MB):
                nc.sync.dma_start(
                    xrows[:, m, :], xr[2 + m * P: 2 + (m + 1) * P, :])
            HC = DC // 2
            for t in range(KU):
                for h in range(2):
                    nc.sync.dma_start(
                        mut_sb[:, t, h * HC:(h + 1) * HC, :],
                        mut[t, h * HC:(h + 1) * HC].rearrange("c p d -> p c d"),
                    )
            for m in range(MB):
                nc.sync.dma_start(x1rows[:, m, :], x1r[m * P:(m + 1) * P, :])

            def psum_copy(dst, src_ps, idx):
                if idx % 2 == 0:
                    nc.vector.tensor_copy(dst, src_ps)
                else:
                    nc.scalar.activation(
                        dst, src_ps, mybir.ActivationFunctionType.Copy
                    )

            # ---- u^T tiles for the AR term (rmsnorm1 + PE transpose);
            # the 2-row prefix runs after the m blocks so it stays off the
            # critical path ----
            for m in range(MB):
                uo = work.tile([P, D], F32, name="uo")
                rmsnorm_to(uo, xrows[:, m, :], P, dve_sq=m < 2)
                for c in range(DC):
                    pst = psum_pool.tile([P, D], F32, name="ps")[:, 0:P]
                    nc.tensor.transpose(pst, uo[:, c * P:(c + 1) * P], ident)
                    psum_copy(ut_ext[:, c, m, 2:P + 2], pst, c + 1)
            rmsnorm_to(u_pre, xp, 2)
            for c in range(DC):
                pst2 = psum_pool.tile([P, D], F32, name="ps")[:, 0:P]
                nc.tensor.transpose(
                    pst2[:, 0:2], u_pre[:, c * P:(c + 1) * P], ident[0:2, 0:2]
                )
                nc.vector.tensor_copy(ut_ext[:, c, 0, 0:2], pst2[:, 0:2])
            for m in range(1, MB):
                for c in range(DC):
                    nc.gpsimd.tensor_copy(
                        ut_ext[:, c, m, 0:2], ut_ext[:, c, m - 1, P:P + 2]
                    )

            # ---- AR term: all 4 row-blocks accumulate per-(tap, d-half) in
            # mut arrival order so the psum groups start on the first chunk ----
            psa = [psum_pool.tile([P, D], F32, name="ps") for _ in range(MB)]
            for t in range(KU):
                for h in range(2):
                    for m in range(MB):
                        for c in range(h * HC, (h + 1) * HC):
                            st = t == 0 and c == 0
                            fin = t == KU - 1 and c == DC - 1
                            lh = ut_ext[:, c, m, 2 - t:P + 2 - t]
                            nc.tensor.matmul(
                                psa[m][:, 0:F1], lhsT=lh,
                                rhs=mut_sb[:, t, c, 0:F1], start=st, stop=fin,
                            )
                            nc.tensor.matmul(
                                psa[m][:, F1:D], lhsT=lh,
                                rhs=mut_sb[:, t, c, F1:D], start=st, stop=fin,
                            )
            for m in range(MB):
                nc.vector.tensor_tensor(
                    x1p[:, m, :], x1rows[:, m, :], psa[m], mybir.AluOpType.add
                )

            # fc1 weight chunks + fc2/mut resident weights, in first-use order
            fws = []
            for jc in range(JC):
                fw = wstream.tile([P, DC, 2, 2, P], FP8, name="fw")
                nc.sync.dma_start(fw, fc1[jc].rearrange("c p q k f -> p c q k f"))
                fws.append(fw)
                if jc == 8:
                    nc.sync.dma_start(fc2_sb, fc2.rearrange("c p q d -> p c q d"))

            # ---- y = rmsnorm2(x1) transposed ----
            for m in range(MB):
                yf = work.tile([P, D], F32, name="uo")
                rmsnorm_to(yf, x1p[:, m, :], P)
                for c in range(DC):
                    pst = psum_pool.tile([P, D], F32, name="ps")[:, 0:P]
                    nc.tensor.transpose(pst, yf[:, c * P:(c + 1) * P], ident)
                    sl = slice(m * P, (m + 1) * P)
                    psum_copy(yt[:, c, 0, sl], pst, 1)
                    nc.vector.scalar_tensor_tensor(
                        yt[:, c, 1, sl], pst, 1.0, yt[:, c, 0, sl],
                        mybir.AluOpType.mult, mybir.AluOpType.subtract,
                    )

            # ---- fc1 + silu gate ----
            for jc in range(JC):
                ph1 = psum_pool.tile([P, D], F32, name="ps")[:, 0:F1]
                ph2 = psum_pool.tile([P, D], F32, name="ps")[:, 0:F1]
                for m in range(MB):
                    sl = slice(m * P, (m + 1) * P)
                    for hh, ph in ((0, ph1), (1, ph2)):
                        # y_hi @ w_hi, c-pairs packed in DoubleRow
                        for cp in range(DC // 2):
                            nc.tensor.matmul(
                                ph[:, sl],
                                lhsT=fws[jc][:, 2 * cp:2 * cp + 2, 1, hh, :],
                                rhs=yt[:, 2 * cp:2 * cp + 2, 0, sl],
                                start=cp == 0, stop=False, perf_mode=DR,
                            )
                        # cross terms w_lo@y_hi + w_hi@y_lo, DR-paired per c
                        for c in range(DC):
                            nc.tensor.matmul(
                                ph[:, sl],
                                lhsT=fws[jc][:, c, :, hh, :],
                                rhs=yt[:, c, :, sl],
                                start=False, stop=c == DC - 1, perf_mode=DR,
                            )
                sact = work.tile([P, F1], F32, name="sact")
                nc.scalar.activation(sact, ph2,
                                     mybir.ActivationFunctionType.Silu,
                                     scale=float(1.0 / MLP_SCALE))
                g32 = work.tile([P, F1], F32, name="g32")
                nc.vector.scalar_tensor_tensor(
                    g32, ph1, float(1.0 / MLP_SCALE), sact,
                    mybir.AluOpType.mult, mybir.AluOpType.mult,
                )
                nc.scalar.activation(
                    gt[:, jc, 0, :], g32, mybir.ActivationFunctionType.Copy
                )
                nc.vector.scalar_tensor_tensor(
                    gt[:, jc, 1, :], g32, 1.0, gt[:, jc, 0, :],
                    mybir.AluOpType.mult, mybir.AluOpType.subtract,
                )

            # ---- fc2 + residual, m-outer so each row block's residual add
            # and output DMA overlap the next block's matmuls ----
            for m in range(MB):
                po = psum_pool.tile([P, D], F32, name="ps")
                msl = slice(m * P, (m + 1) * P)
                ot = work.tile([P, D], F32, name="ot")
                for d0, d1 in ((0, F1), (F1, D)):
                    for jp in range(JC // 2):
                        lh = gt[:, 2 * jp:2 * jp + 2, 0, msl]
                        rh = fc2_sb[:, 2 * jp:2 * jp + 2, 1, d0:d1]
                        nc.tensor.matmul(po[:, d0:d1], lhsT=lh, rhs=rh,
                                         start=jp == 0, stop=False,
                                         perf_mode=DR)
                    for jc in range(JC):
                        lh = gt[:, jc, :, msl]
                        rh = fc2_sb[:, jc, :, d0:d1]
                        nc.tensor.matmul(po[:, d0:d1], lhsT=lh, rhs=rh,
                                         start=False, stop=jc == JC - 1,
                                         perf_mode=DR)
                    nc.vector.scalar_tensor_tensor(
                        ot[:, d0:d1], po[:, d0:d1], float(1.0 / MLP_SCALE),
                        x1p[:, m, d0:d1],
                        mybir.AluOpType.mult, mybir.AluOpType.add)
                    nc.sync.dma_start(o[m * P:(m + 1) * P, d0:d1],
                                      ot[:, d0:d1])
    nc.compile()
    return nc


def _host_prep(V, sigma, M_u, M_phi_plus, M_phi_minus, rn1):
    """Per-core tap blocks + fused projection matrices for the parity-split
    conv. Core c owns filters (2c, 2c+1). rn1_w folds into the projection's
    contraction axis; sigma^0.25 folds into the taps.

    tw[d0, r, ko, chain, f*128 + po*64 + j] =
        taps_f[128*(d0 - 2*ko) + (2j + po) - 2r - chain]   (0 if idx < 0)
    (chain 0 reads even input rows, 1 odd; output cols parity-major po)
    wt[f, sd, cp, i, ko, d] = {Ws,Wd}[f][d, (2cp+ko)*128 + i] * W_SCALE
    """
    phi = np.fft.irfft(V.astype(np.complex128), n=NFFT, axis=0)[:SL]
    s4 = sigma.astype(np.float64) ** 0.25
    taps = (s4[None, :] * phi)                       # (SL, K)
    Ws = (M_phi_plus + M_phi_minus) * rn1[None, None, :]
    Wd = (M_phi_plus - M_phi_minus) * rn1[None, None, :]

    # col -> s_out map (parity-major)
    s_out = np.empty(P, np.int64)
    s_out[:64] = 2 * np.arange(64)
    s_out[64:] = 2 * np.arange(64) + 1

    d0v = np.arange(NB)[:, None, None, None, None]
    rv = np.arange(P)[None, :, None, None, None]
    kov = np.arange(2)[None, None, :, None, None]
    chv = np.arange(2)[None, None, None, :, None]
    colv = s_out[None, None, None, None, :]
    idx = 128 * (d0v - 2 * kov) + colv - 2 * rv - chv   # (NB,P,2,2,P)

    tw_cores = []
    wt_cores = []
    for core in range(NCORES):
        tw = np.zeros((NB, P, 2, 2, FPC * P), np.float32)
        wt = np.zeros((FPC, 2, DC // 2, P, 2, D), np.float32)
        for f in range(FPC):
            k = core * FPC + f
            tsc = (taps[:, k] * TAP_SCALE).astype(np.float64)
            blk = np.where(idx >= 0, tsc[np.clip(idx, 0, SL - 1)], 0.0)
            tw[:, :, :, :, f * P:(f + 1) * P] = blk
            for sd, W in ((0, Ws[k]), (1, Wd[k])):
                for cp in range(DC // 2):
                    for ko in range(2):
                        c = 2 * cp + ko
                        wt[f, sd, cp, :, ko, :] = (
                            W[:, c * P:(c + 1) * P].T * W_SCALE)
        tw_cores.append(tw.astype(FP8NP))
        wt_cores.append(wt.astype(FP8NP))
    return tw_cores, wt_cores


def kernel(x, V, sigma, M_u, M_phi_plus, M_phi_minus, rn1_w, rn2_w, fc1_w, fc2_w):
    x = np.ascontiguousarray(x, np.float32)
    if "p1" not in _cache:
        _cache["p1"] = _SpmdRunner(_build_phase1(), shared=("xb",), volatile=("xb",))
    if "p2" not in _cache:
        _cache["p2"] = _SpmdRunner(
            _build_phase2(), shared=("mut", "fc1", "fc2"),
            volatile=("xr", "x1r"))

    rn1 = np.ascontiguousarray(rn1_w, np.float32)
    rn2 = np.ascontiguousarray(rn2_w, np.float32)
    tw_cores, wt_cores = _host_prep(V, sigma, M_u, M_phi_plus, M_phi_minus, rn1)
    xb = x.astype(BF16NP)

    in_maps1 = [
        {"xb": xb, "tw": tw_cores[c], "wt": wt_cores[c]}
        for c in range(NCORES)
    ]
    r1 = _cache["p1"]
    sp_cat = r1.run_prepped(r1.prep(in_maps1))[0]
    if "reduce" not in _cache:
        import jax
        import jax.numpy as jnp
        from jax.sharding import NamedSharding, PartitionSpec
        sh = NamedSharding(r1._shardings["xb"].mesh, PartitionSpec())
        _cache["reduce"] = jax.jit(
            lambda spc, xx: xx + spc.reshape(NCORES, B, SL, D)
            .astype(jnp.float32).sum(0),
            out_shardings=sh,
        )
    x1 = np.asarray(_cache["reduce"](sp_cat, np.asarray(x)))

    # phase 2 inputs; rn1 folds into mut's contraction axis, rn2 into fc1's
    mut = np.zeros((KU, DC, P, D), np.float32)
    for t in range(KU):
        for c in range(DC):
            mut[t, c] = (M_u[t] * rn1[None, :])[:, c * P:(c + 1) * P].T
    mut = mut.astype(BF16NP)
    # fc1 pre-paired layout (JC, DC, P, 2, P): [..., 0, :] = y half column
    # block jc, [..., 1, :] = gate half column block jc
    f1s = np.ascontiguousarray(fc1_w, np.float32) * rn2[:, None] * MLP_SCALE
    hi8 = f1s.astype(FP8NP)
    lo8 = (f1s - hi8.astype(np.float32)).astype(FP8NP)

    def _lay(a):
        return np.transpose(a.reshape(DC, P, 2, JC, P), (3, 0, 1, 2, 4))

    # q axis: 0 = lo, 1 = hi (cross-product DR pairs lo/hi against yt's hi/lo)
    fc1p = np.ascontiguousarray(np.stack([_lay(lo8), _lay(hi8)], axis=3))
    f2s = np.ascontiguousarray(fc2_w, np.float32).reshape(JC, P, D) * MLP_SCALE
    f2hi = f2s.astype(FP8NP)
    f2lo = (f2s - f2hi.astype(np.float32)).astype(FP8NP)
    fc2 = np.ascontiguousarray(np.stack([f2lo, f2hi], axis=2))

    x_rows = x.reshape(B * SL, D)
    x1_rows = x1.reshape(B * SL, D)
    in_maps2 = []
    for c in range(NCORES):
        r0 = c * RPC
        xr = np.zeros((RPC + 2, D), np.float32)
        xr[2:] = x_rows[r0:r0 + RPC]
        if r0 % SL != 0:
            xr[0:2] = x_rows[r0 - 2:r0]
        in_maps2.append({
            "xr": xr.astype(BF16NP),
            "x1r": np.ascontiguousarray(x1_rows[r0:r0 + RPC]),
            "mut": mut, "fc1": fc1p, "fc2": fc2,
        })
    res2 = _cache["p2"](in_maps2)
    out = np.concatenate(
        [res2[c]["o"] for c in range(NCORES)], axis=0
    ).reshape(B, SL, D)
    return out



# revision 23
# speedup vs baseline: 1.1074x; 1.0133x over previous
"""Trainium2 Bass kernel for the STU (spectral transform unit) dense-transformer block.

Algorithm (validated against the jax reference in fp64 numpy):
  The FFT causal conv is rewritten as a block-Toeplitz matmul. For each of the
  K=16 filters and each sign branch (the alternating-sign branch folds into the
  filter taps: T^-[s,s'] = phi[s-s'] * (-1)^(s-s')), the causal conv is
    U_br = T_br @ u,  T_br block-Toeplitz with 16 distinct 128x128 blocks.
  sigma^(1/4) folds into the taps. The (k,i)->d projection contracts U with
  M_phi_{plus,minus}; the KU=3 autoregressive taps are shifted-u projections
  with M_u. MLP is a standard gated MLP.

Sharding (8 cores, no cross-core communication, host-side reduce between two
uniform SPMD programs):
  Phase 1: filter-branch-parallel. Core c computes conv + projection for its 4
           of the 32 (k, sign) branches over the full (B, SL): partial spectral.
  Host:    x1 = x + sum_c partial_c
  Phase 2: row-parallel. Core c owns 512 of the 4096 (b, s) rows: adds the AR
           term and computes the gated MLP + residual for its rows.

Precision: the conv runs in fp8 (output magnitude ~0.05 -> noise negligible);
fc1 and fc2 run as compensated hi+lo fp8 splits (h = yh@wh + DoubleRow-paired
cross terms wl@yh + wh@yl, dropping only the second-order yl@wl product),
which gets fp8 DoubleRow matmul rates at bf16-class accuracy. The AR term
stays bf16: its shifted u^T windows would have odd byte offsets in fp8,
which the Ldweights ISA rejects. Measured end-to-end error: 3.9e-3
scale-relative vs the 2e-2 harness gate. rn1/rn2 rmsnorm weights are folded
into the downstream contraction weights host-side (they commute through the
seq-dim conv / shifts).

Schedule notes (all targets are the InstructionCostModel timeline):
 - DMA is a serial ~360GB/s resource; transfers are emitted in the order
   compute needs them (x row-blocks and tw delta-chunks interleaved, weights
   after first-use rows), which removes the 24us/30us startup stalls the
   v1 kernel had.
 - PSUM->SBUF drain copies alternate across DVE/Act (GPSIMD cannot read
   PSUM); Pool takes the SBUF->SBUF rmsnorm multiplies.
 - Phase-1 software-pipelines the next block's conv between conv(I) and
   proj(I) (depth 2-3 for the short early blocks) so the PE covers the
   psum-drain latency; projection iterates cp-outer so its first matmuls
   depend only on the first conv psum drains.
 - Phase-1 warms the PE p-state with dummy matmuls while the first input
   blocks stream in; phase-2 finalizes fc2 m-outer and d-half-outer so each
   row block's residual add and output DMA overlap later matmuls.
 - Both phases issue dummy Square/Sqrt activations at the head of the
   Activation queue so the act-table loads finish before the first rmsnorm
   needs them.
"""

import numpy as np
import ml_dtypes

import concourse.bacc as bacc
import concourse.tile as tile
from concourse import mybir
from concourse.bass_utils import run_bass_kernel_spmd  # noqa: F401 (debug path)
from concourse.masks import make_identity


class _SpmdRunner:
    """Cached-jit SPMD executor: trace/compile once, then repeat calls only
    pay input upload + execution (mirrors bass2jax.run_bass_via_pjrt).

    ``shared`` names inputs that are identical on every core: they are fed
    replicated (host uploads one copy) instead of 8x-concatenated."""

    def __init__(self, nc, shared=(), volatile=()):
        import jax
        import concourse.mybir as _mb
        from concourse.bass2jax import (
            install_neuronx_cc_hook, _bass_exec_p, partition_id_tensor,
        )
        from jax.experimental.shard_map import shard_map
        from jax.sharding import Mesh, PartitionSpec

        install_neuronx_cc_hook()
        self.nc = nc
        assert nc.dbg_addr is None
        pid_name = (nc.partition_id_tensor.name
                    if nc.partition_id_tensor is not None else None)
        in_names, out_names, out_avals = [], [], []
        for alloc in nc.m.functions[0].allocations:
            if not isinstance(alloc, mybir.MemoryLocationSet):
                continue
            name = alloc.memorylocations[0].name
            if alloc.kind == "ExternalInput":
                if name != pid_name:
                    in_names.append(name)
            elif alloc.kind == "ExternalOutput":
                out_names.append(name)
                out_avals.append(jax.core.ShapedArray(
                    tuple(alloc.tensor_shape), mybir.dt.np(alloc.dtype)))
        self.in_names, self.out_names, self.out_avals = in_names, out_names, out_avals
        self.shared = frozenset(shared)
        self.volatile = frozenset(volatile)
        self._dev_cache = {}
        n_params = len(in_names)
        all_names = tuple(in_names + out_names)
        if pid_name is not None:
            all_names = all_names + (pid_name,)

        def _body(*args):
            args = list(args)
            if pid_name is not None:
                args.append(partition_id_tensor())
            return tuple(_bass_exec_p.bind(
                *args,
                out_avals=tuple(out_avals),
                in_names=all_names,
                out_names=tuple(out_names),
                lowering_input_output_aliases=(),
                sim_require_finite=True,
                sim_require_nnan=True,
                nc=nc,
            ))

        import jax.numpy as jnp
        from jax.sharding import NamedSharding
        devices = jax.devices()[:NCORES]
        mesh = Mesh(np.asarray(devices), ("core",))
        rep = PartitionSpec()
        core = PartitionSpec("core")
        in_specs = tuple(
            rep if nm in self.shared else core for nm in in_names
        ) + (core,) * len(out_names)
        out_specs = (core,) * len(out_names)
        donate = tuple(range(n_params, n_params + len(out_names)))
        self._fn = jax.jit(
            shard_map(_body, mesh=mesh, in_specs=in_specs, out_specs=out_specs,
                      check_rep=False),
            donate_argnums=donate, keep_unused=True,
        )
        self._zeros_fn = jax.jit(
            lambda: tuple(
                jnp.zeros((NCORES * a.shape[0], *a.shape[1:]), a.dtype)
                for a in out_avals
            ),
            out_shardings=tuple(
                NamedSharding(mesh, core) for _ in out_avals
            ),
        )
        self._shardings = {
            nm: NamedSharding(mesh, rep if nm in self.shared else core)
            for nm in in_names
        }

    def prep(self, in_maps):
        import hashlib
        import jax
        ins = []
        for nm in self.in_names:
            if nm in self.shared:
                arr = np.ascontiguousarray(in_maps[0][nm])
            else:
                arr = np.concatenate(
                    [np.asarray(in_maps[c][nm]) for c in range(NCORES)], axis=0)
            if nm in self.volatile:
                ins.append(arr)
                continue
            key = (nm, hashlib.md5(arr.tobytes()).hexdigest())
            dev = self._dev_cache.get(key)
            if dev is None:
                self._dev_cache.clear() if len(self._dev_cache) > 32 else None
                dev = jax.device_put(arr, self._shardings[nm])
                self._dev_cache[key] = dev
            ins.append(dev)
        return ins

    def run_prepped(self, ins):
        return self._fn(*ins, *self._zeros_fn())

    def __call__(self, in_maps):
        out_arrs = self.run_prepped(self.prep(in_maps))
        return [
            {nm: np.asarray(out_arrs[i]).reshape(NCORES, *self.out_avals[i].shape)[c]
             for i, nm in enumerate(self.out_names)}
            for c in range(NCORES)
        ]

BF16NP = ml_dtypes.bfloat16
FP8NP = ml_dtypes.float8_e4m3
TAP_SCALE = 1024.0
UT_SCALE = 32.0      # psum (TAP_SCALE*U) -> fp8 ut tiles scale factor: 32/1024
W_SCALE = 16.0       # projection weights scaled by 16 for fp8 range
SP_SCALE = UT_SCALE * W_SCALE  # spectral psum carries 32*16 = 512x
MLP_SCALE = 16.0     # fc1 hi/lo fp8 weights carry 16x for fp8 range
F32 = mybir.dt.float32
F32R = mybir.dt.float32r
F16 = mybir.dt.float16
BF = mybir.dt.bfloat16
FP8 = mybir.dt.float8e4

B, SL, D, K, KU = 2, 2048, 768, 16, 3
NFFT, EPS, P, H = 4096, 1e-5, 128, 3072
NB = SL // P            # 16 seq blocks
DC = D // P             # 6 d-chunks
NBR = 2 * K             # 32 conv branches
NCORES = 8
BPC = NBR // NCORES     # 4 branches per core
FPC = K // NCORES       # 2 filters per core (parity-fused conv)
NSB = SL // (2 * P)     # 8 superblocks (256 rows) per batch
RPC = (B * SL) // NCORES  # 512 rows per core
MB = RPC // P           # 4 row blocks per core in phase 2
JC = H // P             # 24 hidden chunks
F1 = 512                # free-dim split of D=768 into 512+256
DR = mybir.MatmulPerfMode.DoubleRow

_cache: dict = {}


def _build_phase1():
    """Parity-fused spectral conv: since T^- = D T D (D = alt signs), the
    even/odd half-convs C_e, C_o of each filter determine both sign branches:
      spectral[even s] = C_e Ws + C_o Wd,  spectral[odd s] = C_e Wd + C_o Ws
    with Ws = W+ + W-, Wd = W+ - W-. The conv FLOPs halve (each half-conv
    reads only half the input rows); the +- reconstruction is absorbed into
    the projection weights at no extra cost. Projection psums pack the
    same-parity rows of two consecutive 128-row blocks (conv output columns
    are emitted parity-major so the proj lhsT stays a contiguous 128-slice).
    Each core owns FPC=2 of the 16 filters."""
    nc = bacc.Bacc("TRN2", target_bir_lowering=False, debug=False, num_devices=NCORES)
    xb = nc.dram_tensor("xb", (B, SL, D), BF, kind="ExternalInput").ap()
    # tw[d0, r, ko, chain, f*128+col]: tap block pair (delta=d0-2*ko), chain
    # 0=even-input 1=odd-input, col parity-major within each filter's 128
    tw = nc.dram_tensor("tw", (NB, P, 2, 2, FPC * P), FP8, kind="ExternalInput").ap()
    # wt[f, sd(0=Ws,1=Wd), cp, r, ko, d_out]
    wt = nc.dram_tensor("wt", (FPC, 2, DC // 2, P, 2, D), FP8, kind="ExternalInput").ap()
    sp = nc.dram_tensor("sp", (B, SL, D), F16, kind="ExternalOutput").ap()

    with tile.TileContext(nc) as tc:
        with (
            tc.tile_pool(name="const", bufs=1) as const_pool,
            tc.tile_pool(name="ubuf", bufs=1) as ubuf_pool,
            tc.tile_pool(name="work", bufs=3) as work,
            tc.tile_pool(name="drain", bufs=4) as drain_pool,
            tc.tile_pool(name="spill", bufs=3) as spill_pool,
            tc.tile_pool(name="psum_u", bufs=4, space="PSUM") as psum_u_pool,
            tc.tile_pool(name="psum_sp", bufs=2, space="PSUM") as psum_sp_pool,
        ):
            eps_sb = const_pool.tile([P, 1], F32)
            nc.vector.memset(eps_sb, float(EPS))
            dummy = const_pool.tile([P, 1], F32, name="dummy")
            nc.scalar.activation(
                dummy, eps_sb, mybir.ActivationFunctionType.Square)
            nc.scalar.activation(
                dummy, dummy, mybir.ActivationFunctionType.Sqrt)
            tw_sb = const_pool.tile([P, NB, 2, 2, FPC * P], FP8)
            wt_sb = const_pool.tile([P, FPC, 2, DC // 2, 2, D], FP8)

            # one persistent fp8 u tile per (b, parity, even/odd-block pair):
            # u_t[b][par][jp][:, ko, :] holds rmsnormed rows
            # 256*(2*jp+ko) + 2r + par of batch b
            u_t = [[[ubuf_pool.tile([P, 2, D], FP8, name=f"u{b}_{par}_{jp}")
                     for jp in range(NB // 4)] for par in range(2)]
                   for b in range(B)]

            def jprep(b, par, blk, dve_sq=False, seng=0):
                """strided x row DMA (one parity class) -> rmsnorm -> fp8.
                (rn1_w is folded into the projection weights host-side.)
                seng: engine for the final scale (0=Pool, 1=DVE, 2=Act) --
                the head jpreps fan out so the Pool queue doesn't serialize
                the first conv's inputs."""
                xt = work.tile([P, D], BF, name="xt")
                r0 = 256 * blk + par
                nc.sync.dma_start(xt, xb[b, r0:r0 + 255:2, :])
                sq = work.tile([P, D], F32, name="sq")
                ms = work.tile([P, 1], F32, name="ms")
                if dve_sq:
                    nc.vector.scalar_tensor_tensor(
                        sq, xt, 1.0, xt, mybir.AluOpType.mult,
                        mybir.AluOpType.mult, accum_out=ms,
                    )
                else:
                    nc.scalar.activation(
                        sq, xt, mybir.ActivationFunctionType.Square,
                        accum_out=ms,
                    )
                nc.scalar.activation(
                    ms, ms, mybir.ActivationFunctionType.Sqrt,
                    bias=eps_sb, scale=1.0 / D,
                )
                nc.vector.reciprocal(ms, ms)
                dst = u_t[b][par][blk // 2][:, blk % 2, :]
                if seng == 1:
                    nc.vector.tensor_scalar_mul(dst, xt, ms)
                elif seng == 2:
                    nc.scalar.activation(
                        dst, xt, mybir.ActivationFunctionType.Copy, scale=ms)
                else:
                    nc.gpsimd.tensor_scalar_mul(dst, xt, ms)

            # PE warmup: dummy matmuls on a zero tile ramp the tensor
            # engine p-state while the first input blocks stream in (memset
            # on gpsimd so the PE isn't gated on the busier DVE queue)
            wz = const_pool.tile([P, 2, 2 * P], FP8, name="wz")
            nc.gpsimd.memset(wz, 0.0)
            wps = psum_u_pool.tile([P, 2 * P], F32, name="psu")
            NW = 24
            for i in range(NW):
                nc.tensor.matmul(wps, lhsT=wz[:, :, 0:P], rhs=wz,
                                 start=i == 0, stop=i == NW - 1, perf_mode=DR)

            # prologue: x rows for conv blocks 0,1 stream before the tap
            # blocks (the taps are only needed once the PE issues Ldweights);
            # scale engines fan out so no single queue serializes readiness
            jprep(0, 0, 0, dve_sq=True, seng=1)
            jprep(0, 1, 0, seng=2)
            nc.sync.dma_start(tw_sb[:, 0], tw[0])
            nc.sync.dma_start(tw_sb[:, 1], tw[1])
            jprep(0, 0, 1, seng=0)
            jprep(0, 1, 1, seng=1)
            nc.sync.dma_start(
                wt_sb[:, :, :, 0, :, :],
                wt[:, :, 0].rearrange("f s p k d -> p f s k d"))
            next_blk = [2, 0]

            def conv_block(b, I, ut_sb):
                """both half-conv chains for seq block I into one psum; drain
                into the superblock ut tile (cols parity-major per filter)."""
                npair = I // 4 + 1
                half = I % 2
                for c in range(DC):
                    ps = psum_u_pool.tile([P, 2 * FPC * P], F32, name="psu")
                    for chain in range(2):
                        dst = ps[:, chain * FPC * P:(chain + 1) * FPC * P]
                        if b == 0 and I < 2:
                            # deltas (I, I-2): the ko=1 half is all-zero taps;
                            # a plain (non-DR) matmul on the first pair-half
                            # depends only on x rows 0..255, so the first
                            # convs start ~2us earlier
                            nc.tensor.matmul(
                                dst,
                                lhsT=u_t[b][chain][0][:, 0, c * P:(c + 1) * P],
                                rhs=tw_sb[:, I, 0, chain, :],
                                start=True, stop=True,
                            )
                            continue
                        for Jp in range(npair):
                            nc.tensor.matmul(
                                dst,
                                lhsT=u_t[b][chain][Jp][:, :, c * P:(c + 1) * P],
                                rhs=tw_sb[:, I - 4 * Jp, :, chain, :],
                                start=(Jp == 0),
                                stop=(Jp == npair - 1),
                                perf_mode=DR,
                            )
                    # psum free dim = (chain, f, par, 64); ut free dim per
                    # chunk = (chain*FPC+f, par, 128=(half,64))
                    pv = ps.rearrange("p (s q j) -> p s q j", s=2 * FPC, q=2)
                    dst = ut_sb[:, c, :, :, half * 64:half * 64 + 64]
                    if c % 2 == 0:
                        nc.vector.tensor_scalar_mul(
                            dst, pv, float(UT_SCALE / TAP_SCALE))
                    else:
                        nc.scalar.activation(
                            dst, pv, mybir.ActivationFunctionType.Copy,
                            scale=float(UT_SCALE / TAP_SCALE),
                        )

            def proj_block(b, Ip, par, ut_sb, last=False):
                """one parity's projection for superblock Ip: 128 same-parity
                rows of blocks (2Ip, 2Ip+1); sd picks Ws for the matching
                parity chain, Wd for the crossed one."""
                psp = psum_sp_pool.tile([P, D], F32, name="psp")
                sp_t = spill_pool.tile([P, D], F16, name="spt")
                r0 = 256 * Ip + par
                if last:
                    # tail only: sequential F1/256 chains so the F1 drain +
                    # DMA overlap the 256 matmuls (elsewhere this loses --
                    # the drain read blocks the tile's second chain)
                    halves = [[(0, F1)], [(F1, D)]]
                else:
                    halves = [[(0, F1), (F1, D)]]
                for grp_i, grp in enumerate(halves):
                    pst = psp
                    if last and grp_i == 1:
                        # separate psum tile: no tile-level WAR against the
                        # F1 chain's drain read
                        pst = psum_sp_pool.tile([P, D], F32, name="psp")
                    i_mm = 0
                    n_mm = 2 * FPC * (DC // 2)
                    for cp in range(DC // 2):
                        for st_i in range(2 * FPC):
                            chain, f = divmod(st_i, FPC)
                            sd = par if chain == 0 else 1 - par
                            lh = ut_sb[:, 2 * cp:2 * cp + 2, st_i, par, :]
                            for d0, d1 in grp:
                                nc.tensor.matmul(
                                    pst[:, d0:d1], lhsT=lh,
                                    rhs=wt_sb[:, f, sd, cp, :, d0:d1],
                                    start=i_mm == 0, stop=i_mm == n_mm - 1,
                                    perf_mode=DR,
                                )
                            i_mm += 1
                    for gi, (d0, d1) in enumerate(grp):
                        if (par + gi + grp_i) % 2 == 0:
                            nc.scalar.activation(
                                sp_t[:, d0:d1], pst[:, d0:d1],
                                mybir.ActivationFunctionType.Copy,
                                scale=float(1.0 / SP_SCALE),
                            )
                        else:
                            nc.vector.tensor_scalar_mul(
                                sp_t[:, d0:d1], pst[:, d0:d1],
                                float(1.0 / SP_SCALE))
                        nc.sync.dma_start(
                            sp[b, r0:r0 + 255:2, d0:d1], sp_t[:, d0:d1])

            # software pipeline: emit convs ahead of projections so the PE
            # covers psum-drain latency; a superblock's proj needs both its
            # conv blocks drained
            from collections import deque
            pend = deque()
            for b in range(B):
                for Ip in range(NSB):
                    ut_sb = drain_pool.tile([P, DC, 2 * FPC, 2, P], FP8,
                                            name="ut")
                    for half in range(2):
                        I = 2 * Ip + half
                        # pace the DMA queue: tw chunk I+2, remaining wt
                        # chunks, upcoming u row blocks
                        if b == 0 and I + 2 < NB:
                            nc.sync.dma_start(tw_sb[:, I + 2], tw[I + 2])
                        if b == 0 and 1 <= I < 3:
                            cp = I
                            nc.sync.dma_start(
                                wt_sb[:, :, :, cp, :, :],
                                wt[:, :, cp].rearrange(
                                    "f s p k d -> p f s k d"))
                        need = min(2 * (((I + 4) // 4)) + 1, NB // 2 - 1)
                        while next_blk[b] <= need:
                            jprep(b, 0, next_blk[b])
                            jprep(b, 1, next_blk[b])
                            next_blk[b] += 1
                        if b == 0 and I >= 10:
                            while next_blk[1] <= min(I - 10, NB // 2 - 1):
                                jprep(1, 0, next_blk[1])
                                jprep(1, 1, next_blk[1])
                                next_blk[1] += 1
                        conv_block(b, I, ut_sb)
                    pend.append((b, Ip, ut_sb))
                    depth = 2 if Ip < 3 else 1
                    while len(pend) > depth:
                        pb, pIp, put = pend.popleft()
                        proj_block(pb, pIp, 0, put)
                        proj_block(pb, pIp, 1, put)
            while pend:
                pb, pIp, put = pend.popleft()
                proj_block(pb, pIp, 0, put)
                proj_block(pb, pIp, 1, put,
                           last=pb == B - 1 and pIp == NSB - 1)
    nc.compile()
    return nc


def _build_phase2():
    nc = bacc.Bacc("TRN2", target_bir_lowering=False, debug=False, num_devices=NCORES)
    xr = nc.dram_tensor("xr", (RPC + 2, D), BF, kind="ExternalInput").ap()
    x1r = nc.dram_tensor("x1r", (RPC, D), F32, kind="ExternalInput").ap()
    mut = nc.dram_tensor("mut", (KU, DC, P, D), BF, kind="ExternalInput").ap()
    fc1 = nc.dram_tensor("fc1", (JC, DC, P, 2, 2, P), FP8, kind="ExternalInput").ap()
    fc2 = nc.dram_tensor("fc2", (JC, P, 2, D), FP8, kind="ExternalInput").ap()
    o = nc.dram_tensor("o", (RPC, D), F32, kind="ExternalOutput").ap()


    with tile.TileContext(nc) as tc:
        with (
            tc.tile_pool(name="const", bufs=1) as const_pool,
            tc.tile_pool(name="persist", bufs=1) as persist,
            tc.tile_pool(name="work", bufs=3) as work,
            tc.tile_pool(name="wstream", bufs=3) as wstream,
            tc.tile_pool(name="psum", bufs=4, space="PSUM") as psum_pool,
        ):
            ident = const_pool.tile([P, P], F32)
            make_identity(nc, ident)
            eps_sb = const_pool.tile([P, 1], F32)
            nc.vector.memset(eps_sb, float(EPS))
            dummy = const_pool.tile([P, 1], F32, name="dummy")
            nc.scalar.activation(
                dummy, eps_sb, mybir.ActivationFunctionType.Square)
            nc.scalar.activation(
                dummy, dummy, mybir.ActivationFunctionType.Sqrt)

            ut_ext = persist.tile([P, DC, MB, P + 2], BF)
            x1p = persist.tile([P, MB, D], F32)
            xrows = persist.tile([P, MB, D], BF)
            x1rows = persist.tile([P, MB, D], F32)
            yt = persist.tile([P, DC, 2, MB * P], FP8)
            gt = persist.tile([P, JC, 2, MB * P], FP8)
            mut_sb = persist.tile([P, KU, DC, D], BF)
            fc2_sb = persist.tile([P, JC, 2, D], FP8)

            def rmsnorm_to(dst, src, rows, dve_sq=False):
                """dst = src / rms(src); the rmsnorm weight is folded into
                the downstream contraction weights host-side."""
                sq = work.tile([P, D], F32, name="sq")
                ms = work.tile([P, 1], F32, name="ms")
                if dve_sq:
                    nc.vector.scalar_tensor_tensor(
                        sq[:rows], src[:rows], 1.0, src[:rows],
                        mybir.AluOpType.mult, mybir.AluOpType.mult,
                        accum_out=ms[:rows],
                    )
                else:
                    nc.scalar.activation(
                        sq[:rows], src[:rows],
                        mybir.ActivationFunctionType.Square,
                        accum_out=ms[:rows],
                    )
                nc.scalar.activation(
                    ms[:rows], ms[:rows], mybir.ActivationFunctionType.Sqrt,
                    bias=eps_sb[:rows], scale=1.0 / D,
                )
                nc.vector.reciprocal(ms[:rows], ms[:rows])
                nc.gpsimd.tensor_scalar_mul(dst, src[:rows], ms[:rows])

            # DMA queue front: prefix rows, rn1, the 4 u row blocks, then the
            # mut taps (in per-tap-half chunks so AR starts on the first),
            # then x1 rows; fc1/fc2 stream later in the fws loop
            u_pre = persist.tile([2, D], F32)
            xp = work.tile([P, D], BF, name="xt")[:2]
            nc.sync.dma_start(xp, xr[0:2, :])
            for m in range(MB):
                nc.sync.dma_start(
                    xrows[:, m, :], xr[2 + m * P: 2 + (m + 1) * P, :])
            HC = DC // 2
            for t in range(KU):
                for h in range(2):
                    nc.sync.dma_start(
                        mut_sb[:, t, h * HC:(h + 1) * HC, :],
                        mut[t, h * HC:(h + 1) * HC].rearrange("c p d -> p c d"),
                    )
            for m in range(MB):
                nc.sync.dma_start(x1rows[:, m, :], x1r[m * P:(m + 1) * P, :])

            def psum_copy(dst, src_ps, idx):
                if idx % 2 == 0:
                    nc.vector.tensor_copy(dst, src_ps)
                else:
                    nc.scalar.activation(
                        dst, src_ps, mybir.ActivationFunctionType.Copy
                    )

            # ---- u^T tiles for the AR term (rmsnorm1 + PE transpose);
            # the 2-row prefix runs after the m blocks so it stays off the
            # critical path ----
            for m in range(MB):
                uo = work.tile([P, D], F32, name="uo")
                rmsnorm_to(uo, xrows[:, m, :], P, dve_sq=m < 2)
                for c in range(DC):
                    pst = psum_pool.tile([P, D], F32, name="ps")[:, 0:P]
                    nc.tensor.transpose(pst, uo[:, c * P:(c + 1) * P], ident)
                    psum_copy(ut_ext[:, c, m, 2:P + 2], pst, c + 1)
            rmsnorm_to(u_pre, xp, 2)
            for c in range(DC):
                pst2 = psum_pool.tile([P, D], F32, name="ps")[:, 0:P]
                nc.tensor.transpose(
                    pst2[:, 0:2], u_pre[:, c * P:(c + 1) * P], ident[0:2, 0:2]
                )
                nc.vector.tensor_copy(ut_ext[:, c, 0, 0:2], pst2[:, 0:2])
            for m in range(1, MB):
                for c in range(DC):
                    nc.gpsimd.tensor_copy(
                        ut_ext[:, c, m, 0:2], ut_ext[:, c, m - 1, P:P + 2]
                    )

            # ---- AR term: all 4 row-blocks accumulate per-(tap, d-half) in
            # mut arrival order so the psum groups start on the first chunk ----
            psa = [psum_pool.tile([P, D], F32, name="ps") for _ in range(MB)]
            for t in range(KU):
                for h in range(2):
                    for m in range(MB):
                        for c in range(h * HC, (h + 1) * HC):
                            st = t == 0 and c == 0
                            fin = t == KU - 1 and c == DC - 1
                            lh = ut_ext[:, c, m, 2 - t:P + 2 - t]
                            nc.tensor.matmul(
                                psa[m][:, 0:F1], lhsT=lh,
                                rhs=mut_sb[:, t, c, 0:F1], start=st, stop=fin,
                            )
                            nc.tensor.matmul(
                                psa[m][:, F1:D], lhsT=lh,
                                rhs=mut_sb[:, t, c, F1:D], start=st, stop=fin,
                            )
            for m in range(MB):
                nc.vector.tensor_tensor(
                    x1p[:, m, :], x1rows[:, m, :], psa[m], mybir.AluOpType.add
                )

            # fc1 weight chunks + fc2/mut resident weights, in first-use order
            fws = []
            for jc in range(JC):
                fw = wstream.tile([P, DC, 2, 2, P], FP8, name="fw")
                nc.sync.dma_start(fw, fc1[jc].rearrange("c p q k f -> p c q k f"))
                fws.append(fw)
                if jc == 8:
                    nc.sync.dma_start(fc2_sb, fc2.rearrange("c p q d -> p c q d"))

            # ---- y = rmsnorm2(x1) transposed ----
            for m in range(MB):
                yf = work.tile([P, D], F32, name="uo")
                rmsnorm_to(yf, x1p[:, m, :], P)
                for c in range(DC):
                    pst = psum_pool.tile([P, D], F32, name="ps")[:, 0:P]
                    nc.tensor.transpose(pst, yf[:, c * P:(c + 1) * P], ident)
                    sl = slice(m * P, (m + 1) * P)
                    psum_copy(yt[:, c, 0, sl], pst, 1)
                    nc.vector.scalar_tensor_tensor(
                        yt[:, c, 1, sl], pst, 1.0, yt[:, c, 0, sl],
                        mybir.AluOpType.mult, mybir.AluOpType.subtract,
                    )

            # ---- fc1 + silu gate ----
            for jc in range(JC):
                ph1 = psum_pool.tile([P, D], F32, name="ps")[:, 0:F1]
                ph2 = psum_pool.tile([P, D], F32, name="ps")[:, 0:F1]
                for m in range(MB):
                    sl = slice(m * P, (m + 1) * P)
                    for hh, ph in ((0, ph1), (1, ph2)):
                        # y_hi @ w_hi, c-pairs packed in DoubleRow
                        for cp in range(DC // 2):
                            nc.tensor.matmul(
                                ph[:, sl],
                                lhsT=fws[jc][:, 2 * cp:2 * cp + 2, 1, hh, :],
                                rhs=yt[:, 2 * cp:2 * cp + 2, 0, sl],
                                start=cp == 0, stop=False, perf_mode=DR,
                            )
                        # cross terms w_lo@y_hi + w_hi@y_lo, DR-paired per c
                        for c in range(DC):
                            nc.tensor.matmul(
                                ph[:, sl],
                                lhsT=fws[jc][:, c, :, hh, :],
                                rhs=yt[:, c, :, sl],
                                start=False, stop=c == DC - 1, perf_mode=DR,
                            )
                sact = work.tile([P, F1], F32, name="sact")
                nc.scalar.activation(sact, ph2,
                                     mybir.ActivationFunctionType.Silu,
                                     scale=float(1.0 / MLP_SCALE))
                g32 = work.tile([P, F1], F32, name="g32")
                nc.vector.scalar_tensor_tensor(
                    g32, ph1, float(1.0 / MLP_SCALE), sact,
                    mybir.AluOpType.mult, mybir.AluOpType.mult,
                )
                nc.scalar.activation(
                    gt[:, jc, 0, :], g32, mybir.ActivationFunctionType.Copy
                )
                nc.vector.scalar_tensor_tensor(
                    gt[:, jc, 1, :], g32, 1.0, gt[:, jc, 0, :],
                    mybir.AluOpType.mult, mybir.AluOpType.subtract,
                )

            # ---- fc2 + residual, m-outer so each row block's residual add
            # and output DMA overlap the next block's matmuls ----
            for m in range(MB):
                po = psum_pool.tile([P, D], F32, name="ps")
                msl = slice(m * P, (m + 1) * P)
                ot = work.tile([P, D], F32, name="ot")
                for d0, d1 in ((0, F1), (F1, D)):
                    for jp in range(JC // 2):
                        lh = gt[:, 2 * jp:2 * jp + 2, 0, msl]
                        rh = fc2_sb[:, 2 * jp:2 * jp + 2, 1, d0:d1]
                        nc.tensor.matmul(po[:, d0:d1], lhsT=lh, rhs=rh,
                                         start=jp == 0, stop=False,
                                         perf_mode=DR)
                    for jc in range(JC):
                        lh = gt[:, jc, :, msl]
                        rh = fc2_sb[:, jc, :, d0:d1]
                        nc.tensor.matmul(po[:, d0:d1], lhsT=lh, rhs=rh,
                                         start=False, stop=jc == JC - 1,
                                         perf_mode=DR)
                    nc.vector.scalar_tensor_tensor(
                        ot[:, d0:d1], po[:, d0:d1], float(1.0 / MLP_SCALE),
                        x1p[:, m, d0:d1],
                        mybir.AluOpType.mult, mybir.AluOpType.add)
                    nc.sync.dma_start(o[m * P:(m + 1) * P, d0:d1],
                                      ot[:, d0:d1])
    nc.compile()
    return nc


def _host_prep(V, sigma, M_u, M_phi_plus, M_phi_minus, rn1):
    """Per-core tap blocks + fused projection matrices for the parity-split
    conv. Core c owns filters (2c, 2c+1). rn1_w folds into the projection's
    contraction axis; sigma^0.25 folds into the taps.

    tw[d0, r, ko, chain, f*128 + po*64 + j] =
        taps_f[128*(d0 - 2*ko) + (2j + po) - 2r - chain]   (0 if idx < 0)
    (chain 0 reads even input rows, 1 odd; output cols parity-major po)
    wt[f, sd, cp, i, ko, d] = {Ws,Wd}[f][d, (2cp+ko)*128 + i] * W_SCALE
    """
    phi = np.fft.irfft(V.astype(np.complex128), n=NFFT, axis=0)[:SL]
    s4 = sigma.astype(np.float64) ** 0.25
    taps = (s4[None, :] * phi)                       # (SL, K)
    Ws = (M_phi_plus + M_phi_minus) * rn1[None, None, :]
    Wd = (M_phi_plus - M_phi_minus) * rn1[None, None, :]

    # col -> s_out map (parity-major)
    s_out = np.empty(P, np.int64)
    s_out[:64] = 2 * np.arange(64)
    s_out[64:] = 2 * np.arange(64) + 1

    d0v = np.arange(NB)[:, None, None, None, None]
    rv = np.arange(P)[None, :, None, None, None]
    kov = np.arange(2)[None, None, :, None, None]
    chv = np.arange(2)[None, None, None, :, None]
    colv = s_out[None, None, None, None, :]
    idx = 128 * (d0v - 2 * kov) + colv - 2 * rv - chv   # (NB,P,2,2,P)

    tw_cores = []
    wt_cores = []
    for core in range(NCORES):
        tw = np.zeros((NB, P, 2, 2, FPC * P), np.float32)
        wt = np.zeros((FPC, 2, DC // 2, P, 2, D), np.float32)
        for f in range(FPC):
            k = core * FPC + f
            tsc = (taps[:, k] * TAP_SCALE).astype(np.float64)
            blk = np.where(idx >= 0, tsc[np.clip(idx, 0, SL - 1)], 0.0)
            tw[:, :, :, :, f * P:(f + 1) * P] = blk
            for sd, W in ((0, Ws[k]), (1, Wd[k])):
                for cp in range(DC // 2):
                    for ko in range(2):
                        c = 2 * cp + ko
                        wt[f, sd, cp, :, ko, :] = (
                            W[:, c * P:(c + 1) * P].T * W_SCALE)
        tw_cores.append(tw.astype(FP8NP))
        wt_cores.append(wt.astype(FP8NP))
    return tw_cores, wt_cores


def kernel(x, V, sigma, M_u, M_phi_plus, M_phi_minus, rn1_w, rn2_w, fc1_w, fc2_w):
    x = np.ascontiguousarray(x, np.float32)
    if "p1" not in _cache:
        _cache["p1"] = _SpmdRunner(_build_phase1(), shared=("xb",), volatile=("xb",))
    if "p2" not in _cache:
        _cache["p2"] = _SpmdRunner(
            _build_phase2(), shared=("mut", "fc1", "fc2"),
            volatile=("xr", "x1r"))

    rn1 = np.ascontiguousarray(rn1_w, np.float32)
    rn2 = np.ascontiguousarray(rn2_w, np.float32)
    tw_cores, wt_cores = _host_prep(V, sigma, M_u, M_phi_plus, M_phi_minus, rn1)
    xb = x.astype(BF16NP)

    in_maps1 = [
        {"xb": xb, "tw": tw_cores[c], "wt": wt_cores[c]}
        for c in range(NCORES)
    ]
    r1 = _cache["p1"]
    sp_cat = r1.run_prepped(r1.prep(in_maps1))[0]
    if "reduce" not in _cache:
        import jax
        import jax.numpy as jnp
        from jax.sharding import NamedSharding, PartitionSpec
        sh = NamedSharding(r1._shardings["xb"].mesh, PartitionSpec())
        _cache["reduce"] = jax.jit(
            lambda spc, xx: xx + spc.reshape(NCORES, B, SL, D)
            .astype(jnp.float32).sum(0),
            out_shardings=sh,
        )
    x1 = np.asarray(_cache["reduce"](sp_cat, np.asarray(x)))

    # phase 2 inputs; rn1 folds into mut's contraction axis, rn2 into fc1's
    mut = np.zeros((KU, DC, P, D), np.float32)
    for t in range(KU):
        for c in range(DC):
            mut[t, c] = (M_u[t] * rn1[None, :])[:, c * P:(c + 1) * P].T
    mut = mut.astype(BF16NP)
    # fc1 pre-paired layout (JC, DC, P, 2, P): [..., 0, :] = y half column
    # block jc, [..., 1, :] = gate half column block jc
    f1s = np.ascontiguousarray(fc1_w, np.float32) * rn2[:, None] * MLP_SCALE
    hi8 = f1s.astype(FP8NP)
    lo8 = (f1s - hi8.astype(np.float32)).astype(FP8NP)

    def _lay(a):
        return np.transpose(a.reshape(DC, P, 2, JC, P), (3, 0, 1, 2, 4))

    # q axis: 0 = lo, 1 = hi (cross-product DR pairs lo/hi against yt's hi/lo)
    fc1p = np.ascontiguousarray(np.stack([_lay(lo8), _lay(hi8)], axis=3))
    f2s = np.ascontiguousarray(fc2_w, np.float32).reshape(JC, P, D) * MLP_SCALE
    f2hi = f2s.astype(FP8NP)
    f2lo = (f2s - f2hi.astype(np.float32)).astype(FP8NP)
    fc2 = np.ascontiguousarray(np.stack([f2lo, f2hi], axis=2))

    x_rows = x.reshape(B * SL, D)
    x1_rows = x1.reshape(B * SL, D)
    in_maps2 = []
    for c in range(NCORES):
        r0 = c * RPC
        xr = np.zeros((RPC + 2, D), np.float32)
        xr[2:] = x_rows[r0:r0 + RPC]
        if r0 % SL != 0:
            xr[0:2] = x_rows[r0 - 2:r0]
        in_maps2.append({
            "xr": xr.astype(BF16NP),
            "x1r": np.ascontiguousarray(x1_rows[r0:r0 + RPC]),
            "mut": mut, "fc1": fc1p, "fc2": fc2,
        })
    res2 = _cache["p2"](in_maps2)
    out = np.concatenate(
        [res2[c]["o"] for c in range(NCORES)], axis=0
    ).reshape(B, SL, D)
    return out



# revision 26
# speedup vs baseline: 1.1961x; 1.0801x over previous
"""Trainium2 Bass kernel for the STU (spectral transform unit) dense-transformer block.

Algorithm (validated against the jax reference in fp64 numpy):
  The FFT causal conv is rewritten as a block-Toeplitz matmul. For each of the
  K=16 filters and each sign branch (the alternating-sign branch folds into the
  filter taps: T^-[s,s'] = phi[s-s'] * (-1)^(s-s')), the causal conv is
    U_br = T_br @ u,  T_br block-Toeplitz with 16 distinct 128x128 blocks.
  sigma^(1/4) folds into the taps. The (k,i)->d projection contracts U with
  M_phi_{plus,minus}; the KU=3 autoregressive taps are shifted-u projections
  with M_u. MLP is a standard gated MLP.

Sharding (8 cores, no cross-core communication, host-side reduce between two
uniform SPMD programs):
  Phase 1: filter-branch-parallel. Core c computes conv + projection for its 4
           of the 32 (k, sign) branches over the full (B, SL): partial spectral.
  Host:    x1 = x + sum_c partial_c
  Phase 2: row-parallel. Core c owns 512 of the 4096 (b, s) rows: adds the AR
           term and computes the gated MLP + residual for its rows.

Precision: the conv runs in fp8 (output magnitude ~0.05 -> noise negligible);
fc1 and fc2 run as compensated hi+lo fp8 splits (h = yh@wh + DoubleRow-paired
cross terms wl@yh + wh@yl, dropping only the second-order yl@wl product),
which gets fp8 DoubleRow matmul rates at bf16-class accuracy. The AR term
stays bf16: its shifted u^T windows would have odd byte offsets in fp8,
which the Ldweights ISA rejects. Measured end-to-end error: 3.9e-3
scale-relative vs the 2e-2 harness gate. rn1/rn2 rmsnorm weights are folded
into the downstream contraction weights host-side (they commute through the
seq-dim conv / shifts).

Schedule notes (all targets are the InstructionCostModel timeline):
 - DMA is a serial ~360GB/s resource; transfers are emitted in the order
   compute needs them (x row-blocks and tw delta-chunks interleaved, weights
   after first-use rows), which removes the 24us/30us startup stalls the
   v1 kernel had.
 - PSUM->SBUF drain copies alternate across DVE/Act (GPSIMD cannot read
   PSUM); Pool takes the SBUF->SBUF rmsnorm multiplies.
 - Phase-1 software-pipelines the next block's conv between conv(I) and
   proj(I) (depth 2-3 for the short early blocks) so the PE covers the
   psum-drain latency; projection iterates cp-outer so its first matmuls
   depend only on the first conv psum drains.
 - Phase-1 warms the PE p-state with dummy matmuls while the first input
   blocks stream in; phase-2 finalizes fc2 m-outer and d-half-outer so each
   row block's residual add and output DMA overlap later matmuls.
 - Both phases issue dummy Square/Sqrt activations at the head of the
   Activation queue so the act-table loads finish before the first rmsnorm
   needs them.
"""

import numpy as np
import ml_dtypes

import concourse.bacc as bacc
import concourse.tile as tile
from concourse import mybir
from concourse.bass_utils import run_bass_kernel_spmd  # noqa: F401 (debug path)
from concourse.masks import make_identity


class _SpmdRunner:
    """Cached-jit SPMD executor: trace/compile once, then repeat calls only
    pay input upload + execution (mirrors bass2jax.run_bass_via_pjrt).

    ``shared`` names inputs that are identical on every core: they are fed
    replicated (host uploads one copy) instead of 8x-concatenated."""

    def __init__(self, nc, shared=(), volatile=()):
        import jax
        import concourse.mybir as _mb
        from concourse.bass2jax import (
            install_neuronx_cc_hook, _bass_exec_p, partition_id_tensor,
        )
        from jax.experimental.shard_map import shard_map
        from jax.sharding import Mesh, PartitionSpec

        install_neuronx_cc_hook()
        self.nc = nc
        assert nc.dbg_addr is None
        pid_name = (nc.partition_id_tensor.name
                    if nc.partition_id_tensor is not None else None)
        in_names, out_names, out_avals = [], [], []
        for alloc in nc.m.functions[0].allocations:
            if not isinstance(alloc, mybir.MemoryLocationSet):
                continue
            name = alloc.memorylocations[0].name
            if alloc.kind == "ExternalInput":
                if name != pid_name:
                    in_names.append(name)
            elif alloc.kind == "ExternalOutput":
                out_names.append(name)
                out_avals.append(jax.core.ShapedArray(
                    tuple(alloc.tensor_shape), mybir.dt.np(alloc.dtype)))
        self.in_names, self.out_names, self.out_avals = in_names, out_names, out_avals
        self.shared = frozenset(shared)
        self.volatile = frozenset(volatile)
        self._dev_cache = {}
        n_params = len(in_names)
        all_names = tuple(in_names + out_names)
        if pid_name is not None:
            all_names = all_names + (pid_name,)

        def _body(*args):
            args = list(args)
            if pid_name is not None:
                args.append(partition_id_tensor())
            return tuple(_bass_exec_p.bind(
                *args,
                out_avals=tuple(out_avals),
                in_names=all_names,
                out_names=tuple(out_names),
                lowering_input_output_aliases=(),
                sim_require_finite=True,
                sim_require_nnan=True,
                nc=nc,
            ))

        import jax.numpy as jnp
        from jax.sharding import NamedSharding
        devices = jax.devices()[:NCORES]
        mesh = Mesh(np.asarray(devices), ("core",))
        rep = PartitionSpec()
        core = PartitionSpec("core")
        in_specs = tuple(
            rep if nm in self.shared else core for nm in in_names
        ) + (core,) * len(out_names)
        out_specs = (core,) * len(out_names)
        donate = tuple(range(n_params, n_params + len(out_names)))
        self._fn = jax.jit(
            shard_map(_body, mesh=mesh, in_specs=in_specs, out_specs=out_specs,
                      check_rep=False),
            donate_argnums=donate, keep_unused=True,
        )
        self._zeros_fn = jax.jit(
            lambda: tuple(
                jnp.zeros((NCORES * a.shape[0], *a.shape[1:]), a.dtype)
                for a in out_avals
            ),
            out_shardings=tuple(
                NamedSharding(mesh, core) for _ in out_avals
            ),
        )
        self._shardings = {
            nm: NamedSharding(mesh, rep if nm in self.shared else core)
            for nm in in_names
        }

    def prep(self, in_maps):
        import hashlib
        import jax
        ins = []
        for nm in self.in_names:
            if nm in self.shared:
                arr = np.ascontiguousarray(in_maps[0][nm])
            else:
                arr = np.concatenate(
                    [np.asarray(in_maps[c][nm]) for c in range(NCORES)], axis=0)
            if nm in self.volatile:
                ins.append(arr)
                continue
            key = (nm, hashlib.md5(arr.tobytes()).hexdigest())
            dev = self._dev_cache.get(key)
            if dev is None:
                self._dev_cache.clear() if len(self._dev_cache) > 32 else None
                dev = jax.device_put(arr, self._shardings[nm])
                self._dev_cache[key] = dev
            ins.append(dev)
        return ins

    def run_prepped(self, ins):
        return self._fn(*ins, *self._zeros_fn())

    def __call__(self, in_maps):
        out_arrs = self.run_prepped(self.prep(in_maps))
        return [
            {nm: np.asarray(out_arrs[i]).reshape(NCORES, *self.out_avals[i].shape)[c]
             for i, nm in enumerate(self.out_names)}
            for c in range(NCORES)
        ]

BF16NP = ml_dtypes.bfloat16
FP8NP = ml_dtypes.float8_e4m3
TAP_SCALE = 1024.0
UT_SCALE = 32.0      # psum (TAP_SCALE*U) -> fp8 ut tiles scale factor: 32/1024
W_SCALE = 16.0       # projection weights scaled by 16 for fp8 range
SP_SCALE = UT_SCALE * W_SCALE  # spectral psum carries 32*16 = 512x
MLP_SCALE = 16.0     # fc1 hi/lo fp8 weights carry 16x for fp8 range
F32 = mybir.dt.float32
F32R = mybir.dt.float32r
F16 = mybir.dt.float16
BF = mybir.dt.bfloat16
FP8 = mybir.dt.float8e4

B, SL, D, K, KU = 2, 2048, 768, 16, 3
NFFT, EPS, P, H = 4096, 1e-5, 128, 3072
NB = SL // P            # 16 seq blocks
DC = D // P             # 6 d-chunks
NBR = 2 * K             # 32 conv branches
NCORES = 8
BPC = NBR // NCORES     # 4 branches per core
FPC = K // NCORES       # 2 filters per core (parity-fused conv)
NSB = SL // (2 * P)     # 8 superblocks (256 rows) per batch
RPC = (B * SL) // NCORES  # 512 rows per core
MB = RPC // P           # 4 row blocks per core in phase 2
JC = H // P             # 24 hidden chunks
F1 = 512                # free-dim split of D=768 into 512+256
DR = mybir.MatmulPerfMode.DoubleRow

_cache: dict = {}


def _build_phase1():
    """Parity-fused spectral conv: since T^- = D T D (D = alt signs), the
    even/odd half-convs C_e, C_o of each filter determine both sign branches:
      spectral[even s] = C_e Ws + C_o Wd,  spectral[odd s] = C_e Wd + C_o Ws
    with Ws = W+ + W-, Wd = W+ - W-. The conv FLOPs halve (each half-conv
    reads only half the input rows); the +- reconstruction is absorbed into
    the projection weights at no extra cost. Projection psums pack the
    same-parity rows of two consecutive 128-row blocks (conv output columns
    are emitted parity-major so the proj lhsT stays a contiguous 128-slice).
    Each core owns FPC=2 of the 16 filters."""
    nc = bacc.Bacc("TRN2", target_bir_lowering=False, debug=False, num_devices=NCORES)
    xb = nc.dram_tensor("xb", (B, SL, D), BF, kind="ExternalInput").ap()
    # tw[d0, r, ko, chain, f*128+col]: tap block pair (delta=d0-2*ko), chain
    # 0=even-input 1=odd-input, col parity-major within each filter's 128
    tw = nc.dram_tensor("tw", (NB, P, 2, 2, FPC * P), FP8, kind="ExternalInput").ap()
    # wt[f, sd(0=Ws,1=Wd), cp, r, ko, d_out]
    wt = nc.dram_tensor("wt", (FPC, 2, DC // 2, P, 2, D), FP8, kind="ExternalInput").ap()
    sp = nc.dram_tensor("sp", (B, SL, D), F16, kind="ExternalOutput").ap()

    with tile.TileContext(nc) as tc:
        with (
            tc.tile_pool(name="const", bufs=1) as const_pool,
            tc.tile_pool(name="ubuf", bufs=1) as ubuf_pool,
            tc.tile_pool(name="work", bufs=3) as work,
            tc.tile_pool(name="drain", bufs=4) as drain_pool,
            tc.tile_pool(name="spill", bufs=3) as spill_pool,
            tc.tile_pool(name="psum_u", bufs=4, space="PSUM") as psum_u_pool,
            tc.tile_pool(name="psum_sp", bufs=2, space="PSUM") as psum_sp_pool,
        ):
            eps_sb = const_pool.tile([P, 1], F32)
            nc.vector.memset(eps_sb, float(EPS))
            dummy = const_pool.tile([P, 1], F32, name="dummy")
            nc.scalar.activation(
                dummy, eps_sb, mybir.ActivationFunctionType.Square)
            nc.scalar.activation(
                dummy, dummy, mybir.ActivationFunctionType.Sqrt)
            tw_sb = const_pool.tile([P, NB, 2, 2, FPC * P], FP8)
            wt_sb = const_pool.tile([P, FPC, 2, DC // 2, 2, D], FP8)

            # one persistent fp8 u tile per (b, parity, even/odd-block pair):
            # u_t[b][par][jp][:, ko, :] holds rmsnormed rows
            # 256*(2*jp+ko) + 2r + par of batch b
            u_t = [[[ubuf_pool.tile([P, 2, D], FP8, name=f"u{b}_{par}_{jp}")
                     for jp in range(NB // 4)] for par in range(2)]
                   for b in range(B)]

            def jprep(b, par, blk, dve_sq=False, seng=0):
                """strided x row DMA (one parity class) -> rmsnorm -> fp8.
                (rn1_w is folded into the projection weights host-side.)
                seng: engine for the final scale (0=Pool, 1=DVE, 2=Act) --
                the head jpreps fan out so the Pool queue doesn't serialize
                the first conv's inputs."""
                xt = work.tile([P, D], BF, name="xt")
                r0 = 256 * blk + par
                nc.sync.dma_start(xt, xb[b, r0:r0 + 255:2, :])
                sq = work.tile([P, D], F32, name="sq")
                ms = work.tile([P, 1], F32, name="ms")
                if dve_sq:
                    nc.vector.scalar_tensor_tensor(
                        sq, xt, 1.0, xt, mybir.AluOpType.mult,
                        mybir.AluOpType.mult, accum_out=ms,
                    )
                else:
                    nc.scalar.activation(
                        sq, xt, mybir.ActivationFunctionType.Square,
                        accum_out=ms,
                    )
                nc.scalar.activation(
                    ms, ms, mybir.ActivationFunctionType.Sqrt,
                    bias=eps_sb, scale=1.0 / D,
                )
                nc.vector.reciprocal(ms, ms)
                dst = u_t[b][par][blk // 2][:, blk % 2, :]
                if seng == 1:
                    nc.vector.tensor_scalar_mul(dst, xt, ms)
                elif seng == 2:
                    nc.scalar.activation(
                        dst, xt, mybir.ActivationFunctionType.Copy, scale=ms)
                else:
                    nc.gpsimd.tensor_scalar_mul(dst, xt, ms)

            # PE warmup: dummy matmuls on a zero tile ramp the tensor
            # engine p-state while the first input blocks stream in (memset
            # on gpsimd so the PE isn't gated on the busier DVE queue)
            wz = const_pool.tile([P, 2, 2 * P], FP8, name="wz")
            nc.gpsimd.memset(wz, 0.0)
            wps = psum_u_pool.tile([P, 2 * P], F32, name="psu")
            NW = 24
            for i in range(NW):
                nc.tensor.matmul(wps, lhsT=wz[:, :, 0:P], rhs=wz,
                                 start=i == 0, stop=i == NW - 1, perf_mode=DR)

            # prologue: x rows for conv blocks 0,1 stream before the tap
            # blocks (the taps are only needed once the PE issues Ldweights);
            # scale engines fan out so no single queue serializes readiness
            jprep(0, 0, 0, dve_sq=True, seng=1)
            jprep(0, 1, 0, seng=2)
            nc.sync.dma_start(tw_sb[:, 0], tw[0])
            nc.sync.dma_start(tw_sb[:, 1], tw[1])
            jprep(0, 0, 1, seng=0)
            jprep(0, 1, 1, seng=1)
            nc.sync.dma_start(
                wt_sb[:, :, :, 0, :, :],
                wt[:, :, 0].rearrange("f s p k d -> p f s k d"))
            next_blk = [2, 0]

            def conv_block(b, I, ut_sb):
                """both half-conv chains for seq block I into one psum; drain
                into the superblock ut tile (cols parity-major per filter)."""
                npair = I // 4 + 1
                half = I % 2
                for c in range(DC):
                    ps = psum_u_pool.tile([P, 2 * FPC * P], F32, name="psu")
                    for chain in range(2):
                        dst = ps[:, chain * FPC * P:(chain + 1) * FPC * P]
                        if b == 0 and I < 2:
                            # deltas (I, I-2): the ko=1 half is all-zero taps;
                            # a plain (non-DR) matmul on the first pair-half
                            # depends only on x rows 0..255, so the first
                            # convs start ~2us earlier
                            nc.tensor.matmul(
                                dst,
                                lhsT=u_t[b][chain][0][:, 0, c * P:(c + 1) * P],
                                rhs=tw_sb[:, I, 0, chain, :],
                                start=True, stop=True,
                            )
                            continue
                        for Jp in range(npair):
                            nc.tensor.matmul(
                                dst,
                                lhsT=u_t[b][chain][Jp][:, :, c * P:(c + 1) * P],
                                rhs=tw_sb[:, I - 4 * Jp, :, chain, :],
                                start=(Jp == 0),
                                stop=(Jp == npair - 1),
                                perf_mode=DR,
                            )
                    # psum free dim = (chain, f, par, 64); ut free dim per
                    # chunk = (chain*FPC+f, par, 128=(half,64))
                    pv = ps.rearrange("p (s q j) -> p s q j", s=2 * FPC, q=2)
                    dst = ut_sb[:, c, :, :, half * 64:half * 64 + 64]
                    if c % 2 == 0:
                        nc.vector.tensor_scalar_mul(
                            dst, pv, float(UT_SCALE / TAP_SCALE))
                    else:
                        nc.scalar.activation(
                            dst, pv, mybir.ActivationFunctionType.Copy,
                            scale=float(UT_SCALE / TAP_SCALE),
                        )

            def proj_block(b, Ip, par, ut_sb, last=False):
                """one parity's projection for superblock Ip: 128 same-parity
                rows of blocks (2Ip, 2Ip+1); sd picks Ws for the matching
                parity chain, Wd for the crossed one."""
                psp = psum_sp_pool.tile([P, D], F32, name="psp")
                sp_t = spill_pool.tile([P, D], F16, name="spt")
                r0 = 256 * Ip + par
                if last:
                    # tail only: sequential F1/256 chains so the F1 drain +
                    # DMA overlap the 256 matmuls (elsewhere this loses --
                    # the drain read blocks the tile's second chain)
                    halves = [[(0, F1)], [(F1, D)]]
                else:
                    halves = [[(0, F1), (F1, D)]]
                for grp_i, grp in enumerate(halves):
                    pst = psp
                    if last and grp_i == 1:
                        # separate psum tile: no tile-level WAR against the
                        # F1 chain's drain read
                        pst = psum_sp_pool.tile([P, D], F32, name="psp")
                    i_mm = 0
                    n_mm = 2 * FPC * (DC // 2)
                    for cp in range(DC // 2):
                        for st_i in range(2 * FPC):
                            chain, f = divmod(st_i, FPC)
                            sd = par if chain == 0 else 1 - par
                            lh = ut_sb[:, 2 * cp:2 * cp + 2, st_i, par, :]
                            for d0, d1 in grp:
                                nc.tensor.matmul(
                                    pst[:, d0:d1], lhsT=lh,
                                    rhs=wt_sb[:, f, sd, cp, :, d0:d1],
                                    start=i_mm == 0, stop=i_mm == n_mm - 1,
                                    perf_mode=DR,
                                )
                            i_mm += 1
                    for gi, (d0, d1) in enumerate(grp):
                        if (par + gi + grp_i) % 2 == 0:
                            nc.scalar.activation(
                                sp_t[:, d0:d1], pst[:, d0:d1],
                                mybir.ActivationFunctionType.Copy,
                                scale=float(1.0 / SP_SCALE),
                            )
                        else:
                            nc.vector.tensor_scalar_mul(
                                sp_t[:, d0:d1], pst[:, d0:d1],
                                float(1.0 / SP_SCALE))
                        nc.sync.dma_start(
                            sp[b, r0:r0 + 255:2, d0:d1], sp_t[:, d0:d1])

            # software pipeline: emit convs ahead of projections so the PE
            # covers psum-drain latency; a superblock's proj needs both its
            # conv blocks drained
            from collections import deque
            pend = deque()
            for b in range(B):
                for Ip in range(NSB):
                    ut_sb = drain_pool.tile([P, DC, 2 * FPC, 2, P], FP8,
                                            name="ut")
                    for half in range(2):
                        I = 2 * Ip + half
                        # pace the DMA queue: tw chunk I+2, remaining wt
                        # chunks, upcoming u row blocks
                        if b == 0 and I + 2 < NB:
                            nc.sync.dma_start(tw_sb[:, I + 2], tw[I + 2])
                        if b == 0 and 1 <= I < 3:
                            cp = I
                            nc.sync.dma_start(
                                wt_sb[:, :, :, cp, :, :],
                                wt[:, :, cp].rearrange(
                                    "f s p k d -> p f s k d"))
                        need = min(2 * (((I + 4) // 4)) + 1, NB // 2 - 1)
                        while next_blk[b] <= need:
                            jprep(b, 0, next_blk[b])
                            jprep(b, 1, next_blk[b])
                            next_blk[b] += 1
                        if b == 0 and I >= 10:
                            while next_blk[1] <= min(I - 10, NB // 2 - 1):
                                jprep(1, 0, next_blk[1])
                                jprep(1, 1, next_blk[1])
                                next_blk[1] += 1
                        conv_block(b, I, ut_sb)
                    pend.append((b, Ip, ut_sb))
                    depth = 2 if Ip < 3 else 1
                    while len(pend) > depth:
                        pb, pIp, put = pend.popleft()
                        proj_block(pb, pIp, 0, put)
                        proj_block(pb, pIp, 1, put)
            while pend:
                pb, pIp, put = pend.popleft()
                proj_block(pb, pIp, 0, put)
                proj_block(pb, pIp, 1, put,
                           last=pb == B - 1 and pIp == NSB - 1)
    nc.compile()
    return nc


def _build_phase2():
    """Row-parallel AR + gated MLP. The MLP runs fp8 weights-hi-only with
    activation-side compensation: h = (yh + yl) @ wh (yl = y - fp8(y)), so
    the only dropped term is y @ (w - fp8(w)) -- a fixed small weight
    perturbation. Same for fc2 with g hi/lo. Halves both the matmul count
    (vs the hi/lo cross scheme) and the weight DMA. The AR term stays bf16
    (shifted fp8 u^T windows would need odd byte offsets in Ldweights)."""
    nc = bacc.Bacc("TRN2", target_bir_lowering=False, debug=False, num_devices=NCORES)
    xr = nc.dram_tensor("xr", (RPC + 2, D), BF, kind="ExternalInput").ap()
    x1r = nc.dram_tensor("x1r", (RPC, D), F16, kind="ExternalInput").ap()
    mut = nc.dram_tensor("mut", (KU, DC, P, D), BF, kind="ExternalInput").ap()
    fc1 = nc.dram_tensor("fc1", (JC, DC, P, 2, P), FP8, kind="ExternalInput").ap()
    fc2 = nc.dram_tensor("fc2", (JC, P, D), FP8, kind="ExternalInput").ap()
    o = nc.dram_tensor("o", (RPC, D), F16, kind="ExternalOutput").ap()

    with tile.TileContext(nc) as tc:
        with (
            tc.tile_pool(name="const", bufs=1) as const_pool,
            tc.tile_pool(name="persist", bufs=1) as persist,
            tc.tile_pool(name="work", bufs=3) as work,
            tc.tile_pool(name="wstream", bufs=3) as wstream,
            tc.tile_pool(name="psum", bufs=4, space="PSUM") as psum_pool,
        ):
            ident = const_pool.tile([P, P], F32)
            make_identity(nc, ident)
            eps_sb = const_pool.tile([P, 1], F32)
            nc.vector.memset(eps_sb, float(EPS))
            dummy = const_pool.tile([P, 1], F32, name="dummy")
            nc.scalar.activation(
                dummy, eps_sb, mybir.ActivationFunctionType.Square)
            nc.scalar.activation(
                dummy, dummy, mybir.ActivationFunctionType.Sqrt)

            ut_ext = persist.tile([P, DC, MB, P + 2], BF)
            x1p = persist.tile([P, MB, D], F32)
            xrows = persist.tile([P, MB, D], BF)
            x1rows = persist.tile([P, MB, D], F16)
            yt = persist.tile([P, DC, 2, MB * P], FP8)
            gt = persist.tile([P, JC, 2, MB * P], FP8)
            mut_sb = persist.tile([P, KU, DC, D], BF)
            fc2_sb = persist.tile([P, JC, D], FP8)

            def rmsnorm_to(dst, src, rows, dve_sq=False):
                """dst = src / rms(src); the rmsnorm weight is folded into
                the downstream contraction weights host-side."""
                sq = work.tile([P, D], F32, name="sq")
                ms = work.tile([P, 1], F32, name="ms")
                if dve_sq:
                    nc.vector.scalar_tensor_tensor(
                        sq[:rows], src[:rows], 1.0, src[:rows],
                        mybir.AluOpType.mult, mybir.AluOpType.mult,
                        accum_out=ms[:rows],
                    )
                else:
                    nc.scalar.activation(
                        sq[:rows], src[:rows],
                        mybir.ActivationFunctionType.Square,
                        accum_out=ms[:rows],
                    )
                nc.scalar.activation(
                    ms[:rows], ms[:rows], mybir.ActivationFunctionType.Sqrt,
                    bias=eps_sb[:rows], scale=1.0 / D,
                )
                nc.vector.reciprocal(ms[:rows], ms[:rows])
                nc.gpsimd.tensor_scalar_mul(dst, src[:rows], ms[:rows])

            # DMA queue front: prefix rows, the 4 x row blocks, mut taps (in
            # per-tap-half chunks so AR starts on the first), then x1 rows;
            # fc1/fc2 stream later in the fws loop
            u_pre = persist.tile([2, D], F32)
            xp = work.tile([P, D], BF, name="xt")[:2]
            nc.sync.dma_start(xp, xr[0:2, :])
            for m in range(MB):
                nc.sync.dma_start(
                    xrows[:, m, :], xr[2 + m * P: 2 + (m + 1) * P, :])
            HC = DC // 2
            for t in range(KU):
                for h in range(2):
                    nc.sync.dma_start(
                        mut_sb[:, t, h * HC:(h + 1) * HC, :],
                        mut[t, h * HC:(h + 1) * HC].rearrange("c p d -> p c d"),
                    )
            for m in range(MB):
                nc.sync.dma_start(x1rows[:, m, :], x1r[m * P:(m + 1) * P, :])

            def psum_copy(dst, src_ps, idx):
                if idx % 2 == 0:
                    nc.vector.tensor_copy(dst, src_ps)
                else:
                    nc.scalar.activation(
                        dst, src_ps, mybir.ActivationFunctionType.Copy
                    )

            # ---- u^T tiles for the AR term (rmsnorm1 + PE transpose);
            # the 2-row prefix runs after the m blocks so it stays off the
            # critical path ----
            for m in range(MB):
                uo = work.tile([P, D], F32, name="uo")
                rmsnorm_to(uo, xrows[:, m, :], P, dve_sq=m < 2)
                for c in range(DC):
                    pst = psum_pool.tile([P, D], F32, name="ps")[:, 0:P]
                    nc.tensor.transpose(pst, uo[:, c * P:(c + 1) * P], ident)
                    psum_copy(ut_ext[:, c, m, 2:P + 2], pst, c + 1)
            rmsnorm_to(u_pre, xp, 2)
            for c in range(DC):
                pst2 = psum_pool.tile([P, D], F32, name="ps")[:, 0:P]
                nc.tensor.transpose(
                    pst2[:, 0:2], u_pre[:, c * P:(c + 1) * P], ident[0:2, 0:2]
                )
                nc.vector.tensor_copy(ut_ext[:, c, 0, 0:2], pst2[:, 0:2])
            for m in range(1, MB):
                for c in range(DC):
                    nc.gpsimd.tensor_copy(
                        ut_ext[:, c, m, 0:2], ut_ext[:, c, m - 1, P:P + 2]
                    )

            # ---- AR term: all 4 row-blocks accumulate per-(tap, d-half) in
            # mut arrival order so the psum groups start on the first chunk ----
            psa = [psum_pool.tile([P, D], F32, name="ps") for _ in range(MB)]
            for t in range(KU):
                for h in range(2):
                    for m in range(MB):
                        for c in range(h * HC, (h + 1) * HC):
                            st = t == 0 and c == 0
                            fin = t == KU - 1 and c == DC - 1
                            lh = ut_ext[:, c, m, 2 - t:P + 2 - t]
                            nc.tensor.matmul(
                                psa[m][:, 0:F1], lhsT=lh,
                                rhs=mut_sb[:, t, c, 0:F1], start=st, stop=fin,
                            )
                            nc.tensor.matmul(
                                psa[m][:, F1:D], lhsT=lh,
                                rhs=mut_sb[:, t, c, F1:D], start=st, stop=fin,
                            )
            for m in range(MB):
                nc.vector.tensor_tensor(
                    x1p[:, m, :], x1rows[:, m, :], psa[m], mybir.AluOpType.add
                )

            # fc1 weight chunks + fc2 resident weights, in first-use order
            fws = []
            for jc in range(JC):
                fw = wstream.tile([P, DC, 2, P], FP8, name="fw")
                nc.sync.dma_start(fw, fc1[jc].rearrange("c p k f -> p c k f"))
                fws.append(fw)
                if jc == 8:
                    nc.sync.dma_start(fc2_sb, fc2.rearrange("c p d -> p c d"))

            # ---- y = rmsnorm2(x1) transposed, hi + compensation lo ----
            for m in range(MB):
                yf = work.tile([P, D], F32, name="uo")
                rmsnorm_to(yf, x1p[:, m, :], P)
                for c in range(DC):
                    pst = psum_pool.tile([P, D], F32, name="ps")[:, 0:P]
                    nc.tensor.transpose(pst, yf[:, c * P:(c + 1) * P], ident)
                    sl = slice(m * P, (m + 1) * P)
                    psum_copy(yt[:, c, 0, sl], pst, 1)
                    nc.vector.scalar_tensor_tensor(
                        yt[:, c, 1, sl], pst, 1.0, yt[:, c, 0, sl],
                        mybir.AluOpType.mult, mybir.AluOpType.subtract,
                    )

            # ---- fc1 + silu gate: (yh + yl) @ wh, 6 DR matmuls per half ----
            for jc in range(JC):
                ph1 = psum_pool.tile([P, D], F32, name="ps")[:, 0:F1]
                ph2 = psum_pool.tile([P, D], F32, name="ps")[:, 0:F1]
                for hh, ph in ((0, ph1), (1, ph2)):
                    i_mm = 0
                    for q in range(2):
                        for cp in range(DC // 2):
                            nc.tensor.matmul(
                                ph,
                                lhsT=fws[jc][:, 2 * cp:2 * cp + 2, hh, :],
                                rhs=yt[:, 2 * cp:2 * cp + 2, q, :],
                                start=i_mm == 0, stop=i_mm == DC - 1,
                                perf_mode=DR,
                            )
                            i_mm += 1
                sact = work.tile([P, F1], F32, name="sact")
                nc.scalar.activation(sact, ph2,
                                     mybir.ActivationFunctionType.Silu,
                                     scale=float(1.0 / MLP_SCALE))
                g32 = work.tile([P, F1], F32, name="g32")
                nc.vector.scalar_tensor_tensor(
                    g32, ph1, float(1.0 / MLP_SCALE), sact,
                    mybir.AluOpType.mult, mybir.AluOpType.mult,
                )
                nc.scalar.activation(
                    gt[:, jc, 0, :], g32, mybir.ActivationFunctionType.Copy
                )
                nc.vector.scalar_tensor_tensor(
                    gt[:, jc, 1, :], g32, 1.0, gt[:, jc, 0, :],
                    mybir.AluOpType.mult, mybir.AluOpType.subtract,
                )

            # ---- fc2 + residual: (gh + gl) @ f2h, m-outer so each row
            # block's residual add and output DMA overlap later matmuls ----
            for m in range(MB):
                po = psum_pool.tile([P, D], F32, name="ps")
                msl = slice(m * P, (m + 1) * P)
                ot = work.tile([P, D], F16, name="ot")
                for d0, d1 in ((0, F1), (F1, D)):
                    i_mm = 0
                    for q in range(2):
                        for jp in range(JC // 2):
                            nc.tensor.matmul(
                                po[:, d0:d1],
                                lhsT=gt[:, 2 * jp:2 * jp + 2, q, msl],
                                rhs=fc2_sb[:, 2 * jp:2 * jp + 2, d0:d1],
                                start=i_mm == 0, stop=i_mm == JC - 1,
                                perf_mode=DR,
                            )
                            i_mm += 1
                    nc.vector.scalar_tensor_tensor(
                        ot[:, d0:d1], po[:, d0:d1], float(1.0 / MLP_SCALE),
                        x1p[:, m, d0:d1],
                        mybir.AluOpType.mult, mybir.AluOpType.add)
                    nc.sync.dma_start(o[m * P:(m + 1) * P, d0:d1],
                                      ot[:, d0:d1])
    nc.compile()
    return nc


def _host_prep(V, sigma, M_u, M_phi_plus, M_phi_minus, rn1):
    """Per-core tap blocks + fused projection matrices for the parity-split
    conv. Core c owns filters (2c, 2c+1). rn1_w folds into the projection's
    contraction axis; sigma^0.25 folds into the taps.

    tw[d0, r, ko, chain, f*128 + po*64 + j] =
        taps_f[128*(d0 - 2*ko) + (2j + po) - 2r - chain]   (0 if idx < 0)
    (chain 0 reads even input rows, 1 odd; output cols parity-major po)
    wt[f, sd, cp, i, ko, d] = {Ws,Wd}[f][d, (2cp+ko)*128 + i] * W_SCALE
    """
    phi = np.fft.irfft(V.astype(np.complex128), n=NFFT, axis=0)[:SL]
    s4 = sigma.astype(np.float64) ** 0.25
    taps = (s4[None, :] * phi)                       # (SL, K)
    Ws = (M_phi_plus + M_phi_minus) * rn1[None, None, :]
    Wd = (M_phi_plus - M_phi_minus) * rn1[None, None, :]

    # col -> s_out map (parity-major)
    s_out = np.empty(P, np.int64)
    s_out[:64] = 2 * np.arange(64)
    s_out[64:] = 2 * np.arange(64) + 1

    d0v = np.arange(NB)[:, None, None, None, None]
    rv = np.arange(P)[None, :, None, None, None]
    kov = np.arange(2)[None, None, :, None, None]
    chv = np.arange(2)[None, None, None, :, None]
    colv = s_out[None, None, None, None, :]
    idx = 128 * (d0v - 2 * kov) + colv - 2 * rv - chv   # (NB,P,2,2,P)

    tw_cores = []
    wt_cores = []
    for core in range(NCORES):
        tw = np.zeros((NB, P, 2, 2, FPC * P), np.float32)
        wt = np.zeros((FPC, 2, DC // 2, P, 2, D), np.float32)
        for f in range(FPC):
            k = core * FPC + f
            tsc = (taps[:, k] * TAP_SCALE).astype(np.float64)
            blk = np.where(idx >= 0, tsc[np.clip(idx, 0, SL - 1)], 0.0)
            tw[:, :, :, :, f * P:(f + 1) * P] = blk
            for sd, W in ((0, Ws[k]), (1, Wd[k])):
                for cp in range(DC // 2):
                    for ko in range(2):
                        c = 2 * cp + ko
                        wt[f, sd, cp, :, ko, :] = (
                            W[:, c * P:(c + 1) * P].T * W_SCALE)
        tw_cores.append(tw.astype(FP8NP))
        wt_cores.append(wt.astype(FP8NP))
    return tw_cores, wt_cores


def kernel(x, V, sigma, M_u, M_phi_plus, M_phi_minus, rn1_w, rn2_w, fc1_w, fc2_w):
    x = np.ascontiguousarray(x, np.float32)
    if "p1" not in _cache:
        _cache["p1"] = _SpmdRunner(_build_phase1(), shared=("xb",), volatile=("xb",))
    if "p2" not in _cache:
        _cache["p2"] = _SpmdRunner(
            _build_phase2(), shared=("mut", "fc1", "fc2"),
            volatile=("xr", "x1r"))

    rn1 = np.ascontiguousarray(rn1_w, np.float32)
    rn2 = np.ascontiguousarray(rn2_w, np.float32)
    tw_cores, wt_cores = _host_prep(V, sigma, M_u, M_phi_plus, M_phi_minus, rn1)
    xb = x.astype(BF16NP)

    in_maps1 = [
        {"xb": xb, "tw": tw_cores[c], "wt": wt_cores[c]}
        for c in range(NCORES)
    ]
    r1 = _cache["p1"]
    sp_cat = r1.run_prepped(r1.prep(in_maps1))[0]
    if "reduce" not in _cache:
        import jax
        import jax.numpy as jnp
        from jax.sharding import NamedSharding, PartitionSpec
        sh = NamedSharding(r1._shardings["xb"].mesh, PartitionSpec())
        _cache["reduce"] = jax.jit(
            lambda spc, xx: xx + spc.reshape(NCORES, B, SL, D)
            .astype(jnp.float32).sum(0),
            out_shardings=sh,
        )
    x1 = np.asarray(_cache["reduce"](sp_cat, np.asarray(x)))

    # phase 2 inputs; rn1 folds into mut's contraction axis, rn2 into fc1's
    mut = np.zeros((KU, DC, P, D), np.float32)
    for t in range(KU):
        for c in range(DC):
            mut[t, c] = (M_u[t] * rn1[None, :])[:, c * P:(c + 1) * P].T
    mut = mut.astype(BF16NP)
    # fc1 layout (JC, DC, P, 2, P): [..., 0, :] = y half column block jc,
    # [..., 1, :] = gate half column block jc (fp8 hi plane only; the y-side
    # hi/lo split on-device compensates the activation quantization)
    f1s = np.ascontiguousarray(fc1_w, np.float32) * rn2[:, None] * MLP_SCALE
    hi8 = f1s.astype(FP8NP)
    fc1p = np.ascontiguousarray(
        np.transpose(hi8.reshape(DC, P, 2, JC, P), (3, 0, 1, 2, 4)))
    f2s = np.ascontiguousarray(fc2_w, np.float32).reshape(JC, P, D) * MLP_SCALE
    fc2 = np.ascontiguousarray(f2s.astype(FP8NP))

    x_rows = x.reshape(B * SL, D)
    x1_rows = x1.reshape(B * SL, D)
    in_maps2 = []
    for c in range(NCORES):
        r0 = c * RPC
        xr = np.zeros((RPC + 2, D), np.float32)
        xr[2:] = x_rows[r0:r0 + RPC]
        if r0 % SL != 0:
            xr[0:2] = x_rows[r0 - 2:r0]
        in_maps2.append({
            "xr": xr.astype(BF16NP),
            "x1r": np.ascontiguousarray(
                x1_rows[r0:r0 + RPC]).astype(np.float16),
            "mut": mut, "fc1": fc1p, "fc2": fc2,
        })
    res2 = _cache["p2"](in_maps2)
    out = np.concatenate(
        [res2[c]["o"] for c in range(NCORES)], axis=0
    ).astype(np.float32).reshape(B, SL, D)
    return out



# revision 34
# speedup vs baseline: 1.2720x; 1.0634x over previous
"""Trainium2 Bass kernel for the STU (spectral transform unit) dense-transformer block.

Algorithm (validated against the jax reference in fp64 numpy):
  The FFT causal conv is rewritten as a block-Toeplitz matmul. For each of the
  K=16 filters and each sign branch (the alternating-sign branch folds into the
  filter taps: T^-[s,s'] = phi[s-s'] * (-1)^(s-s')), the causal conv is
    U_br = T_br @ u,  T_br block-Toeplitz with 16 distinct 128x128 blocks.
  sigma^(1/4) folds into the taps. The (k,i)->d projection contracts U with
  M_phi_{plus,minus}; the KU=3 autoregressive taps are shifted-u projections
  with M_u. MLP is a standard gated MLP.

Sharding (8 cores, no cross-core communication, host-side reduce between two
uniform SPMD programs):
  Phase 1: filter-branch-parallel. Core c computes conv + projection for its 4
           of the 32 (k, sign) branches over the full (B, SL): partial spectral.
  Host:    x1 = x + sum_c partial_c
  Phase 2: row-parallel. Core c owns 512 of the 4096 (b, s) rows: adds the AR
           term and computes the gated MLP + residual for its rows.

Precision: the conv runs in fp8 (output magnitude ~0.05 -> noise negligible);
fc1 and fc2 run as compensated hi+lo fp8 splits (h = yh@wh + DoubleRow-paired
cross terms wl@yh + wh@yl, dropping only the second-order yl@wl product),
which gets fp8 DoubleRow matmul rates at bf16-class accuracy. The AR term
stays bf16: its shifted u^T windows would have odd byte offsets in fp8,
which the Ldweights ISA rejects. Measured end-to-end error: 3.9e-3
scale-relative vs the 2e-2 harness gate. rn1/rn2 rmsnorm weights are folded
into the downstream contraction weights host-side (they commute through the
seq-dim conv / shifts).

Schedule notes (all targets are the InstructionCostModel timeline):
 - DMA is a serial ~360GB/s resource; transfers are emitted in the order
   compute needs them (x row-blocks and tw delta-chunks interleaved, weights
   after first-use rows), which removes the 24us/30us startup stalls the
   v1 kernel had.
 - PSUM->SBUF drain copies alternate across DVE/Act (GPSIMD cannot read
   PSUM); Pool takes the SBUF->SBUF rmsnorm multiplies.
 - Phase-1 software-pipelines the next block's conv between conv(I) and
   proj(I) (depth 2-3 for the short early blocks) so the PE covers the
   psum-drain latency; projection iterates cp-outer so its first matmuls
   depend only on the first conv psum drains.
 - Phase-1 warms the PE p-state with dummy matmuls while the first input
   blocks stream in; phase-2 finalizes fc2 m-outer and d-half-outer so each
   row block's residual add and output DMA overlap later matmuls.
 - Both phases issue dummy Square/Sqrt activations at the head of the
   Activation queue so the act-table loads finish before the first rmsnorm
   needs them.
"""

import numpy as np
import ml_dtypes

import concourse.bacc as bacc
import concourse.tile as tile
from concourse import mybir
from concourse.bass_utils import run_bass_kernel_spmd  # noqa: F401 (debug path)
from concourse.masks import make_identity


class _SpmdRunner:
    """Cached-jit SPMD executor: trace/compile once, then repeat calls only
    pay input upload + execution (mirrors bass2jax.run_bass_via_pjrt).

    ``shared`` names inputs that are identical on every core: they are fed
    replicated (host uploads one copy) instead of 8x-concatenated."""

    def __init__(self, nc, shared=(), volatile=()):
        import jax
        import concourse.mybir as _mb
        from concourse.bass2jax import (
            install_neuronx_cc_hook, _bass_exec_p, partition_id_tensor,
        )
        from jax.experimental.shard_map import shard_map
        from jax.sharding import Mesh, PartitionSpec

        install_neuronx_cc_hook()
        self.nc = nc
        assert nc.dbg_addr is None
        pid_name = (nc.partition_id_tensor.name
                    if nc.partition_id_tensor is not None else None)
        in_names, out_names, out_avals = [], [], []
        for alloc in nc.m.functions[0].allocations:
            if not isinstance(alloc, mybir.MemoryLocationSet):
                continue
            name = alloc.memorylocations[0].name
            if alloc.kind == "ExternalInput":
                if name != pid_name:
                    in_names.append(name)
            elif alloc.kind == "ExternalOutput":
                out_names.append(name)
                out_avals.append(jax.core.ShapedArray(
                    tuple(alloc.tensor_shape), mybir.dt.np(alloc.dtype)))
        self.in_names, self.out_names, self.out_avals = in_names, out_names, out_avals
        self.shared = frozenset(shared)
        self.volatile = frozenset(volatile)
        self._dev_cache = {}
        n_params = len(in_names)
        all_names = tuple(in_names + out_names)
        if pid_name is not None:
            all_names = all_names + (pid_name,)

        def _body(*args):
            args = list(args)
            if pid_name is not None:
                args.append(partition_id_tensor())
            return tuple(_bass_exec_p.bind(
                *args,
                out_avals=tuple(out_avals),
                in_names=all_names,
                out_names=tuple(out_names),
                lowering_input_output_aliases=(),
                sim_require_finite=True,
                sim_require_nnan=True,
                nc=nc,
            ))

        import jax.numpy as jnp
        from jax.sharding import NamedSharding
        devices = jax.devices()[:NCORES]
        mesh = Mesh(np.asarray(devices), ("core",))
        rep = PartitionSpec()
        core = PartitionSpec("core")
        in_specs = tuple(
            rep if nm in self.shared else core for nm in in_names
        ) + (core,) * len(out_names)
        out_specs = (core,) * len(out_names)
        donate = tuple(range(n_params, n_params + len(out_names)))
        self._fn = jax.jit(
            shard_map(_body, mesh=mesh, in_specs=in_specs, out_specs=out_specs,
                      check_rep=False),
            donate_argnums=donate, keep_unused=True,
        )
        self._zeros_fn = jax.jit(
            lambda: tuple(
                jnp.zeros((NCORES * a.shape[0], *a.shape[1:]), a.dtype)
                for a in out_avals
            ),
            out_shardings=tuple(
                NamedSharding(mesh, core) for _ in out_avals
            ),
        )
        self._shardings = {
            nm: NamedSharding(mesh, rep if nm in self.shared else core)
            for nm in in_names
        }

    def prep(self, in_maps):
        import hashlib
        import jax
        ins = []
        for nm in self.in_names:
            if nm in self.shared:
                arr = np.ascontiguousarray(in_maps[0][nm])
            else:
                arr = np.concatenate(
                    [np.asarray(in_maps[c][nm]) for c in range(NCORES)], axis=0)
            if nm in self.volatile:
                ins.append(arr)
                continue
            key = (nm, hashlib.md5(arr.tobytes()).hexdigest())
            dev = self._dev_cache.get(key)
            if dev is None:
                self._dev_cache.clear() if len(self._dev_cache) > 32 else None
                dev = jax.device_put(arr, self._shardings[nm])
                self._dev_cache[key] = dev
            ins.append(dev)
        return ins

    def run_prepped(self, ins):
        return self._fn(*ins, *self._zeros_fn())

    def __call__(self, in_maps):
        out_arrs = self.run_prepped(self.prep(in_maps))
        return [
            {nm: np.asarray(out_arrs[i]).reshape(NCORES, *self.out_avals[i].shape)[c]
             for i, nm in enumerate(self.out_names)}
            for c in range(NCORES)
        ]

BF16NP = ml_dtypes.bfloat16
FP8NP = ml_dtypes.float8_e4m3
TAP_SCALE = 1024.0
UT_SCALE = 32.0      # psum (TAP_SCALE*U) -> fp8 ut tiles scale factor: 32/1024
W_SCALE = 16.0       # projection weights scaled by 16 for fp8 range
SP_SCALE = UT_SCALE * W_SCALE  # spectral psum carries 32*16 = 512x
MLP_SCALE = 16.0     # fc1 hi/lo fp8 weights carry 16x for fp8 range
F32 = mybir.dt.float32
F32R = mybir.dt.float32r
F16 = mybir.dt.float16
BF = mybir.dt.bfloat16
FP8 = mybir.dt.float8e4

B, SL, D, K, KU = 2, 2048, 768, 16, 3
NFFT, EPS, P, H = 4096, 1e-5, 128, 3072
NB = SL // P            # 16 seq blocks
DC = D // P             # 6 d-chunks
NBR = 2 * K             # 32 conv branches
NCORES = 8
BPC = NBR // NCORES     # 4 branches per core
FPC = K // NCORES       # 2 filters per core (parity-fused conv)
NSB = SL // (2 * P)     # 8 superblocks (256 rows) per batch
RPC = (B * SL) // NCORES  # 512 rows per core
MB = RPC // P           # 4 row blocks per core in phase 2
JC = H // P             # 24 hidden chunks
F1 = 512                # free-dim split of D=768 into 512+256
DR = mybir.MatmulPerfMode.DoubleRow

_cache: dict = {}


def _build_phase1():
    """Parity-fused spectral conv: since T^- = D T D (D = alt signs), the
    even/odd half-convs C_e, C_o of each filter determine both sign branches:
      spectral[even s] = C_e Ws + C_o Wd,  spectral[odd s] = C_e Wd + C_o Ws
    with Ws = W+ + W-, Wd = W+ - W-. The conv FLOPs halve (each half-conv
    reads only half the input rows); the +- reconstruction is absorbed into
    the projection weights at no extra cost. Projection psums pack the
    same-parity rows of two consecutive 128-row blocks (conv output columns
    are emitted parity-major so the proj lhsT stays a contiguous 128-slice).
    Each core owns FPC=2 of the 16 filters."""
    nc = bacc.Bacc("TRN2", target_bir_lowering=False, debug=False, num_devices=NCORES)
    xb = nc.dram_tensor("xb", (B, SL, D), BF, kind="ExternalInput").ap()
    # tw[d0, r, ko, chain, f*128+col]: tap block pair (delta=d0-2*ko), chain
    # 0=even-input 1=odd-input, col parity-major within each filter's 128
    tw = nc.dram_tensor("tw", (NB, P, 2, 2, FPC * P), FP8, kind="ExternalInput").ap()
    # wt[f, sd(0=Ws,1=Wd), cp, r, ko, d_out]
    wt = nc.dram_tensor("wt", (FPC, 2, DC // 2, P, 2, D), FP8, kind="ExternalInput").ap()
    sp = nc.dram_tensor("sp", (B, SL, D), F16, kind="ExternalOutput").ap()

    with tile.TileContext(nc) as tc:
        with (
            tc.tile_pool(name="const", bufs=1) as const_pool,
            tc.tile_pool(name="ubuf", bufs=1) as ubuf_pool,
            tc.tile_pool(name="work", bufs=9) as work,
            tc.tile_pool(name="drain", bufs=4) as drain_pool,
            tc.tile_pool(name="spill", bufs=3) as spill_pool,
            tc.tile_pool(name="psum_u", bufs=4, space="PSUM") as psum_u_pool,
            tc.tile_pool(name="psum_sp", bufs=2, space="PSUM") as psum_sp_pool,
        ):
            eps_sb = const_pool.tile([P, 1], F32)
            nc.vector.memset(eps_sb, float(EPS))
            dummy = const_pool.tile([P, 1], F32, name="dummy")
            nc.scalar.activation(
                dummy, eps_sb, mybir.ActivationFunctionType.Square)
            nc.scalar.activation(
                dummy, dummy, mybir.ActivationFunctionType.Sqrt)
            tw_sb = const_pool.tile([P, NB, 2, 2, FPC * P], FP8)
            wt_sb = const_pool.tile([P, FPC, 2, DC // 2, 2, D], FP8)

            # one persistent fp8 u tile per (b, parity, even/odd-block pair):
            # u_t[b][par][jp][:, ko, :] holds rmsnormed rows
            # 256*(2*jp+ko) + 2r + par of batch b
            u_t = [[[ubuf_pool.tile([P, 2, D], FP8, name=f"u{b}_{par}_{jp}")
                     for jp in range(NB // 4)] for par in range(2)]
                   for b in range(B)]

            def jprep(b, par, blk, dve_sq=False, seng=0):
                """strided x row DMA (one parity class) -> rmsnorm -> fp8.
                (rn1_w is folded into the projection weights host-side.)
                seng: engine for the final scale (0=Pool, 1=DVE, 2=Act) --
                the head jpreps fan out so the Pool queue doesn't serialize
                the first conv's inputs."""
                xt = work.tile([P, D], BF, name="xt")
                r0 = 256 * blk + par
                nc.sync.dma_start(xt, xb[b, r0:r0 + 255:2, :])
                sq = work.tile([P, D], F32, name="sq")
                ms = work.tile([P, 1], F32, name="ms")
                if dve_sq:
                    nc.vector.scalar_tensor_tensor(
                        sq, xt, 1.0, xt, mybir.AluOpType.mult,
                        mybir.AluOpType.mult, accum_out=ms,
                    )
                else:
                    nc.scalar.activation(
                        sq, xt, mybir.ActivationFunctionType.Square,
                        accum_out=ms,
                    )
                nc.scalar.activation(
                    ms, ms, mybir.ActivationFunctionType.Sqrt,
                    bias=eps_sb, scale=1.0 / D,
                )
                nc.vector.reciprocal(ms, ms)
                dst = u_t[b][par][blk // 2][:, blk % 2, :]
                if seng == 1:
                    nc.vector.tensor_scalar_mul(dst, xt, ms)
                elif seng == 2:
                    nc.scalar.activation(
                        dst, xt, mybir.ActivationFunctionType.Copy, scale=ms)
                else:
                    nc.gpsimd.tensor_scalar_mul(dst, xt, ms)

            # PE warmup: dummy matmuls on a zero tile ramp the tensor
            # engine p-state while the first input blocks stream in (memset
            # on gpsimd so the PE isn't gated on the busier DVE queue)
            wz = const_pool.tile([P, 2, 2 * P], FP8, name="wz")
            nc.gpsimd.memset(wz, 0.0)
            wps = psum_u_pool.tile([P, 2 * P], F32, name="psu")
            NW = 34
            for i in range(NW):
                nc.tensor.matmul(wps, lhsT=wz[:, :, 0:P], rhs=wz,
                                 start=i == 0, stop=i == NW - 1, perf_mode=DR)

            # prologue: x rows for conv blocks 0,1 stream before the tap
            # blocks (the taps are only needed once the PE issues Ldweights);
            # scale engines fan out so no single queue serializes readiness
            jprep(0, 0, 0, dve_sq=True, seng=1)
            jprep(0, 1, 0, seng=2)
            nc.sync.dma_start(tw_sb[:, 0], tw[0])
            nc.sync.dma_start(tw_sb[:, 1], tw[1])
            jprep(0, 0, 1, seng=0)
            jprep(0, 1, 1, seng=1)
            nc.sync.dma_start(
                wt_sb[:, :, :, 0, :, :],
                wt[:, :, 0].rearrange("f s p k d -> p f s k d"))
            next_blk = [2, 0]

            def conv_block(b, I, ut_sb):
                """both half-conv chains for seq block I into one psum; drain
                into the superblock ut tile (cols parity-major per filter)."""
                npair = I // 4 + 1
                half = I % 2
                for c in range(DC):
                    ps = psum_u_pool.tile([P, 2 * FPC * P], F32, name="psu")
                    for chain in range(2):
                        dst = ps[:, chain * FPC * P:(chain + 1) * FPC * P]
                        if b == 0 and I < 2:
                            # deltas (I, I-2): the ko=1 half is all-zero taps;
                            # a plain (non-DR) matmul on the first pair-half
                            # depends only on x rows 0..255, so the first
                            # convs start ~2us earlier
                            nc.tensor.matmul(
                                dst,
                                lhsT=u_t[b][chain][0][:, 0, c * P:(c + 1) * P],
                                rhs=tw_sb[:, I, 0, chain, :],
                                start=True, stop=True,
                            )
                            continue
                        for Jp in range(npair):
                            nc.tensor.matmul(
                                dst,
                                lhsT=u_t[b][chain][Jp][:, :, c * P:(c + 1) * P],
                                rhs=tw_sb[:, I - 4 * Jp, :, chain, :],
                                start=(Jp == 0),
                                stop=(Jp == npair - 1),
                                perf_mode=DR,
                            )
                    # psum free dim = (chain, f, par, 64); ut free dim per
                    # chunk = (chain*FPC+f, par, 128=(half,64))
                    pv = ps.rearrange("p (s q j) -> p s q j", s=2 * FPC, q=2)
                    dst = ut_sb[:, c, :, :, half * 64:half * 64 + 64]
                    if c % 2 == 0:
                        nc.vector.tensor_scalar_mul(
                            dst, pv, float(UT_SCALE / TAP_SCALE))
                    else:
                        nc.scalar.activation(
                            dst, pv, mybir.ActivationFunctionType.Copy,
                            scale=float(UT_SCALE / TAP_SCALE),
                        )

            def proj_block(b, Ip, par, ut_sb, last=False):
                """one parity's projection for superblock Ip: 128 same-parity
                rows of blocks (2Ip, 2Ip+1); sd picks Ws for the matching
                parity chain, Wd for the crossed one."""
                psp = psum_sp_pool.tile([P, D], F32, name="psp")
                sp_t = spill_pool.tile([P, D], F16, name="spt")
                r0 = 256 * Ip + par
                if last:
                    # tail only: sequential F1/256 chains so the F1 drain +
                    # DMA overlap the 256 matmuls (elsewhere this loses --
                    # the drain read blocks the tile's second chain)
                    halves = [[(0, F1)], [(F1, D)]]
                else:
                    halves = [[(0, F1), (F1, D)]]
                for grp_i, grp in enumerate(halves):
                    pst = psp
                    if last and grp_i == 1:
                        # separate psum tile: no tile-level WAR against the
                        # F1 chain's drain read
                        pst = psum_sp_pool.tile([P, D], F32, name="psp")
                    i_mm = 0
                    n_mm = 2 * FPC * (DC // 2)
                    for cp in range(DC // 2):
                        for st_i in range(2 * FPC):
                            chain, f = divmod(st_i, FPC)
                            sd = par if chain == 0 else 1 - par
                            lh = ut_sb[:, 2 * cp:2 * cp + 2, st_i, par, :]
                            for d0, d1 in grp:
                                nc.tensor.matmul(
                                    pst[:, d0:d1], lhsT=lh,
                                    rhs=wt_sb[:, f, sd, cp, :, d0:d1],
                                    start=i_mm == 0, stop=i_mm == n_mm - 1,
                                    perf_mode=DR,
                                )
                            i_mm += 1
                    for gi, (d0, d1) in enumerate(grp):
                        if (par + gi + grp_i) % 2 == 0:
                            nc.scalar.activation(
                                sp_t[:, d0:d1], pst[:, d0:d1],
                                mybir.ActivationFunctionType.Copy,
                                scale=float(1.0 / SP_SCALE),
                            )
                        else:
                            nc.vector.tensor_scalar_mul(
                                sp_t[:, d0:d1], pst[:, d0:d1],
                                float(1.0 / SP_SCALE))
                        nc.sync.dma_start(
                            sp[b, r0:r0 + 255:2, d0:d1], sp_t[:, d0:d1])

            # software pipeline: emit convs ahead of projections so the PE
            # covers psum-drain latency; a superblock's proj needs both its
            # conv blocks drained
            from collections import deque
            pend = deque()
            sched = [(0, Ip) for Ip in range(NSB)]
            sched += [(1, Ip) for Ip in reversed(range(NSB))]
            for b, Ip in sched:
                if True:
                    ut_sb = drain_pool.tile([P, DC, 2 * FPC, 2, P], FP8,
                                            name="ut")
                    for half in range(2):
                        I = 2 * Ip + half
                        # pace the DMA queue: tw chunk I+2, remaining wt
                        # chunks, upcoming u row blocks
                        if b == 0 and I + 2 < NB:
                            nc.sync.dma_start(tw_sb[:, I + 2], tw[I + 2])
                        if b == 0 and 1 <= I < 3:
                            cp = I
                            nc.sync.dma_start(
                                wt_sb[:, :, :, cp, :, :],
                                wt[:, :, cp].rearrange(
                                    "f s p k d -> p f s k d"))
                        need = min(2 * (((I + 4) // 4)) + 1, NB // 2 - 1)
                        while next_blk[b] <= need:
                            jprep(b, 0, next_blk[b])
                            jprep(b, 1, next_blk[b])
                            next_blk[b] += 1
                        if b == 0 and I >= 10:
                            while next_blk[1] <= min(I - 10, NB // 2 - 1):
                                jprep(1, 0, next_blk[1])
                                jprep(1, 1, next_blk[1])
                                next_blk[1] += 1
                        conv_block(b, I, ut_sb)
                    pend.append((b, Ip, ut_sb))
                    depth = 2 if Ip < 3 else 1
                    while len(pend) > depth:
                        pb, pIp, put = pend.popleft()
                        proj_block(pb, pIp, 0, put)
                        proj_block(pb, pIp, 1, put)
            while pend:
                pb, pIp, put = pend.popleft()
                proj_block(pb, pIp, 0, put)
                proj_block(pb, pIp, 1, put,
                           last=pb == B - 1 and pIp == NSB - 1)
    nc.compile()
    return nc


def _build_phase2():
    """Row-parallel AR + gated MLP. The MLP runs fp8 weights-hi-only with
    activation-side compensation: h = (yh + yl) @ wh (yl = y - fp8(y)), so
    the only dropped term is y @ (w - fp8(w)) -- a fixed small weight
    perturbation. Same for fc2 with g hi/lo. Halves both the matmul count
    (vs the hi/lo cross scheme) and the weight DMA. The AR term stays bf16
    (shifted fp8 u^T windows would need odd byte offsets in Ldweights)."""
    nc = bacc.Bacc("TRN2", target_bir_lowering=False, debug=False, num_devices=NCORES)
    xr = nc.dram_tensor("xr", (RPC + 2, D), BF, kind="ExternalInput").ap()
    x1r = nc.dram_tensor("x1r", (RPC, D), F16, kind="ExternalInput").ap()
    mut = nc.dram_tensor("mut", (KU, DC, P, D), BF, kind="ExternalInput").ap()
    fc1 = nc.dram_tensor("fc1", (JC, DC, P, 2, P), FP8, kind="ExternalInput").ap()
    fc2 = nc.dram_tensor("fc2", (JC, P, D), FP8, kind="ExternalInput").ap()
    o = nc.dram_tensor("o", (RPC, D), F16, kind="ExternalOutput").ap()

    with tile.TileContext(nc) as tc:
        with (
            tc.tile_pool(name="const", bufs=1) as const_pool,
            tc.tile_pool(name="persist", bufs=1) as persist,
            tc.tile_pool(name="work", bufs=6) as work,
            tc.tile_pool(name="wstream", bufs=5) as wstream,
            tc.tile_pool(name="psum", bufs=8, space="PSUM") as psum_pool,
        ):
            ident = const_pool.tile([P, P], F32)
            make_identity(nc, ident)
            eps_sb = const_pool.tile([P, 1], F32)
            nc.vector.memset(eps_sb, float(EPS))
            dummy = const_pool.tile([P, 1], F32, name="dummy")
            nc.scalar.activation(
                dummy, eps_sb, mybir.ActivationFunctionType.Square)
            nc.scalar.activation(
                dummy, dummy, mybir.ActivationFunctionType.Sqrt)

            # PE warmup: ramp the p-state while the first x rows stream in
            wz = const_pool.tile([P, 2, 2 * P], FP8, name="wz")
            nc.gpsimd.memset(wz, 0.0)
            wps = psum_pool.tile([P, 2 * P], F32, name="ps")
            for i in range(20):
                nc.tensor.matmul(wps, lhsT=wz[:, :, 0:P], rhs=wz,
                                 start=i == 0, stop=i == 19, perf_mode=DR)

            ut_ext = persist.tile([P, DC, MB, P + 2], BF)
            x1p = persist.tile([P, MB, D], F32)
            xrows = persist.tile([P, MB, D], BF)
            x1rows = persist.tile([P, MB, D], F16)
            yt = persist.tile([P, DC, 2, MB * P], FP8)
            gt = persist.tile([P, JC, 2, MB * P], FP8)
            mut_sb = persist.tile([P, KU, DC, D], BF)
            fc2_sb = persist.tile([P, JC, D], FP8)

            def rmsnorm_to(dst, src, rows, dve_sq=False, seng=0):
                """dst = src / rms(src); the rmsnorm weight is folded into
                the downstream contraction weights host-side."""
                sq = work.tile([P, D], F32, name="sq")
                ms = work.tile([P, 1], F32, name="ms")
                if dve_sq:
                    nc.vector.scalar_tensor_tensor(
                        sq[:rows], src[:rows], 1.0, src[:rows],
                        mybir.AluOpType.mult, mybir.AluOpType.mult,
                        accum_out=ms[:rows],
                    )
                else:
                    nc.scalar.activation(
                        sq[:rows], src[:rows],
                        mybir.ActivationFunctionType.Square,
                        accum_out=ms[:rows],
                    )
                nc.scalar.activation(
                    ms[:rows], ms[:rows], mybir.ActivationFunctionType.Sqrt,
                    bias=eps_sb[:rows], scale=1.0 / D,
                )
                nc.vector.reciprocal(ms[:rows], ms[:rows])
                if seng == 1:
                    nc.vector.tensor_scalar_mul(dst, src[:rows], ms[:rows])
                elif seng == 2:
                    nc.scalar.activation(
                        dst, src[:rows], mybir.ActivationFunctionType.Copy,
                        scale=ms[:rows])
                else:
                    nc.gpsimd.tensor_scalar_mul(dst, src[:rows], ms[:rows])

            # DMA queue front: prefix rows, the 4 x row blocks, mut taps (in
            # per-tap-half chunks so AR starts on the first), then x1 rows;
            # fc1/fc2 stream later in the fws loop
            u_pre = persist.tile([2, D], F32)
            xp = work.tile([P, D], BF, name="xt")[:2]
            nc.sync.dma_start(xp, xr[0:2, :])
            for m in range(MB):
                nc.sync.dma_start(
                    xrows[:, m, :], xr[2 + m * P: 2 + (m + 1) * P, :])
            HC = DC // 2
            for t in range(KU):
                for c in range(DC):
                    nc.sync.dma_start(
                        mut_sb[:, t, c, :],
                        mut[t, c].rearrange("p d -> p d"),
                    )
            for m in range(MB):
                nc.sync.dma_start(x1rows[:, m, :], x1r[m * P:(m + 1) * P, :])

            def psum_copy(dst, src_ps, idx):
                if idx % 2 == 0:
                    nc.vector.tensor_copy(dst, src_ps)
                else:
                    nc.scalar.activation(
                        dst, src_ps, mybir.ActivationFunctionType.Copy
                    )

            # ---- u^T tiles for the AR term (rmsnorm1 + PE transpose);
            # the 2-row prefix runs after the m blocks so it stays off the
            # critical path ----
            for m in range(MB):
                uo = work.tile([P, D], F32, name="uo")
                rmsnorm_to(uo, xrows[:, m, :], P, dve_sq=m % 2 == 0,
                           seng=(1, 2, 0, 0)[m])
                for c in range(DC):
                    pst = psum_pool.tile([P, F1], F32, name="ps")[:, 0:P]
                    nc.tensor.transpose(pst, uo[:, c * P:(c + 1) * P], ident)
                    psum_copy(ut_ext[:, c, m, 2:P + 2], pst, c + 1)
            rmsnorm_to(u_pre, xp, 2)
            for c in range(DC):
                pst2 = psum_pool.tile([P, F1], F32, name="ps")[:, 0:P]
                nc.tensor.transpose(
                    pst2[:, 0:2], u_pre[:, c * P:(c + 1) * P], ident[0:2, 0:2]
                )
                nc.vector.tensor_copy(ut_ext[:, c, 0, 0:2], pst2[:, 0:2])
            for m in range(1, MB):
                for c in range(DC):
                    nc.gpsimd.tensor_copy(
                        ut_ext[:, c, m, 0:2], ut_ext[:, c, m - 1, P:P + 2]
                    )

            # ---- AR term: all 4 row-blocks accumulate per-(tap, d-half) in
            # mut arrival order so the psum groups start on the first chunk ----
            psa = [(psum_pool.tile([P, F1], F32, name="ps"),
                    psum_pool.tile([P, F1], F32, name="ps"))
                   for _ in range(MB)]
            for t in range(KU):
                for h in range(2):
                    for m in range(MB):
                        for c in range(h * HC, (h + 1) * HC):
                            st = t == 0 and c == 0
                            fin = t == KU - 1 and c == DC - 1
                            lh = ut_ext[:, c, m, 2 - t:P + 2 - t]
                            nc.tensor.matmul(
                                psa[m][0], lhsT=lh,
                                rhs=mut_sb[:, t, c, 0:F1], start=st, stop=fin,
                            )
                            nc.tensor.matmul(
                                psa[m][1][:, 0:D - F1], lhsT=lh,
                                rhs=mut_sb[:, t, c, F1:D], start=st, stop=fin,
                            )
            for m in range(MB):
                nc.vector.tensor_tensor(
                    x1p[:, m, 0:F1], x1rows[:, m, 0:F1], psa[m][0],
                    mybir.AluOpType.add)
                nc.vector.tensor_tensor(
                    x1p[:, m, F1:D], x1rows[:, m, F1:D],
                    psa[m][1][:, 0:D - F1], mybir.AluOpType.add)

            # fc1 weight chunks + fc2 resident weights, in first-use order
            fws = []
            for jc in range(JC):
                fw = wstream.tile([P, DC, 2, P], FP8, name="fw")
                nc.sync.dma_start(fw, fc1[jc].rearrange("c p k f -> p c k f"))
                fws.append(fw)
                if jc in (10, 14, 18, 22):
                    q4 = JC // 4
                    qi = (jc - 10) // 4
                    nc.sync.dma_start(
                        fc2_sb[:, qi * q4:(qi + 1) * q4, :],
                        fc2[qi * q4:(qi + 1) * q4].rearrange("c p d -> p c d"))

            # ---- y = rmsnorm2(x1) transposed, hi + compensation lo ----
            for m in range(MB):
                yf = work.tile([P, D], F32, name="uo")
                rmsnorm_to(yf, x1p[:, m, :], P, dve_sq=m % 2 == 0,
                           seng=(1, 2, 0, 1)[m])
                for c in range(DC):
                    pst = psum_pool.tile([P, F1], F32, name="ps")[:, 0:P]
                    nc.tensor.transpose(pst, yf[:, c * P:(c + 1) * P], ident)
                    sl = slice(m * P, (m + 1) * P)
                    psum_copy(yt[:, c, 0, sl], pst, 1)
                    nc.vector.scalar_tensor_tensor(
                        yt[:, c, 1, sl], pst, 1.0, yt[:, c, 0, sl],
                        mybir.AluOpType.mult, mybir.AluOpType.subtract,
                    )

            # ---- fc1 + silu gate: (yh + yl) @ wh, 6 DR matmuls per half ----
            for jc in range(JC):
                ph1 = psum_pool.tile([P, F1], F32, name="ps")
                ph2 = psum_pool.tile([P, F1], F32, name="ps")
                for hh, ph in ((0, ph1), (1, ph2)):
                    i_mm = 0
                    for q in range(2):
                        for cp in range(DC // 2):
                            nc.tensor.matmul(
                                ph,
                                lhsT=fws[jc][:, 2 * cp:2 * cp + 2, hh, :],
                                rhs=yt[:, 2 * cp:2 * cp + 2, q, :],
                                start=i_mm == 0, stop=i_mm == DC - 1,
                                perf_mode=DR,
                            )
                            i_mm += 1
                sact = work.tile([P, F1], F32, name="sact")
                nc.scalar.activation(sact, ph2,
                                     mybir.ActivationFunctionType.Silu,
                                     scale=float(1.0 / MLP_SCALE))
                g32 = work.tile([P, F1], F32, name="g32")
                nc.vector.scalar_tensor_tensor(
                    g32, ph1, float(1.0 / MLP_SCALE), sact,
                    mybir.AluOpType.mult, mybir.AluOpType.mult,
                )
                nc.scalar.activation(
                    gt[:, jc, 0, :], g32, mybir.ActivationFunctionType.Copy
                )
                nc.vector.scalar_tensor_tensor(
                    gt[:, jc, 1, :], g32, 1.0, gt[:, jc, 0, :],
                    mybir.AluOpType.mult, mybir.AluOpType.subtract,
                )

            # ---- fc2 + residual: (gh + gl) @ f2h, m-outer so each row
            # block's residual add and output DMA overlap later matmuls ----
            for m in range(MB):
                msl = slice(m * P, (m + 1) * P)
                ot = work.tile([P, D], F16, name="ot")
                for hi, (d0, d1) in enumerate(((0, F1), (F1, D))):
                    po = psum_pool.tile([P, F1], F32, name="ps")[:, 0:d1 - d0]
                    i_mm = 0
                    for q in range(2):
                        for jp in range(JC // 2):
                            nc.tensor.matmul(
                                po,
                                lhsT=gt[:, 2 * jp:2 * jp + 2, q, msl],
                                rhs=fc2_sb[:, 2 * jp:2 * jp + 2, d0:d1],
                                start=i_mm == 0, stop=i_mm == JC - 1,
                                perf_mode=DR,
                            )
                            i_mm += 1
                    nc.vector.scalar_tensor_tensor(
                        ot[:, d0:d1], po, float(1.0 / MLP_SCALE),
                        x1p[:, m, d0:d1],
                        mybir.AluOpType.mult, mybir.AluOpType.add)
                    nc.sync.dma_start(o[m * P:(m + 1) * P, d0:d1],
                                      ot[:, d0:d1])
    nc.compile()
    return nc


def _host_prep(V, sigma, M_u, M_phi_plus, M_phi_minus, rn1):
    """Per-core tap blocks + fused projection matrices for the parity-split
    conv. Core c owns filters (2c, 2c+1). rn1_w folds into the projection's
    contraction axis; sigma^0.25 folds into the taps.

    tw[d0, r, ko, chain, f*128 + po*64 + j] =
        taps_f[128*(d0 - 2*ko) + (2j + po) - 2r - chain]   (0 if idx < 0)
    (chain 0 reads even input rows, 1 odd; output cols parity-major po)
    wt[f, sd, cp, i, ko, d] = {Ws,Wd}[f][d, (2cp+ko)*128 + i] * W_SCALE
    """
    phi = np.fft.irfft(V.astype(np.complex128), n=NFFT, axis=0)[:SL]
    s4 = sigma.astype(np.float64) ** 0.25
    taps = (s4[None, :] * phi)                       # (SL, K)
    Ws = (M_phi_plus + M_phi_minus) * rn1[None, None, :]
    Wd = (M_phi_plus - M_phi_minus) * rn1[None, None, :]

    # col -> s_out map (parity-major)
    s_out = np.empty(P, np.int64)
    s_out[:64] = 2 * np.arange(64)
    s_out[64:] = 2 * np.arange(64) + 1

    d0v = np.arange(NB)[:, None, None, None, None]
    rv = np.arange(P)[None, :, None, None, None]
    kov = np.arange(2)[None, None, :, None, None]
    chv = np.arange(2)[None, None, None, :, None]
    colv = s_out[None, None, None, None, :]
    idx = 128 * (d0v - 2 * kov) + colv - 2 * rv - chv   # (NB,P,2,2,P)

    tw_cores = []
    wt_cores = []
    for core in range(NCORES):
        tw = np.zeros((NB, P, 2, 2, FPC * P), np.float32)
        wt = np.zeros((FPC, 2, DC // 2, P, 2, D), np.float32)
        for f in range(FPC):
            k = core * FPC + f
            tsc = (taps[:, k] * TAP_SCALE).astype(np.float64)
            blk = np.where(idx >= 0, tsc[np.clip(idx, 0, SL - 1)], 0.0)
            tw[:, :, :, :, f * P:(f + 1) * P] = blk
            for sd, W in ((0, Ws[k]), (1, Wd[k])):
                for cp in range(DC // 2):
                    for ko in range(2):
                        c = 2 * cp + ko
                        wt[f, sd, cp, :, ko, :] = (
                            W[:, c * P:(c + 1) * P].T * W_SCALE)
        tw_cores.append(tw.astype(FP8NP))
        wt_cores.append(wt.astype(FP8NP))
    return tw_cores, wt_cores


def kernel(x, V, sigma, M_u, M_phi_plus, M_phi_minus, rn1_w, rn2_w, fc1_w, fc2_w):
    x = np.ascontiguousarray(x, np.float32)
    if "p1" not in _cache:
        _cache["p1"] = _SpmdRunner(_build_phase1(), shared=("xb",), volatile=("xb",))
    if "p2" not in _cache:
        _cache["p2"] = _SpmdRunner(
            _build_phase2(), shared=("mut", "fc1", "fc2"),
            volatile=("xr", "x1r"))

    rn1 = np.ascontiguousarray(rn1_w, np.float32)
    rn2 = np.ascontiguousarray(rn2_w, np.float32)
    tw_cores, wt_cores = _host_prep(V, sigma, M_u, M_phi_plus, M_phi_minus, rn1)
    xb = x.astype(BF16NP)

    in_maps1 = [
        {"xb": xb, "tw": tw_cores[c], "wt": wt_cores[c]}
        for c in range(NCORES)
    ]
    r1 = _cache["p1"]
    sp_cat = r1.run_prepped(r1.prep(in_maps1))[0]
    if "reduce" not in _cache:
        import jax
        import jax.numpy as jnp
        from jax.sharding import NamedSharding, PartitionSpec
        sh = NamedSharding(r1._shardings["xb"].mesh, PartitionSpec())
        _cache["reduce"] = jax.jit(
            lambda spc, xx: xx + spc.reshape(NCORES, B, SL, D)
            .astype(jnp.float32).sum(0),
            out_shardings=sh,
        )
    x1 = np.asarray(_cache["reduce"](sp_cat, np.asarray(x)))

    # phase 2 inputs; rn1 folds into mut's contraction axis, rn2 into fc1's
    mut = np.zeros((KU, DC, P, D), np.float32)
    for t in range(KU):
        for c in range(DC):
            mut[t, c] = (M_u[t] * rn1[None, :])[:, c * P:(c + 1) * P].T
    mut = mut.astype(BF16NP)
    # fc1 layout (JC, DC, P, 2, P): [..., 0, :] = y half column block jc,
    # [..., 1, :] = gate half column block jc (fp8 hi plane only; the y-side
    # hi/lo split on-device compensates the activation quantization)
    f1s = np.ascontiguousarray(fc1_w, np.float32) * rn2[:, None] * MLP_SCALE
    hi8 = f1s.astype(FP8NP)
    fc1p = np.ascontiguousarray(
        np.transpose(hi8.reshape(DC, P, 2, JC, P), (3, 0, 1, 2, 4)))
    f2s = np.ascontiguousarray(fc2_w, np.float32).reshape(JC, P, D) * MLP_SCALE
    fc2 = np.ascontiguousarray(f2s.astype(FP8NP))

    x_rows = x.reshape(B * SL, D)
    x1_rows = x1.reshape(B * SL, D)
    in_maps2 = []
    for c in range(NCORES):
        r0 = c * RPC
        xr = np.zeros((RPC + 2, D), np.float32)
        xr[2:] = x_rows[r0:r0 + RPC]
        if r0 % SL != 0:
            xr[0:2] = x_rows[r0 - 2:r0]
        in_maps2.append({
            "xr": xr.astype(BF16NP),
            "x1r": np.ascontiguousarray(
                x1_rows[r0:r0 + RPC]).astype(np.float16),
            "mut": mut, "fc1": fc1p, "fc2": fc2,
        })
    res2 = _cache["p2"](in_maps2)
    out = np.concatenate(
        [res2[c]["o"] for c in range(NCORES)], axis=0
    ).astype(np.float32).reshape(B, SL, D)
    return out



# revision 39
# speedup vs baseline: 1.2880x; 1.0126x over previous
"""Trainium2 Bass kernel for the STU (spectral transform unit) dense-transformer block.

Algorithm (validated against the jax reference in fp64 numpy):
  The FFT causal conv is rewritten as a block-Toeplitz matmul. For each of the
  K=16 filters and each sign branch (the alternating-sign branch folds into the
  filter taps: T^-[s,s'] = phi[s-s'] * (-1)^(s-s')), the causal conv is
    U_br = T_br @ u,  T_br block-Toeplitz with 16 distinct 128x128 blocks.
  sigma^(1/4) folds into the taps. The (k,i)->d projection contracts U with
  M_phi_{plus,minus}; the KU=3 autoregressive taps are shifted-u projections
  with M_u. MLP is a standard gated MLP.

Sharding (8 cores, no cross-core communication, host-side reduce between two
uniform SPMD programs):
  Phase 1: filter-branch-parallel. Core c computes conv + projection for its 4
           of the 32 (k, sign) branches over the full (B, SL): partial spectral.
  Host:    x1 = x + sum_c partial_c
  Phase 2: row-parallel. Core c owns 512 of the 4096 (b, s) rows: adds the AR
           term and computes the gated MLP + residual for its rows.

Precision: the conv runs in fp8 (output magnitude ~0.05 -> noise negligible);
fc1 and fc2 run as compensated hi+lo fp8 splits (h = yh@wh + DoubleRow-paired
cross terms wl@yh + wh@yl, dropping only the second-order yl@wl product),
which gets fp8 DoubleRow matmul rates at bf16-class accuracy. The AR term
stays bf16: its shifted u^T windows would have odd byte offsets in fp8,
which the Ldweights ISA rejects. Measured end-to-end error: 3.9e-3
scale-relative vs the 2e-2 harness gate. rn1/rn2 rmsnorm weights are folded
into the downstream contraction weights host-side (they commute through the
seq-dim conv / shifts).

Schedule notes (all targets are the InstructionCostModel timeline):
 - DMA is a serial ~360GB/s resource; transfers are emitted in the order
   compute needs them (x row-blocks and tw delta-chunks interleaved, weights
   after first-use rows), which removes the 24us/30us startup stalls the
   v1 kernel had.
 - PSUM->SBUF drain copies alternate across DVE/Act (GPSIMD cannot read
   PSUM); Pool takes the SBUF->SBUF rmsnorm multiplies.
 - Phase-1 software-pipelines the next block's conv between conv(I) and
   proj(I) (depth 2-3 for the short early blocks) so the PE covers the
   psum-drain latency; projection iterates cp-outer so its first matmuls
   depend only on the first conv psum drains.
 - Phase-1 warms the PE p-state with dummy matmuls while the first input
   blocks stream in; phase-2 finalizes fc2 m-outer and d-half-outer so each
   row block's residual add and output DMA overlap later matmuls.
 - Both phases issue dummy Square/Sqrt activations at the head of the
   Activation queue so the act-table loads finish before the first rmsnorm
   needs them.
"""

import numpy as np
import ml_dtypes

import concourse.bacc as bacc
import concourse.tile as tile
from concourse import mybir
from concourse.bass_utils import run_bass_kernel_spmd  # noqa: F401 (debug path)
from concourse.masks import make_identity


class _SpmdRunner:
    """Cached-jit SPMD executor: trace/compile once, then repeat calls only
    pay input upload + execution (mirrors bass2jax.run_bass_via_pjrt).

    ``shared`` names inputs that are identical on every core: they are fed
    replicated (host uploads one copy) instead of 8x-concatenated."""

    def __init__(self, nc, shared=(), volatile=()):
        import jax
        import concourse.mybir as _mb
        from concourse.bass2jax import (
            install_neuronx_cc_hook, _bass_exec_p, partition_id_tensor,
        )
        from jax.experimental.shard_map import shard_map
        from jax.sharding import Mesh, PartitionSpec

        install_neuronx_cc_hook()
        self.nc = nc
        assert nc.dbg_addr is None
        pid_name = (nc.partition_id_tensor.name
                    if nc.partition_id_tensor is not None else None)
        in_names, out_names, out_avals = [], [], []
        for alloc in nc.m.functions[0].allocations:
            if not isinstance(alloc, mybir.MemoryLocationSet):
                continue
            name = alloc.memorylocations[0].name
            if alloc.kind == "ExternalInput":
                if name != pid_name:
                    in_names.append(name)
            elif alloc.kind == "ExternalOutput":
                out_names.append(name)
                out_avals.append(jax.core.ShapedArray(
                    tuple(alloc.tensor_shape), mybir.dt.np(alloc.dtype)))
        self.in_names, self.out_names, self.out_avals = in_names, out_names, out_avals
        self.shared = frozenset(shared)
        self.volatile = frozenset(volatile)
        self._dev_cache = {}
        n_params = len(in_names)
        all_names = tuple(in_names + out_names)
        if pid_name is not None:
            all_names = all_names + (pid_name,)

        def _body(*args):
            args = list(args)
            if pid_name is not None:
                args.append(partition_id_tensor())
            return tuple(_bass_exec_p.bind(
                *args,
                out_avals=tuple(out_avals),
                in_names=all_names,
                out_names=tuple(out_names),
                lowering_input_output_aliases=(),
                sim_require_finite=True,
                sim_require_nnan=True,
                nc=nc,
            ))

        import jax.numpy as jnp
        from jax.sharding import NamedSharding
        devices = jax.devices()[:NCORES]
        mesh = Mesh(np.asarray(devices), ("core",))
        rep = PartitionSpec()
        core = PartitionSpec("core")
        in_specs = tuple(
            rep if nm in self.shared else core for nm in in_names
        ) + (core,) * len(out_names)
        out_specs = (core,) * len(out_names)
        donate = tuple(range(n_params, n_params + len(out_names)))
        self._fn = jax.jit(
            shard_map(_body, mesh=mesh, in_specs=in_specs, out_specs=out_specs,
                      check_rep=False),
            donate_argnums=donate, keep_unused=True,
        )
        self._zeros_fn = jax.jit(
            lambda: tuple(
                jnp.zeros((NCORES * a.shape[0], *a.shape[1:]), a.dtype)
                for a in out_avals
            ),
            out_shardings=tuple(
                NamedSharding(mesh, core) for _ in out_avals
            ),
        )
        self._shardings = {
            nm: NamedSharding(mesh, rep if nm in self.shared else core)
            for nm in in_names
        }

    def prep(self, in_maps):
        import hashlib
        import jax
        ins = []
        for nm in self.in_names:
            if nm in self.shared:
                arr = np.ascontiguousarray(in_maps[0][nm])
            else:
                arr = np.concatenate(
                    [np.asarray(in_maps[c][nm]) for c in range(NCORES)], axis=0)
            if nm in self.volatile:
                ins.append(arr)
                continue
            key = (nm, hashlib.md5(arr.tobytes()).hexdigest())
            dev = self._dev_cache.get(key)
            if dev is None:
                self._dev_cache.clear() if len(self._dev_cache) > 32 else None
                dev = jax.device_put(arr, self._shardings[nm])
                self._dev_cache[key] = dev
            ins.append(dev)
        return ins

    def run_prepped(self, ins):
        return self._fn(*ins, *self._zeros_fn())

    def __call__(self, in_maps):
        out_arrs = self.run_prepped(self.prep(in_maps))
        return [
            {nm: np.asarray(out_arrs[i]).reshape(NCORES, *self.out_avals[i].shape)[c]
             for i, nm in enumerate(self.out_names)}
            for c in range(NCORES)
        ]

BF16NP = ml_dtypes.bfloat16
FP8NP = ml_dtypes.float8_e4m3
TAP_SCALE = 1024.0
UT_SCALE = 32.0      # psum (TAP_SCALE*U) -> fp8 ut tiles scale factor: 32/1024
W_SCALE = 16.0       # projection weights scaled by 16 for fp8 range
SP_SCALE = UT_SCALE * W_SCALE  # spectral psum carries 32*16 = 512x
MLP_SCALE = 16.0     # fc1 hi/lo fp8 weights carry 16x for fp8 range
F32 = mybir.dt.float32
F32R = mybir.dt.float32r
F16 = mybir.dt.float16
BF = mybir.dt.bfloat16
FP8 = mybir.dt.float8e4

B, SL, D, K, KU = 2, 2048, 768, 16, 3
NFFT, EPS, P, H = 4096, 1e-5, 128, 3072
NB = SL // P            # 16 seq blocks
DC = D // P             # 6 d-chunks
NBR = 2 * K             # 32 conv branches
NCORES = 8
BPC = NBR // NCORES     # 4 branches per core
FPC = K // NCORES       # 2 filters per core (parity-fused conv)
NSB = SL // (2 * P)     # 8 superblocks (256 rows) per batch
RPC = (B * SL) // NCORES  # 512 rows per core
MB = RPC // P           # 4 row blocks per core in phase 2
JC = H // P             # 24 hidden chunks
F1 = 512                # free-dim split of D=768 into 512+256
DR = mybir.MatmulPerfMode.DoubleRow

_cache: dict = {}


def _build_phase1():
    """Parity-fused spectral conv: since T^- = D T D (D = alt signs), the
    even/odd half-convs C_e, C_o of each filter determine both sign branches:
      spectral[even s] = C_e Ws + C_o Wd,  spectral[odd s] = C_e Wd + C_o Ws
    with Ws = W+ + W-, Wd = W+ - W-. The conv FLOPs halve (each half-conv
    reads only half the input rows); the +- reconstruction is absorbed into
    the projection weights at no extra cost. Projection psums pack the
    same-parity rows of two consecutive 128-row blocks (conv output columns
    are emitted parity-major so the proj lhsT stays a contiguous 128-slice).
    Each core owns FPC=2 of the 16 filters."""
    nc = bacc.Bacc("TRN2", target_bir_lowering=False, debug=False, num_devices=NCORES)
    xb = nc.dram_tensor("xb", (B, SL, D), BF, kind="ExternalInput").ap()
    # tw[d0, r, ko, chain, f*128+col]: tap block pair (delta=d0-2*ko), chain
    # 0=even-input 1=odd-input, col parity-major within each filter's 128
    tw = nc.dram_tensor("tw", (NB, P, 2, 2, FPC * P), FP8, kind="ExternalInput").ap()
    # wt[f, sd(0=Ws,1=Wd), cp, r, ko, d_out]
    wt = nc.dram_tensor("wt", (FPC, 2, DC // 2, P, 2, D), FP8, kind="ExternalInput").ap()
    sp = nc.dram_tensor("sp", (B, SL, D), F16, kind="ExternalOutput").ap()

    with tile.TileContext(nc) as tc:
        with (
            tc.tile_pool(name="const", bufs=1) as const_pool,
            tc.tile_pool(name="ubuf", bufs=1) as ubuf_pool,
            tc.tile_pool(name="work", bufs=9) as work,
            tc.tile_pool(name="drain", bufs=4) as drain_pool,
            tc.tile_pool(name="spill", bufs=3) as spill_pool,
            tc.tile_pool(name="psum_u", bufs=4, space="PSUM") as psum_u_pool,
            tc.tile_pool(name="psum_sp", bufs=2, space="PSUM") as psum_sp_pool,
        ):
            eps_sb = const_pool.tile([P, 1], F32)
            nc.vector.memset(eps_sb, float(EPS))
            dummy = const_pool.tile([P, 1], F32, name="dummy")
            nc.scalar.activation(
                dummy, eps_sb, mybir.ActivationFunctionType.Square)
            nc.scalar.activation(
                dummy, dummy, mybir.ActivationFunctionType.Sqrt)
            tw_sb = const_pool.tile([P, NB, 2, 2, FPC * P], FP8)
            wt_sb = const_pool.tile([P, FPC, 2, DC // 2, 2, D], FP8)

            # one persistent fp8 u tile per (b, parity, even/odd-block pair):
            # u_t[b][par][jp][:, ko, :] holds rmsnormed rows
            # 256*(2*jp+ko) + 2r + par of batch b
            u_t = [[[ubuf_pool.tile([P, 2, D], FP8, name=f"u{b}_{par}_{jp}")
                     for jp in range(NB // 4)] for par in range(2)]
                   for b in range(B)]

            def jprep(b, par, blk, dve_sq=False, seng=0):
                """strided x row DMA (one parity class) -> rmsnorm -> fp8.
                (rn1_w is folded into the projection weights host-side.)
                seng: engine for the final scale (0=Pool, 1=DVE, 2=Act) --
                the head jpreps fan out so the Pool queue doesn't serialize
                the first conv's inputs."""
                xt = work.tile([P, D], BF, name="xt")
                r0 = 256 * blk + par
                nc.sync.dma_start(xt, xb[b, r0:r0 + 255:2, :])
                sq = work.tile([P, D], F32, name="sq")
                ms = work.tile([P, 1], F32, name="ms")
                if dve_sq:
                    nc.vector.scalar_tensor_tensor(
                        sq, xt, 1.0, xt, mybir.AluOpType.mult,
                        mybir.AluOpType.mult, accum_out=ms,
                    )
                else:
                    nc.scalar.activation(
                        sq, xt, mybir.ActivationFunctionType.Square,
                        accum_out=ms,
                    )
                nc.scalar.activation(
                    ms, ms, mybir.ActivationFunctionType.Sqrt,
                    bias=eps_sb, scale=1.0 / D,
                )
                nc.vector.reciprocal(ms, ms)
                dst = u_t[b][par][blk // 2][:, blk % 2, :]
                if seng == 1:
                    nc.vector.tensor_scalar_mul(dst, xt, ms)
                elif seng == 2:
                    nc.scalar.activation(
                        dst, xt, mybir.ActivationFunctionType.Copy, scale=ms)
                else:
                    nc.gpsimd.tensor_scalar_mul(dst, xt, ms)

            # PE warmup: dummy matmuls on a zero tile ramp the tensor
            # engine p-state while the first input blocks stream in (memset
            # on gpsimd so the PE isn't gated on the busier DVE queue)
            wz = const_pool.tile([P, 2, 2 * P], FP8, name="wz")
            nc.gpsimd.memset(wz, 0.0)
            wps = psum_u_pool.tile([P, 2 * P], F32, name="psu")
            NW = 34
            for i in range(NW):
                nc.tensor.matmul(wps, lhsT=wz[:, :, 0:P], rhs=wz,
                                 start=i == 0, stop=i == NW - 1, perf_mode=DR)

            # prologue: x rows for conv blocks 0,1 stream before the tap
            # blocks (the taps are only needed once the PE issues Ldweights);
            # scale engines fan out so no single queue serializes readiness
            jprep(0, 0, 0, dve_sq=True, seng=1)
            jprep(0, 1, 0, seng=2)
            nc.sync.dma_start(tw_sb[:, 0], tw[0])
            nc.sync.dma_start(tw_sb[:, 1], tw[1])
            jprep(0, 0, 1, seng=0)
            jprep(0, 1, 1, seng=1)
            nc.sync.dma_start(
                wt_sb[:, :, :, 0, :, :],
                wt[:, :, 0].rearrange("f s p k d -> p f s k d"))
            next_blk = [2, 0]

            def conv_block(b, I, ut_sb):
                """both half-conv chains for seq block I into one psum; drain
                into the superblock ut tile (cols parity-major per filter)."""
                npair = I // 4 + 1
                half = I % 2
                for c in range(DC):
                    ps = psum_u_pool.tile([P, 2 * FPC * P], F32, name="psu")
                    for chain in range(2):
                        dst = ps[:, chain * FPC * P:(chain + 1) * FPC * P]
                        if b == 0 and I < 2:
                            # deltas (I, I-2): the ko=1 half is all-zero taps;
                            # a plain (non-DR) matmul on the first pair-half
                            # depends only on x rows 0..255, so the first
                            # convs start ~2us earlier
                            nc.tensor.matmul(
                                dst,
                                lhsT=u_t[b][chain][0][:, 0, c * P:(c + 1) * P],
                                rhs=tw_sb[:, I, 0, chain, :],
                                start=True, stop=True,
                            )
                            continue
                        for Jp in range(npair):
                            nc.tensor.matmul(
                                dst,
                                lhsT=u_t[b][chain][Jp][:, :, c * P:(c + 1) * P],
                                rhs=tw_sb[:, I - 4 * Jp, :, chain, :],
                                start=(Jp == 0),
                                stop=(Jp == npair - 1),
                                perf_mode=DR,
                            )
                    # psum free dim = (chain, f, par, 64); ut free dim per
                    # chunk = (chain*FPC+f, par, 128=(half,64))
                    pv = ps.rearrange("p (s q j) -> p s q j", s=2 * FPC, q=2)
                    dst = ut_sb[:, c, :, :, half * 64:half * 64 + 64]
                    if c % 2 == 0:
                        nc.vector.tensor_scalar_mul(
                            dst, pv, float(UT_SCALE / TAP_SCALE))
                    else:
                        nc.scalar.activation(
                            dst, pv, mybir.ActivationFunctionType.Copy,
                            scale=float(UT_SCALE / TAP_SCALE),
                        )

            def proj_block(b, Ip, par, ut_sb, last=False):
                """one parity's projection for superblock Ip: 128 same-parity
                rows of blocks (2Ip, 2Ip+1); sd picks Ws for the matching
                parity chain, Wd for the crossed one."""
                psp = psum_sp_pool.tile([P, D], F32, name="psp")
                sp_t = spill_pool.tile([P, D], F16, name="spt")
                r0 = 256 * Ip + par
                if last:
                    # tail only: sequential F1/256 chains so the F1 drain +
                    # DMA overlap the 256 matmuls (elsewhere this loses --
                    # the drain read blocks the tile's second chain)
                    halves = [[(0, F1)], [(F1, D)]]
                else:
                    halves = [[(0, F1), (F1, D)]]
                for grp_i, grp in enumerate(halves):
                    pst = psp
                    if last and grp_i == 1:
                        # separate psum tile: no tile-level WAR against the
                        # F1 chain's drain read
                        pst = psum_sp_pool.tile([P, D], F32, name="psp")
                    i_mm = 0
                    n_mm = 2 * FPC * (DC // 2)
                    for cp in range(DC // 2):
                        for st_i in range(2 * FPC):
                            chain, f = divmod(st_i, FPC)
                            sd = par if chain == 0 else 1 - par
                            lh = ut_sb[:, 2 * cp:2 * cp + 2, st_i, par, :]
                            for d0, d1 in grp:
                                nc.tensor.matmul(
                                    pst[:, d0:d1], lhsT=lh,
                                    rhs=wt_sb[:, f, sd, cp, :, d0:d1],
                                    start=i_mm == 0, stop=i_mm == n_mm - 1,
                                    perf_mode=DR,
                                )
                            i_mm += 1
                    for gi, (d0, d1) in enumerate(grp):
                        if (par + gi + grp_i) % 2 == 0:
                            nc.scalar.activation(
                                sp_t[:, d0:d1], pst[:, d0:d1],
                                mybir.ActivationFunctionType.Copy,
                                scale=float(1.0 / SP_SCALE),
                            )
                        else:
                            nc.vector.tensor_scalar_mul(
                                sp_t[:, d0:d1], pst[:, d0:d1],
                                float(1.0 / SP_SCALE))
                        nc.sync.dma_start(
                            sp[b, r0:r0 + 255:2, d0:d1], sp_t[:, d0:d1])

            # software pipeline: emit convs ahead of projections so the PE
            # covers psum-drain latency; a superblock's proj needs both its
            # conv blocks drained
            from collections import deque
            pend = deque()
            # b=0 ascending, b=1 descending: b1's large-I blocks (conv-heavy)
            # land right after b0's (drain-heavy) first half, keeping the
            # drain engines the bottleneck only briefly
            sched = [(0, Ip) for Ip in range(NSB)]
            sched += [(1, Ip) for Ip in reversed(range(NSB))]
            for si, (b, Ip) in enumerate(sched):
                ut_sb = drain_pool.tile([P, DC, 2 * FPC, 2, P], FP8,
                                        name="ut")
                for half in range(2):
                    I = 2 * Ip + half
                    # pace the DMA queue: tw chunk I+2, remaining wt
                    # chunks, upcoming u row blocks
                    if b == 0 and I + 2 < NB:
                        nc.sync.dma_start(tw_sb[:, I + 2], tw[I + 2])
                    if b == 0 and 1 <= I < 3:
                        cp = I
                        nc.sync.dma_start(
                            wt_sb[:, :, :, cp, :, :],
                            wt[:, :, cp].rearrange(
                                "f s p k d -> p f s k d"))
                    need = min(2 * (((I + 4) // 4)) + 1, NB // 2 - 1)
                    while next_blk[b] <= need:
                        jprep(b, 0, next_blk[b])
                        jprep(b, 1, next_blk[b])
                        next_blk[b] += 1
                    if b == 0 and I >= 8:
                        while next_blk[1] <= min(I - 8, NB // 2 - 1):
                            jprep(1, 0, next_blk[1])
                            jprep(1, 1, next_blk[1])
                            next_blk[1] += 1
                    conv_block(b, I, ut_sb)
                pend.append((b, Ip, ut_sb))
                depth = 2 if si < 3 else 1
                while len(pend) > depth:
                    pb, pIp, put = pend.popleft()
                    proj_block(pb, pIp, 0, put)
                    proj_block(pb, pIp, 1, put)
            while pend:
                pb, pIp, put = pend.popleft()
                proj_block(pb, pIp, 0, put)
                proj_block(pb, pIp, 1, put, last=True)
    nc.compile()
    return nc


def _build_phase2():
    """Row-parallel AR + gated MLP. The MLP runs fp8 weights-hi-only with
    activation-side compensation: h = (yh + yl) @ wh (yl = y - fp8(y)), so
    the only dropped term is y @ (w - fp8(w)) -- a fixed small weight
    perturbation. Same for fc2 with g hi/lo. Halves both the matmul count
    (vs the hi/lo cross scheme) and the weight DMA. The AR term stays bf16
    (shifted fp8 u^T windows would need odd byte offsets in Ldweights)."""
    nc = bacc.Bacc("TRN2", target_bir_lowering=False, debug=False, num_devices=NCORES)
    xr = nc.dram_tensor("xr", (RPC + 2, D), BF, kind="ExternalInput").ap()
    x1r = nc.dram_tensor("x1r", (RPC, D), F16, kind="ExternalInput").ap()
    mut = nc.dram_tensor("mut", (KU, DC, P, D), BF, kind="ExternalInput").ap()
    fc1 = nc.dram_tensor("fc1", (JC, DC, P, 2, P), FP8, kind="ExternalInput").ap()
    fc2 = nc.dram_tensor("fc2", (JC, P, D), FP8, kind="ExternalInput").ap()
    o = nc.dram_tensor("o", (RPC, D), F16, kind="ExternalOutput").ap()

    with tile.TileContext(nc) as tc:
        with (
            tc.tile_pool(name="const", bufs=1) as const_pool,
            tc.tile_pool(name="persist", bufs=1) as persist,
            tc.tile_pool(name="work", bufs=6) as work,
            tc.tile_pool(name="wstream", bufs=5) as wstream,
            tc.tile_pool(name="psum", bufs=8, space="PSUM") as psum_pool,
        ):
            ident = const_pool.tile([P, P], F32)
            make_identity(nc, ident)
            identb = const_pool.tile([P, P], BF)
            nc.vector.tensor_copy(identb, ident)
            eps_sb = const_pool.tile([P, 1], F32)
            nc.vector.memset(eps_sb, float(EPS))
            dummy = const_pool.tile([P, 1], F32, name="dummy")
            nc.scalar.activation(
                dummy, eps_sb, mybir.ActivationFunctionType.Square)
            nc.scalar.activation(
                dummy, dummy, mybir.ActivationFunctionType.Sqrt)

            # PE warmup: ramp the p-state while the first x rows stream in
            wz = const_pool.tile([P, 2, 2 * P], FP8, name="wz")
            nc.gpsimd.memset(wz, 0.0)
            wps = psum_pool.tile([P, 2 * P], F32, name="ps")
            for i in range(40):
                nc.tensor.matmul(wps, lhsT=wz[:, :, 0:P], rhs=wz,
                                 start=i == 0, stop=i == 39, perf_mode=DR)

            ut_ext = persist.tile([P, DC, MB, P + 2], BF)
            x1p = persist.tile([P, MB, D], F32)
            xrows = persist.tile([P, MB, D], BF)
            x1rows = persist.tile([P, MB, D], F16)
            yt = persist.tile([P, DC, 2, MB * P], FP8)
            gt = persist.tile([P, JC, 2, MB * P], FP8)
            mut_sb = persist.tile([P, KU, DC, D], BF)
            fc2_sb = persist.tile([P, JC, D], FP8)

            def rmsnorm_to(dst, src, rows, dve_sq=False, seng=0):
                """dst = src / rms(src); the rmsnorm weight is folded into
                the downstream contraction weights host-side."""
                sq = work.tile([P, D], F32, name="sq")
                ms = work.tile([P, 1], F32, name="ms")
                if dve_sq:
                    nc.vector.scalar_tensor_tensor(
                        sq[:rows], src[:rows], 1.0, src[:rows],
                        mybir.AluOpType.mult, mybir.AluOpType.mult,
                        accum_out=ms[:rows],
                    )
                else:
                    nc.scalar.activation(
                        sq[:rows], src[:rows],
                        mybir.ActivationFunctionType.Square,
                        accum_out=ms[:rows],
                    )
                nc.scalar.activation(
                    ms[:rows], ms[:rows], mybir.ActivationFunctionType.Sqrt,
                    bias=eps_sb[:rows], scale=1.0 / D,
                )
                nc.vector.reciprocal(ms[:rows], ms[:rows])
                if seng == 1:
                    nc.vector.tensor_scalar_mul(dst, src[:rows], ms[:rows])
                elif seng == 2:
                    nc.scalar.activation(
                        dst, src[:rows], mybir.ActivationFunctionType.Copy,
                        scale=ms[:rows])
                else:
                    nc.gpsimd.tensor_scalar_mul(dst, src[:rows], ms[:rows])

            # DMA queue front: prefix rows, the 4 x row blocks, mut taps (in
            # per-tap-half chunks so AR starts on the first), then x1 rows;
            # fc1/fc2 stream later in the fws loop
            u_pre = persist.tile([2, D], F32)
            xp = work.tile([P, D], BF, name="xt")[:2]
            nc.sync.dma_start(xp, xr[0:2, :])
            for m in range(MB):
                nc.sync.dma_start(
                    xrows[:, m, :], xr[2 + m * P: 2 + (m + 1) * P, :])
            HC = DC // 2
            for t in range(KU):
                for c in range(DC):
                    nc.sync.dma_start(
                        mut_sb[:, t, c, :],
                        mut[t, c].rearrange("p d -> p d"),
                    )
            for m in range(MB):
                nc.sync.dma_start(x1rows[:, m, :], x1r[m * P:(m + 1) * P, :])

            def psum_copy(dst, src_ps, idx):
                if idx % 2 == 0:
                    nc.vector.tensor_copy(dst, src_ps)
                else:
                    nc.scalar.activation(
                        dst, src_ps, mybir.ActivationFunctionType.Copy
                    )

            # ---- u^T tiles for the AR term (rmsnorm1 + PE transpose);
            # the 2-row prefix runs after the m blocks so it stays off the
            # critical path ----
            for m in range(MB):
                uo = work.tile([P, D], BF, name="uo")
                rmsnorm_to(uo, xrows[:, m, :], P, dve_sq=m % 2 == 0,
                           seng=(1, 2, 0, 0)[m])
                for c in range(DC):
                    pst = psum_pool.tile([P, F1], BF, name="ps")[:, 0:P]
                    nc.tensor.transpose(pst, uo[:, c * P:(c + 1) * P],
                                        identb)
                    psum_copy(ut_ext[:, c, m, 2:P + 2], pst, c + 1)
            rmsnorm_to(u_pre, xp, 2)
            for c in range(DC):
                pst2 = psum_pool.tile([P, F1], F32, name="ps")[:, 0:P]
                nc.tensor.transpose(
                    pst2[:, 0:2], u_pre[:, c * P:(c + 1) * P], ident[0:2, 0:2]
                )
                nc.vector.tensor_copy(ut_ext[:, c, 0, 0:2], pst2[:, 0:2])
            for m in range(1, MB):
                for c in range(DC):
                    nc.gpsimd.tensor_copy(
                        ut_ext[:, c, m, 0:2], ut_ext[:, c, m - 1, P:P + 2]
                    )

            # ---- AR term: all 4 row-blocks accumulate per-(tap, d-half) in
            # mut arrival order so the psum groups start on the first chunk ----
            psa = [(psum_pool.tile([P, F1], F32, name="ps"),
                    psum_pool.tile([P, F1], F32, name="ps"))
                   for _ in range(MB)]
            # last tap m-outer: psa[m] stops stagger by ~2us so each row
            # block's x1 add + rmsnorm2 chain overlaps the remaining taps
            groups = [(t, h, m) for t in range(KU - 1) for h in range(2)
                      for m in range(MB)]
            groups += [(KU - 1, h, m) for m in range(MB) for h in range(2)]
            for t, h, m in groups:
                for c in range(h * HC, (h + 1) * HC):
                    st = t == 0 and c == 0
                    fin = t == KU - 1 and c == DC - 1
                    lh = ut_ext[:, c, m, 2 - t:P + 2 - t]
                    nc.tensor.matmul(
                        psa[m][0], lhsT=lh,
                        rhs=mut_sb[:, t, c, 0:F1], start=st, stop=fin,
                    )
                    nc.tensor.matmul(
                        psa[m][1][:, 0:D - F1], lhsT=lh,
                        rhs=mut_sb[:, t, c, F1:D], start=st, stop=fin,
                    )
            for m in range(MB):
                nc.vector.tensor_tensor(
                    x1p[:, m, 0:F1], x1rows[:, m, 0:F1], psa[m][0],
                    mybir.AluOpType.add)
                nc.vector.tensor_tensor(
                    x1p[:, m, F1:D], x1rows[:, m, F1:D],
                    psa[m][1][:, 0:D - F1], mybir.AluOpType.add)

            # fc1 weight chunks + fc2 resident weights, in first-use order
            fws = []
            for jc in range(JC):
                fw = wstream.tile([P, DC, 2, P], FP8, name="fw")
                nc.sync.dma_start(fw, fc1[jc].rearrange("c p k f -> p c k f"))
                fws.append(fw)
                if jc in (10, 14, 18, 22):
                    q4 = JC // 4
                    qi = (jc - 10) // 4
                    nc.sync.dma_start(
                        fc2_sb[:, qi * q4:(qi + 1) * q4, :],
                        fc2[qi * q4:(qi + 1) * q4].rearrange("c p d -> p c d"))

            # ---- y = rmsnorm2(x1) transposed, hi + compensation lo ----
            for m in range(MB):
                yf = work.tile([P, D], BF, name="uo")
                rmsnorm_to(yf, x1p[:, m, :], P, dve_sq=m % 2 == 0,
                           seng=(1, 2, 0, 1)[m])
                for c in range(DC):
                    pst = psum_pool.tile([P, F1], BF, name="ps")[:, 0:P]
                    nc.tensor.transpose(pst, yf[:, c * P:(c + 1) * P],
                                        identb)
                    sl = slice(m * P, (m + 1) * P)
                    psum_copy(yt[:, c, 0, sl], pst, 1)
                    nc.vector.scalar_tensor_tensor(
                        yt[:, c, 1, sl], pst, 1.0, yt[:, c, 0, sl],
                        mybir.AluOpType.mult, mybir.AluOpType.subtract,
                    )

            # ---- fc1 + silu gate: (yh + yl) @ wh, 6 DR matmuls per half ----
            for jc in range(JC):
                ph1 = psum_pool.tile([P, F1], F32, name="ps")
                ph2 = psum_pool.tile([P, F1], F32, name="ps")
                for hh, ph in ((0, ph1), (1, ph2)):
                    i_mm = 0
                    for q in range(2):
                        for cp in range(DC // 2):
                            nc.tensor.matmul(
                                ph,
                                lhsT=fws[jc][:, 2 * cp:2 * cp + 2, hh, :],
                                rhs=yt[:, 2 * cp:2 * cp + 2, q, :],
                                start=i_mm == 0, stop=i_mm == DC - 1,
                                perf_mode=DR,
                            )
                            i_mm += 1
                sact = work.tile([P, F1], F32, name="sact")
                nc.scalar.activation(sact, ph2,
                                     mybir.ActivationFunctionType.Silu,
                                     scale=float(1.0 / MLP_SCALE))
                g32 = work.tile([P, F1], F32, name="g32")
                nc.vector.scalar_tensor_tensor(
                    g32, ph1, float(1.0 / MLP_SCALE), sact,
                    mybir.AluOpType.mult, mybir.AluOpType.mult,
                )
                nc.scalar.activation(
                    gt[:, jc, 0, :], g32, mybir.ActivationFunctionType.Copy
                )
                nc.vector.scalar_tensor_tensor(
                    gt[:, jc, 1, :], g32, 1.0, gt[:, jc, 0, :],
                    mybir.AluOpType.mult, mybir.AluOpType.subtract,
                )

            # ---- fc2 + residual: (gh + gl) @ f2h, m-outer so each row
            # block's residual add and output DMA overlap later matmuls ----
            for m in range(MB):
                msl = slice(m * P, (m + 1) * P)
                ot = work.tile([P, D], F16, name="ot")
                for hi, (d0, d1) in enumerate(((0, F1), (F1, D))):
                    po = psum_pool.tile([P, F1], F32, name="ps")[:, 0:d1 - d0]
                    i_mm = 0
                    for q in range(2):
                        for jp in range(JC // 2):
                            nc.tensor.matmul(
                                po,
                                lhsT=gt[:, 2 * jp:2 * jp + 2, q, msl],
                                rhs=fc2_sb[:, 2 * jp:2 * jp + 2, d0:d1],
                                start=i_mm == 0, stop=i_mm == JC - 1,
                                perf_mode=DR,
                            )
                            i_mm += 1
                    nc.vector.scalar_tensor_tensor(
                        ot[:, d0:d1], po, float(1.0 / MLP_SCALE),
                        x1p[:, m, d0:d1],
                        mybir.AluOpType.mult, mybir.AluOpType.add)
                    nc.sync.dma_start(o[m * P:(m + 1) * P, d0:d1],
                                      ot[:, d0:d1])
    nc.compile()
    return nc


def _host_prep(V, sigma, M_u, M_phi_plus, M_phi_minus, rn1):
    """Per-core tap blocks + fused projection matrices for the parity-split
    conv. Core c owns filters (2c, 2c+1). rn1_w folds into the projection's
    contraction axis; sigma^0.25 folds into the taps.

    tw[d0, r, ko, chain, f*128 + po*64 + j] =
        taps_f[128*(d0 - 2*ko) + (2j + po) - 2r - chain]   (0 if idx < 0)
    (chain 0 reads even input rows, 1 odd; output cols parity-major po)
    wt[f, sd, cp, i, ko, d] = {Ws,Wd}[f][d, (2cp+ko)*128 + i] * W_SCALE
    """
    phi = np.fft.irfft(V.astype(np.complex128), n=NFFT, axis=0)[:SL]
    s4 = sigma.astype(np.float64) ** 0.25
    taps = (s4[None, :] * phi)                       # (SL, K)
    Ws = (M_phi_plus + M_phi_minus) * rn1[None, None, :]
    Wd = (M_phi_plus - M_phi_minus) * rn1[None, None, :]

    # col -> s_out map (parity-major)
    s_out = np.empty(P, np.int64)
    s_out[:64] = 2 * np.arange(64)
    s_out[64:] = 2 * np.arange(64) + 1

    d0v = np.arange(NB)[:, None, None, None, None]
    rv = np.arange(P)[None, :, None, None, None]
    kov = np.arange(2)[None, None, :, None, None]
    chv = np.arange(2)[None, None, None, :, None]
    colv = s_out[None, None, None, None, :]
    idx = 128 * (d0v - 2 * kov) + colv - 2 * rv - chv   # (NB,P,2,2,P)

    tw_cores = []
    wt_cores = []
    for core in range(NCORES):
        tw = np.zeros((NB, P, 2, 2, FPC * P), np.float32)
        wt = np.zeros((FPC, 2, DC // 2, P, 2, D), np.float32)
        for f in range(FPC):
            k = core * FPC + f
            tsc = (taps[:, k] * TAP_SCALE).astype(np.float64)
            blk = np.where(idx >= 0, tsc[np.clip(idx, 0, SL - 1)], 0.0)
            tw[:, :, :, :, f * P:(f + 1) * P] = blk
            for sd, W in ((0, Ws[k]), (1, Wd[k])):
                for cp in range(DC // 2):
                    for ko in range(2):
                        c = 2 * cp + ko
                        wt[f, sd, cp, :, ko, :] = (
                            W[:, c * P:(c + 1) * P].T * W_SCALE)
        tw_cores.append(tw.astype(FP8NP))
        wt_cores.append(wt.astype(FP8NP))
    return tw_cores, wt_cores


def kernel(x, V, sigma, M_u, M_phi_plus, M_phi_minus, rn1_w, rn2_w, fc1_w, fc2_w):
    x = np.ascontiguousarray(x, np.float32)
    if "p1" not in _cache:
        _cache["p1"] = _SpmdRunner(_build_phase1(), shared=("xb",), volatile=("xb",))
    if "p2" not in _cache:
        _cache["p2"] = _SpmdRunner(
            _build_phase2(), shared=("mut", "fc1", "fc2"),
            volatile=("xr", "x1r"))

    rn1 = np.ascontiguousarray(rn1_w, np.float32)
    rn2 = np.ascontiguousarray(rn2_w, np.float32)
    tw_cores, wt_cores = _host_prep(V, sigma, M_u, M_phi_plus, M_phi_minus, rn1)
    xb = x.astype(BF16NP)

    in_maps1 = [
        {"xb": xb, "tw": tw_cores[c], "wt": wt_cores[c]}
        for c in range(NCORES)
    ]
    r1 = _cache["p1"]
    sp_cat = r1.run_prepped(r1.prep(in_maps1))[0]
    if "reduce" not in _cache:
        import jax
        import jax.numpy as jnp
        from jax.sharding import NamedSharding, PartitionSpec
        sh = NamedSharding(r1._shardings["xb"].mesh, PartitionSpec())
        _cache["reduce"] = jax.jit(
            lambda spc, xx: xx + spc.reshape(NCORES, B, SL, D)
            .astype(jnp.float32).sum(0),
            out_shardings=sh,
        )
    x1 = np.asarray(_cache["reduce"](sp_cat, np.asarray(x)))

    # phase 2 inputs; rn1 folds into mut's contraction axis, rn2 into fc1's
    mut = np.zeros((KU, DC, P, D), np.float32)
    for t in range(KU):
        for c in range(DC):
            mut[t, c] = (M_u[t] * rn1[None, :])[:, c * P:(c + 1) * P].T
    mut = mut.astype(BF16NP)
    # fc1 layout (JC, DC, P, 2, P): [..., 0, :] = y half column block jc,
    # [..., 1, :] = gate half column block jc (fp8 hi plane only; the y-side
    # hi/lo split on-device compensates the activation quantization)
    f1s = np.ascontiguousarray(fc1_w, np.float32) * rn2[:, None] * MLP_SCALE
    hi8 = f1s.astype(FP8NP)
    fc1p = np.ascontiguousarray(
        np.transpose(hi8.reshape(DC, P, 2, JC, P), (3, 0, 1, 2, 4)))
    f2s = np.ascontiguousarray(fc2_w, np.float32).reshape(JC, P, D) * MLP_SCALE
    fc2 = np.ascontiguousarray(f2s.astype(FP8NP))

    x_rows = x.reshape(B * SL, D)
    x1_rows = x1.reshape(B * SL, D)
    in_maps2 = []
    for c in range(NCORES):
        r0 = c * RPC
        xr = np.zeros((RPC + 2, D), np.float32)
        xr[2:] = x_rows[r0:r0 + RPC]
        if r0 % SL != 0:
            xr[0:2] = x_rows[r0 - 2:r0]
        in_maps2.append({
            "xr": xr.astype(BF16NP),
            "x1r": np.ascontiguousarray(
                x1_rows[r0:r0 + RPC]).astype(np.float16),
            "mut": mut, "fc1": fc1p, "fc2": fc2,
        })
    res2 = _cache["p2"](in_maps2)
    out = np.concatenate(
        [res2[c]["o"] for c in range(NCORES)], axis=0
    ).astype(np.float32).reshape(B, SL, D)
    return out



# revision 44
# speedup vs baseline: 1.3060x; 1.0140x over previous
"""Trainium2 Bass kernel for the STU (spectral transform unit) dense-transformer block.

Algorithm (validated against the jax reference in fp64 numpy):
  The FFT causal conv is rewritten as a block-Toeplitz matmul. For each of the
  K=16 filters and each sign branch (the alternating-sign branch folds into the
  filter taps: T^-[s,s'] = phi[s-s'] * (-1)^(s-s')), the causal conv is
    U_br = T_br @ u,  T_br block-Toeplitz with 16 distinct 128x128 blocks.
  sigma^(1/4) folds into the taps. The (k,i)->d projection contracts U with
  M_phi_{plus,minus}; the KU=3 autoregressive taps are shifted-u projections
  with M_u. MLP is a standard gated MLP.

Sharding (8 cores, no cross-core communication, host-side reduce between two
uniform SPMD programs):
  Phase 1: filter-branch-parallel. Core c computes conv + projection for its 4
           of the 32 (k, sign) branches over the full (B, SL): partial spectral.
  Host:    x1 = x + sum_c partial_c
  Phase 2: row-parallel. Core c owns 512 of the 4096 (b, s) rows: adds the AR
           term and computes the gated MLP + residual for its rows.

Precision: the conv runs in fp8 (output magnitude ~0.05 -> noise negligible);
fc1 and fc2 run as compensated hi+lo fp8 splits (h = yh@wh + DoubleRow-paired
cross terms wl@yh + wh@yl, dropping only the second-order yl@wl product),
which gets fp8 DoubleRow matmul rates at bf16-class accuracy. The AR term
stays bf16: its shifted u^T windows would have odd byte offsets in fp8,
which the Ldweights ISA rejects. Measured end-to-end error: 3.9e-3
scale-relative vs the 2e-2 harness gate. rn1/rn2 rmsnorm weights are folded
into the downstream contraction weights host-side (they commute through the
seq-dim conv / shifts).

Schedule notes (all targets are the InstructionCostModel timeline):
 - DMA is a serial ~360GB/s resource; transfers are emitted in the order
   compute needs them (x row-blocks and tw delta-chunks interleaved, weights
   after first-use rows), which removes the 24us/30us startup stalls the
   v1 kernel had.
 - PSUM->SBUF drain copies alternate across DVE/Act (GPSIMD cannot read
   PSUM); Pool takes the SBUF->SBUF rmsnorm multiplies.
 - Phase-1 software-pipelines the next block's conv between conv(I) and
   proj(I) (depth 2-3 for the short early blocks) so the PE covers the
   psum-drain latency; projection iterates cp-outer so its first matmuls
   depend only on the first conv psum drains.
 - Phase-1 warms the PE p-state with dummy matmuls while the first input
   blocks stream in; phase-2 finalizes fc2 m-outer and d-half-outer so each
   row block's residual add and output DMA overlap later matmuls.
 - Both phases issue dummy Square/Sqrt activations at the head of the
   Activation queue so the act-table loads finish before the first rmsnorm
   needs them.
"""

import numpy as np
import ml_dtypes

import concourse.bacc as bacc
import concourse.tile as tile
from concourse import mybir
from concourse.bass_utils import run_bass_kernel_spmd  # noqa: F401 (debug path)
from concourse.masks import make_identity


class _SpmdRunner:
    """Cached-jit SPMD executor: trace/compile once, then repeat calls only
    pay input upload + execution (mirrors bass2jax.run_bass_via_pjrt).

    ``shared`` names inputs that are identical on every core: they are fed
    replicated (host uploads one copy) instead of 8x-concatenated."""

    def __init__(self, nc, shared=(), volatile=()):
        import jax
        import concourse.mybir as _mb
        from concourse.bass2jax import (
            install_neuronx_cc_hook, _bass_exec_p, partition_id_tensor,
        )
        from jax.experimental.shard_map import shard_map
        from jax.sharding import Mesh, PartitionSpec

        install_neuronx_cc_hook()
        self.nc = nc
        assert nc.dbg_addr is None
        pid_name = (nc.partition_id_tensor.name
                    if nc.partition_id_tensor is not None else None)
        in_names, out_names, out_avals = [], [], []
        for alloc in nc.m.functions[0].allocations:
            if not isinstance(alloc, mybir.MemoryLocationSet):
                continue
            name = alloc.memorylocations[0].name
            if alloc.kind == "ExternalInput":
                if name != pid_name:
                    in_names.append(name)
            elif alloc.kind == "ExternalOutput":
                out_names.append(name)
                out_avals.append(jax.core.ShapedArray(
                    tuple(alloc.tensor_shape), mybir.dt.np(alloc.dtype)))
        self.in_names, self.out_names, self.out_avals = in_names, out_names, out_avals
        self.shared = frozenset(shared)
        self.volatile = frozenset(volatile)
        self._dev_cache = {}
        n_params = len(in_names)
        all_names = tuple(in_names + out_names)
        if pid_name is not None:
            all_names = all_names + (pid_name,)

        def _body(*args):
            args = list(args)
            if pid_name is not None:
                args.append(partition_id_tensor())
            return tuple(_bass_exec_p.bind(
                *args,
                out_avals=tuple(out_avals),
                in_names=all_names,
                out_names=tuple(out_names),
                lowering_input_output_aliases=(),
                sim_require_finite=True,
                sim_require_nnan=True,
                nc=nc,
            ))

        import jax.numpy as jnp
        from jax.sharding import NamedSharding
        devices = jax.devices()[:NCORES]
        mesh = Mesh(np.asarray(devices), ("core",))
        rep = PartitionSpec()
        core = PartitionSpec("core")
        in_specs = tuple(
            rep if nm in self.shared else core for nm in in_names
        ) + (core,) * len(out_names)
        out_specs = (core,) * len(out_names)
        donate = tuple(range(n_params, n_params + len(out_names)))
        self._fn = jax.jit(
            shard_map(_body, mesh=mesh, in_specs=in_specs, out_specs=out_specs,
                      check_rep=False),
            donate_argnums=donate, keep_unused=True,
        )
        self._zeros_fn = jax.jit(
            lambda: tuple(
                jnp.zeros((NCORES * a.shape[0], *a.shape[1:]), a.dtype)
                for a in out_avals
            ),
            out_shardings=tuple(
                NamedSharding(mesh, core) for _ in out_avals
            ),
        )
        self._shardings = {
            nm: NamedSharding(mesh, rep if nm in self.shared else core)
            for nm in in_names
        }

    def prep(self, in_maps):
        import hashlib
        import jax
        ins = []
        for nm in self.in_names:
            if nm in self.shared:
                arr = np.ascontiguousarray(in_maps[0][nm])
            else:
                arr = np.concatenate(
                    [np.asarray(in_maps[c][nm]) for c in range(NCORES)], axis=0)
            if nm in self.volatile:
                ins.append(arr)
                continue
            key = (nm, hashlib.md5(arr.tobytes()).hexdigest())
            dev = self._dev_cache.get(key)
            if dev is None:
                self._dev_cache.clear() if len(self._dev_cache) > 32 else None
                dev = jax.device_put(arr, self._shardings[nm])
                self._dev_cache[key] = dev
            ins.append(dev)
        return ins

    def run_prepped(self, ins):
        return self._fn(*ins, *self._zeros_fn())

    def __call__(self, in_maps):
        out_arrs = self.run_prepped(self.prep(in_maps))
        return [
            {nm: np.asarray(out_arrs[i]).reshape(NCORES, *self.out_avals[i].shape)[c]
             for i, nm in enumerate(self.out_names)}
            for c in range(NCORES)
        ]

BF16NP = ml_dtypes.bfloat16
FP8NP = ml_dtypes.float8_e4m3
TAP_SCALE = 1024.0
UT_SCALE = 32.0      # psum (TAP_SCALE*U) -> fp8 ut tiles scale factor: 32/1024
W_SCALE = 16.0       # projection weights scaled by 16 for fp8 range
SP_SCALE = UT_SCALE * W_SCALE  # spectral psum carries 32*16 = 512x
MLP_SCALE = 16.0     # fc1 hi/lo fp8 weights carry 16x for fp8 range
F32 = mybir.dt.float32
F32R = mybir.dt.float32r
F16 = mybir.dt.float16
BF = mybir.dt.bfloat16
FP8 = mybir.dt.float8e4

B, SL, D, K, KU = 2, 2048, 768, 16, 3
NFFT, EPS, P, H = 4096, 1e-5, 128, 3072
NB = SL // P            # 16 seq blocks
DC = D // P             # 6 d-chunks
NBR = 2 * K             # 32 conv branches
NCORES = 8
BPC = NBR // NCORES     # 4 branches per core
FPC = K // NCORES       # 2 filters per core (parity-fused conv)
NSB = SL // (2 * P)     # 8 superblocks (256 rows) per batch
RPC = (B * SL) // NCORES  # 512 rows per core
MB = RPC // P           # 4 row blocks per core in phase 2
JC = H // P             # 24 hidden chunks
F1 = 512                # free-dim split of D=768 into 512+256
DR = mybir.MatmulPerfMode.DoubleRow

_cache: dict = {}


def _build_phase1():
    """Parity-fused spectral conv: since T^- = D T D (D = alt signs), the
    even/odd half-convs C_e, C_o of each filter determine both sign branches:
      spectral[even s] = C_e Ws + C_o Wd,  spectral[odd s] = C_e Wd + C_o Ws
    with Ws = W+ + W-, Wd = W+ - W-. The conv FLOPs halve (each half-conv
    reads only half the input rows); the +- reconstruction is absorbed into
    the projection weights at no extra cost. Projection psums pack the
    same-parity rows of two consecutive 128-row blocks (conv output columns
    are emitted parity-major so the proj lhsT stays a contiguous 128-slice).
    Each core owns FPC=2 of the 16 filters."""
    nc = bacc.Bacc("TRN2", target_bir_lowering=False, debug=False, num_devices=NCORES)
    xb = nc.dram_tensor("xb", (B, SL, D), BF, kind="ExternalInput").ap()
    # tw[d0, r, ko, chain, f*128+col]: tap block pair (delta=d0-2*ko), chain
    # 0=even-input 1=odd-input, col parity-major within each filter's 128
    tw = nc.dram_tensor("tw", (NB, P, 2, 2, FPC * P), FP8, kind="ExternalInput").ap()
    # wt[f, sd(0=Ws,1=Wd), cp, r, ko, d_out]
    wt = nc.dram_tensor("wt", (FPC, 2, DC // 2, P, 2, D), FP8, kind="ExternalInput").ap()
    sp = nc.dram_tensor("sp", (B, SL, D), F16, kind="ExternalOutput").ap()

    with tile.TileContext(nc) as tc:
        with (
            tc.tile_pool(name="const", bufs=1) as const_pool,
            tc.tile_pool(name="ubuf", bufs=1) as ubuf_pool,
            tc.tile_pool(name="work", bufs=9) as work,
            tc.tile_pool(name="drain", bufs=4) as drain_pool,
            tc.tile_pool(name="spill", bufs=3) as spill_pool,
            tc.tile_pool(name="psum_u", bufs=4, space="PSUM") as psum_u_pool,
            tc.tile_pool(name="psum_sp", bufs=2, space="PSUM") as psum_sp_pool,
        ):
            eps_sb = const_pool.tile([P, 1], F32)
            nc.vector.memset(eps_sb, float(EPS))
            dummy = const_pool.tile([P, 1], F32, name="dummy")
            nc.scalar.activation(
                dummy, eps_sb, mybir.ActivationFunctionType.Square)
            nc.scalar.activation(
                dummy, dummy, mybir.ActivationFunctionType.Sqrt)
            tw_sb = const_pool.tile([P, NB, 2, 2, FPC * P], FP8)
            wt_sb = const_pool.tile([P, FPC, 2, DC // 2, 2, D], FP8)

            # one persistent fp8 u tile per (b, parity, even/odd-block pair):
            # u_t[b][par][jp][:, ko, :] holds rmsnormed rows
            # 256*(2*jp+ko) + 2r + par of batch b
            u_t = [[[ubuf_pool.tile([P, 2, D], FP8, name=f"u{b}_{par}_{jp}")
                     for jp in range(NB // 4)] for par in range(2)]
                   for b in range(B)]

            def jprep(b, par, blk, dve_sq=False, seng=0):
                """strided x row DMA (one parity class) -> rmsnorm -> fp8.
                (rn1_w is folded into the projection weights host-side.)
                seng: engine for the final scale (0=Pool, 1=DVE, 2=Act) --
                the head jpreps fan out so the Pool queue doesn't serialize
                the first conv's inputs."""
                xt = work.tile([P, D], BF, name="xt")
                r0 = 256 * blk + par
                nc.sync.dma_start(xt, xb[b, r0:r0 + 255:2, :])
                sq = work.tile([P, D], F32, name="sq")
                ms = work.tile([P, 1], F32, name="ms")
                if dve_sq:
                    nc.vector.scalar_tensor_tensor(
                        sq, xt, 1.0, xt, mybir.AluOpType.mult,
                        mybir.AluOpType.mult, accum_out=ms,
                    )
                else:
                    nc.scalar.activation(
                        sq, xt, mybir.ActivationFunctionType.Square,
                        accum_out=ms,
                    )
                nc.scalar.activation(
                    ms, ms, mybir.ActivationFunctionType.Sqrt,
                    bias=eps_sb, scale=1.0 / D,
                )
                nc.vector.reciprocal(ms, ms)
                dst = u_t[b][par][blk // 2][:, blk % 2, :]
                if seng == 1:
                    nc.vector.tensor_scalar_mul(dst, xt, ms)
                elif seng == 2:
                    nc.scalar.activation(
                        dst, xt, mybir.ActivationFunctionType.Copy, scale=ms)
                else:
                    nc.gpsimd.tensor_scalar_mul(dst, xt, ms)

            # PE warmup: dummy matmuls on a zero tile ramp the tensor
            # engine p-state while the first input blocks stream in (memset
            # on gpsimd so the PE isn't gated on the busier DVE queue)
            wz = const_pool.tile([P, 2, 2 * P], FP8, name="wz")
            nc.gpsimd.memset(wz, 0.0)
            wps = psum_u_pool.tile([P, 2 * P], F32, name="psu")
            NW = 34
            for i in range(NW):
                nc.tensor.matmul(wps, lhsT=wz[:, :, 0:P], rhs=wz,
                                 start=i == 0, stop=i == NW - 1, perf_mode=DR)

            # prologue: x rows for conv blocks 0,1 stream before the tap
            # blocks (the taps are only needed once the PE issues Ldweights);
            # scale engines fan out so no single queue serializes readiness
            jprep(0, 0, 0, dve_sq=True, seng=1)
            jprep(0, 1, 0, seng=2)
            nc.sync.dma_start(tw_sb[:, 0], tw[0])
            nc.sync.dma_start(tw_sb[:, 1], tw[1])
            jprep(0, 0, 1, seng=0)
            jprep(0, 1, 1, seng=1)
            nc.sync.dma_start(
                wt_sb[:, :, :, 0, :, :],
                wt[:, :, 0].rearrange("f s p k d -> p f s k d"))
            next_blk = [2, 0]
            jseng = [0]

            def conv_block(b, I, ut_sb):
                """both half-conv chains for seq block I into one psum; drain
                into the superblock ut tile (cols parity-major per filter)."""
                npair = I // 4 + 1
                half = I % 2
                for c in range(DC):
                    ps = psum_u_pool.tile([P, 2 * FPC * P], F32, name="psu")
                    for chain in range(2):
                        dst = ps[:, chain * FPC * P:(chain + 1) * FPC * P]
                        if b == 0 and I < 2:
                            # deltas (I, I-2): the ko=1 half is all-zero taps;
                            # a plain (non-DR) matmul on the first pair-half
                            # depends only on x rows 0..255, so the first
                            # convs start ~2us earlier
                            nc.tensor.matmul(
                                dst,
                                lhsT=u_t[b][chain][0][:, 0, c * P:(c + 1) * P],
                                rhs=tw_sb[:, I, 0, chain, :],
                                start=True, stop=True,
                            )
                            continue
                        for Jp in range(npair):
                            nc.tensor.matmul(
                                dst,
                                lhsT=u_t[b][chain][Jp][:, :, c * P:(c + 1) * P],
                                rhs=tw_sb[:, I - 4 * Jp, :, chain, :],
                                start=(Jp == 0),
                                stop=(Jp == npair - 1),
                                perf_mode=DR,
                            )
                    # psum free dim = (chain, f, par, 64); ut free dim per
                    # chunk = (chain*FPC+f, par, 128=(half,64))
                    pv = ps.rearrange("p (s q j) -> p s q j", s=2 * FPC, q=2)
                    dst = ut_sb[:, c, :, :, half * 64:half * 64 + 64]
                    if c % 2 == 0:
                        nc.vector.tensor_scalar_mul(
                            dst, pv, float(UT_SCALE / TAP_SCALE))
                    else:
                        nc.scalar.activation(
                            dst, pv, mybir.ActivationFunctionType.Copy,
                            scale=float(UT_SCALE / TAP_SCALE),
                        )

            def proj_block(b, Ip, par, ut_sb, last=False):
                """one parity's projection for superblock Ip: 128 same-parity
                rows of blocks (2Ip, 2Ip+1); sd picks Ws for the matching
                parity chain, Wd for the crossed one."""
                psp = psum_sp_pool.tile([P, D], F32, name="psp")
                sp_t = spill_pool.tile([P, D], F16, name="spt")
                r0 = 256 * Ip + par
                if last:
                    # tail only: sequential F1/256 chains so the F1 drain +
                    # DMA overlap the 256 matmuls (elsewhere this loses --
                    # the drain read blocks the tile's second chain)
                    halves = [[(0, F1)], [(F1, D)]]
                else:
                    halves = [[(0, F1), (F1, D)]]
                for grp_i, grp in enumerate(halves):
                    pst = psp
                    if last and grp_i == 1:
                        # separate psum tile: no tile-level WAR against the
                        # F1 chain's drain read
                        pst = psum_sp_pool.tile([P, D], F32, name="psp")
                    i_mm = 0
                    n_mm = 2 * FPC * (DC // 2)
                    for cp in range(DC // 2):
                        for st_i in range(2 * FPC):
                            chain, f = divmod(st_i, FPC)
                            sd = par if chain == 0 else 1 - par
                            lh = ut_sb[:, 2 * cp:2 * cp + 2, st_i, par, :]
                            for d0, d1 in grp:
                                nc.tensor.matmul(
                                    pst[:, d0:d1], lhsT=lh,
                                    rhs=wt_sb[:, f, sd, cp, :, d0:d1],
                                    start=i_mm == 0, stop=i_mm == n_mm - 1,
                                    perf_mode=DR,
                                )
                            i_mm += 1
                    for gi, (d0, d1) in enumerate(grp):
                        if (par + gi + grp_i) % 2 == 0:
                            nc.scalar.activation(
                                sp_t[:, d0:d1], pst[:, d0:d1],
                                mybir.ActivationFunctionType.Copy,
                                scale=float(1.0 / SP_SCALE),
                            )
                        else:
                            nc.vector.tensor_scalar_mul(
                                sp_t[:, d0:d1], pst[:, d0:d1],
                                float(1.0 / SP_SCALE))
                        nc.sync.dma_start(
                            sp[b, r0:r0 + 255:2, d0:d1], sp_t[:, d0:d1])

            # software pipeline: emit convs ahead of projections so the PE
            # covers psum-drain latency; a superblock's proj needs both its
            # conv blocks drained
            from collections import deque
            pend = deque()
            # b=0 ascending, b=1 descending: b1's large-I blocks (conv-heavy)
            # land right after b0's (drain-heavy) first half, keeping the
            # drain engines the bottleneck only briefly
            sched = [(0, Ip) for Ip in range(NSB)]
            sched += [(1, Ip) for Ip in reversed(range(NSB))]
            for si, (b, Ip) in enumerate(sched):
                ut_sb = drain_pool.tile([P, DC, 2 * FPC, 2, P], FP8,
                                        name="ut")
                for half in range(2):
                    I = 2 * Ip + half
                    # pace the DMA queue: tw chunk I+2, remaining wt
                    # chunks, upcoming u row blocks
                    if b == 0 and I + 2 < NB:
                        nc.sync.dma_start(tw_sb[:, I + 2], tw[I + 2])
                    if b == 0 and 1 <= I < 3:
                        cp = I
                        nc.sync.dma_start(
                            wt_sb[:, :, :, cp, :, :],
                            wt[:, :, cp].rearrange(
                                "f s p k d -> p f s k d"))
                    need = min(2 * (((I + 4) // 4)) + 1, NB // 2 - 1)
                    while next_blk[b] <= need:
                        jprep(b, 0, next_blk[b], seng=jseng[0] % 3)
                        jprep(b, 1, next_blk[b], dve_sq=True,
                              seng=(jseng[0] + 1) % 3)
                        jseng[0] += 2
                        next_blk[b] += 1
                    if b == 0 and I >= 8:
                        while next_blk[1] <= min(I - 8, NB // 2 - 1):
                            jprep(1, 0, next_blk[1], seng=jseng[0] % 3)
                            jprep(1, 1, next_blk[1], dve_sq=True,
                                  seng=(jseng[0] + 1) % 3)
                            jseng[0] += 2
                            next_blk[1] += 1
                    conv_block(b, I, ut_sb)
                pend.append((b, Ip, ut_sb))
                depth = 2 if si < 3 else 1
                while len(pend) > depth:
                    pb, pIp, put = pend.popleft()
                    proj_block(pb, pIp, 0, put)
                    proj_block(pb, pIp, 1, put)
            while pend:
                pb, pIp, put = pend.popleft()
                proj_block(pb, pIp, 0, put)
                proj_block(pb, pIp, 1, put, last=True)
    nc.compile()
    return nc


def _build_phase2():
    """Row-parallel AR + gated MLP. The MLP runs fp8 weights-hi-only with
    activation-side compensation: h = (yh + yl) @ wh (yl = y - fp8(y)), so
    the only dropped term is y @ (w - fp8(w)) -- a fixed small weight
    perturbation. Same for fc2 with g hi/lo. Halves both the matmul count
    (vs the hi/lo cross scheme) and the weight DMA. The AR term stays bf16
    (shifted fp8 u^T windows would need odd byte offsets in Ldweights)."""
    nc = bacc.Bacc("TRN2", target_bir_lowering=False, debug=False, num_devices=NCORES)
    xr = nc.dram_tensor("xr", (RPC + 2, D), BF, kind="ExternalInput").ap()
    x1r = nc.dram_tensor("x1r", (RPC, D), F16, kind="ExternalInput").ap()
    mut = nc.dram_tensor("mut", (KU, DC, P, D), BF, kind="ExternalInput").ap()
    fc1 = nc.dram_tensor("fc1", (JC, DC, P, 2, P), FP8, kind="ExternalInput").ap()
    fc2 = nc.dram_tensor("fc2", (JC, P, D), FP8, kind="ExternalInput").ap()
    o = nc.dram_tensor("o", (RPC, D), F16, kind="ExternalOutput").ap()

    with tile.TileContext(nc) as tc:
        with (
            tc.tile_pool(name="const", bufs=1) as const_pool,
            tc.tile_pool(name="persist", bufs=1) as persist,
            tc.tile_pool(name="work", bufs=6) as work,
            tc.tile_pool(name="wstream", bufs=5) as wstream,
            tc.tile_pool(name="psum", bufs=8, space="PSUM") as psum_pool,
        ):
            ident = const_pool.tile([P, P], F32)
            make_identity(nc, ident)
            eps_sb = const_pool.tile([P, 1], F32)
            nc.vector.memset(eps_sb, float(EPS))
            dummy = const_pool.tile([P, 1], F32, name="dummy")
            nc.scalar.activation(
                dummy, eps_sb, mybir.ActivationFunctionType.Square)
            nc.scalar.activation(
                dummy, dummy, mybir.ActivationFunctionType.Sqrt)

            # PE warmup: ramp the p-state while the first x rows stream in
            wz = const_pool.tile([P, 2, 2 * P], FP8, name="wz")
            nc.gpsimd.memset(wz, 0.0)
            wps = psum_pool.tile([P, 2 * P], F32, name="ps")
            for i in range(40):
                nc.tensor.matmul(wps, lhsT=wz[:, :, 0:P], rhs=wz,
                                 start=i == 0, stop=i == 39, perf_mode=DR)

            ut_ext = persist.tile([P, DC, MB, P + 2], BF)
            x1p = persist.tile([P, MB, D], F32)
            xrows = persist.tile([P, MB, D], BF)
            x1rows = persist.tile([P, MB, D], F16)
            yt = persist.tile([P, DC, 2, MB * P], FP8)
            gt = persist.tile([P, JC, 2, MB * P], FP8)
            mut_sb = persist.tile([P, KU, DC, D], BF)
            fc2_sb = persist.tile([P, JC, D], FP8)

            def rmsnorm_to(dst, src, rows, dve_sq=False, seng=0):
                """dst = src / rms(src); the rmsnorm weight is folded into
                the downstream contraction weights host-side."""
                sq = work.tile([P, D], F32, name="sq")
                ms = work.tile([P, 1], F32, name="ms")
                if dve_sq:
                    nc.vector.scalar_tensor_tensor(
                        sq[:rows], src[:rows], 1.0, src[:rows],
                        mybir.AluOpType.mult, mybir.AluOpType.mult,
                        accum_out=ms[:rows],
                    )
                else:
                    nc.scalar.activation(
                        sq[:rows], src[:rows],
                        mybir.ActivationFunctionType.Square,
                        accum_out=ms[:rows],
                    )
                nc.scalar.activation(
                    ms[:rows], ms[:rows], mybir.ActivationFunctionType.Sqrt,
                    bias=eps_sb[:rows], scale=1.0 / D,
                )
                nc.vector.reciprocal(ms[:rows], ms[:rows])
                if seng == 1:
                    nc.vector.tensor_scalar_mul(dst, src[:rows], ms[:rows])
                elif seng == 2:
                    nc.scalar.activation(
                        dst, src[:rows], mybir.ActivationFunctionType.Copy,
                        scale=ms[:rows])
                else:
                    nc.gpsimd.tensor_scalar_mul(dst, src[:rows], ms[:rows])

            # DMA queue front: prefix rows, the 4 x row blocks, mut taps (in
            # per-tap-half chunks so AR starts on the first), then x1 rows;
            # fc1/fc2 stream later in the fws loop
            u_pre = persist.tile([2, D], F32)
            xp = work.tile([P, D], BF, name="xt")[:2]
            nc.sync.dma_start(xp, xr[0:2, :])
            for m in range(MB):
                nc.sync.dma_start(
                    xrows[:, m, :], xr[2 + m * P: 2 + (m + 1) * P, :])
            HC = DC // 2
            for t in range(KU):
                for c in range(DC):
                    nc.sync.dma_start(
                        mut_sb[:, t, c, :],
                        mut[t, c].rearrange("p d -> p d"),
                    )
            for m in range(MB):
                nc.sync.dma_start(x1rows[:, m, :], x1r[m * P:(m + 1) * P, :])

            def psum_copy(dst, src_ps, idx):
                if idx % 2 == 0:
                    nc.vector.tensor_copy(dst, src_ps)
                else:
                    nc.scalar.activation(
                        dst, src_ps, mybir.ActivationFunctionType.Copy
                    )

            # ---- u^T tiles for the AR term (rmsnorm1 + PE transpose);
            # the 2-row prefix runs after the m blocks so it stays off the
            # critical path ----
            for m in range(MB):
                uo = work.tile([P, D], F32, name="uo")
                rmsnorm_to(uo, xrows[:, m, :], P, dve_sq=m % 2 == 0,
                           seng=(1, 2, 0, 0)[m])
                for c in range(DC):
                    pst = psum_pool.tile([P, F1], F32, name="ps")[:, 0:P]
                    nc.tensor.transpose(pst, uo[:, c * P:(c + 1) * P], ident)
                    psum_copy(ut_ext[:, c, m, 2:P + 2], pst, c + 1)
            rmsnorm_to(u_pre, xp, 2)
            for c in range(DC):
                pst2 = psum_pool.tile([P, F1], F32, name="ps")[:, 0:P]
                nc.tensor.transpose(
                    pst2[:, 0:2], u_pre[:, c * P:(c + 1) * P], ident[0:2, 0:2]
                )
                nc.vector.tensor_copy(ut_ext[:, c, 0, 0:2], pst2[:, 0:2])
            for m in range(1, MB):
                for c in range(DC):
                    nc.gpsimd.tensor_copy(
                        ut_ext[:, c, m, 0:2], ut_ext[:, c, m - 1, P:P + 2]
                    )

            # ---- AR term: all 4 row-blocks accumulate per-(tap, d-half) in
            # mut arrival order so the psum groups start on the first chunk ----
            psa = [(psum_pool.tile([P, F1], F32, name="ps"),
                    psum_pool.tile([P, F1], F32, name="ps"))
                   for _ in range(MB)]
            # last tap m-outer: psa[m] stops stagger by ~2us so each row
            # block's x1 add + rmsnorm2 chain overlaps the remaining taps
            groups = [(t, h, m) for t in range(KU - 1) for h in range(2)
                      for m in range(MB)]
            groups += [(KU - 1, h, m) for m in range(MB) for h in range(2)]
            for t, h, m in groups:
                for c in range(h * HC, (h + 1) * HC):
                    st = t == 0 and c == 0
                    fin = t == KU - 1 and c == DC - 1
                    lh = ut_ext[:, c, m, 2 - t:P + 2 - t]
                    nc.tensor.matmul(
                        psa[m][0], lhsT=lh,
                        rhs=mut_sb[:, t, c, 0:F1], start=st, stop=fin,
                    )
                    nc.tensor.matmul(
                        psa[m][1][:, 0:D - F1], lhsT=lh,
                        rhs=mut_sb[:, t, c, F1:D], start=st, stop=fin,
                    )
            for m in range(MB):
                nc.vector.tensor_tensor(
                    x1p[:, m, 0:F1], x1rows[:, m, 0:F1], psa[m][0],
                    mybir.AluOpType.add)
                nc.vector.tensor_tensor(
                    x1p[:, m, F1:D], x1rows[:, m, F1:D],
                    psa[m][1][:, 0:D - F1], mybir.AluOpType.add)

            # fc1 weight chunks + fc2 resident weights, in first-use order
            fws = []
            for jc in range(JC):
                fw = wstream.tile([P, DC, 2, P], FP8, name="fw")
                nc.sync.dma_start(fw, fc1[jc].rearrange("c p k f -> p c k f"))
                fws.append(fw)
                if jc in (10, 14, 18, 22):
                    q4 = JC // 4
                    qi = (jc - 10) // 4
                    nc.sync.dma_start(
                        fc2_sb[:, qi * q4:(qi + 1) * q4, :],
                        fc2[qi * q4:(qi + 1) * q4].rearrange("c p d -> p c d"))

            # ---- y = rmsnorm2(x1) transposed, hi + compensation lo ----
            for m in range(MB):
                yf = work.tile([P, D], F32, name="uo")
                rmsnorm_to(yf, x1p[:, m, :], P, dve_sq=m % 2 == 0,
                           seng=(1, 2, 0, 1)[m])
                for c in range(DC):
                    pst = psum_pool.tile([P, F1], F32, name="ps")[:, 0:P]
                    nc.tensor.transpose(pst, yf[:, c * P:(c + 1) * P], ident)
                    sl = slice(m * P, (m + 1) * P)
                    psum_copy(yt[:, c, 0, sl], pst, 1)
                    nc.vector.scalar_tensor_tensor(
                        yt[:, c, 1, sl], pst, 1.0, yt[:, c, 0, sl],
                        mybir.AluOpType.mult, mybir.AluOpType.subtract,
                    )

            # ---- fc1 + silu gate: (yh + yl) @ wh, 6 DR matmuls per half;
            # the first two jc's run per-row-block so they start while the
            # later row blocks' rmsnorm2/transpose chains still drain ----
            for jc in range(JC):
                ph1 = psum_pool.tile([P, F1], F32, name="ps")
                ph2 = psum_pool.tile([P, F1], F32, name="ps")
                for hh, ph in ((0, ph1), (1, ph2)):
                    if jc < 2:
                        for m in range(MB):
                            msl = slice(m * P, (m + 1) * P)
                            i_mm = 0
                            for q in range(2):
                                for cp in range(DC // 2):
                                    nc.tensor.matmul(
                                        ph[:, msl],
                                        lhsT=fws[jc][:, 2 * cp:2 * cp + 2,
                                                     hh, :],
                                        rhs=yt[:, 2 * cp:2 * cp + 2, q, msl],
                                        start=i_mm == 0, stop=i_mm == DC - 1,
                                        perf_mode=DR,
                                    )
                                    i_mm += 1
                        continue
                    i_mm = 0
                    for q in range(2):
                        for cp in range(DC // 2):
                            nc.tensor.matmul(
                                ph,
                                lhsT=fws[jc][:, 2 * cp:2 * cp + 2, hh, :],
                                rhs=yt[:, 2 * cp:2 * cp + 2, q, :],
                                start=i_mm == 0, stop=i_mm == DC - 1,
                                perf_mode=DR,
                            )
                            i_mm += 1
                sact = work.tile([P, F1], F32, name="sact")
                nc.scalar.activation(sact, ph2,
                                     mybir.ActivationFunctionType.Silu,
                                     scale=float(1.0 / MLP_SCALE))
                g32 = work.tile([P, F1], F32, name="g32")
                nc.vector.scalar_tensor_tensor(
                    g32, ph1, float(1.0 / MLP_SCALE), sact,
                    mybir.AluOpType.mult, mybir.AluOpType.mult,
                )
                nc.scalar.activation(
                    gt[:, jc, 0, :], g32, mybir.ActivationFunctionType.Copy
                )
                nc.vector.scalar_tensor_tensor(
                    gt[:, jc, 1, :], g32, 1.0, gt[:, jc, 0, :],
                    mybir.AluOpType.mult, mybir.AluOpType.subtract,
                )

            # ---- fc2 + residual: (gh + gl) @ f2h, m-outer so each row
            # block's residual add and output DMA overlap later matmuls ----
            for m in range(MB):
                msl = slice(m * P, (m + 1) * P)
                ot = work.tile([P, D], F16, name="ot")
                halves = (((0, F1), (F1, D)) if m < MB - 1 else
                          ((0, 256), (256, F1), (F1, D)))
                for hi, (d0, d1) in enumerate(halves):
                    po = psum_pool.tile([P, F1], F32, name="ps")[:, 0:d1 - d0]
                    i_mm = 0
                    for q in range(2):
                        for jp in range(JC // 2):
                            nc.tensor.matmul(
                                po,
                                lhsT=gt[:, 2 * jp:2 * jp + 2, q, msl],
                                rhs=fc2_sb[:, 2 * jp:2 * jp + 2, d0:d1],
                                start=i_mm == 0, stop=i_mm == JC - 1,
                                perf_mode=DR,
                            )
                            i_mm += 1
                    nc.vector.scalar_tensor_tensor(
                        ot[:, d0:d1], po, float(1.0 / MLP_SCALE),
                        x1p[:, m, d0:d1],
                        mybir.AluOpType.mult, mybir.AluOpType.add)
                    nc.sync.dma_start(o[m * P:(m + 1) * P, d0:d1],
                                      ot[:, d0:d1])
    nc.compile()
    return nc


def _host_prep(V, sigma, M_u, M_phi_plus, M_phi_minus, rn1):
    """Per-core tap blocks + fused projection matrices for the parity-split
    conv. Core c owns filters (2c, 2c+1). rn1_w folds into the projection's
    contraction axis; sigma^0.25 folds into the taps.

    tw[d0, r, ko, chain, f*128 + po*64 + j] =
        taps_f[128*(d0 - 2*ko) + (2j + po) - 2r - chain]   (0 if idx < 0)
    (chain 0 reads even input rows, 1 odd; output cols parity-major po)
    wt[f, sd, cp, i, ko, d] = {Ws,Wd}[f][d, (2cp+ko)*128 + i] * W_SCALE
    """
    phi = np.fft.irfft(V.astype(np.complex128), n=NFFT, axis=0)[:SL]
    s4 = sigma.astype(np.float64) ** 0.25
    taps = (s4[None, :] * phi)                       # (SL, K)
    Ws = (M_phi_plus + M_phi_minus) * rn1[None, None, :]
    Wd = (M_phi_plus - M_phi_minus) * rn1[None, None, :]

    # col -> s_out map (parity-major)
    s_out = np.empty(P, np.int64)
    s_out[:64] = 2 * np.arange(64)
    s_out[64:] = 2 * np.arange(64) + 1

    d0v = np.arange(NB)[:, None, None, None, None]
    rv = np.arange(P)[None, :, None, None, None]
    kov = np.arange(2)[None, None, :, None, None]
    chv = np.arange(2)[None, None, None, :, None]
    colv = s_out[None, None, None, None, :]
    idx = 128 * (d0v - 2 * kov) + colv - 2 * rv - chv   # (NB,P,2,2,P)

    tw_cores = []
    wt_cores = []
    for core in range(NCORES):
        tw = np.zeros((NB, P, 2, 2, FPC * P), np.float32)
        wt = np.zeros((FPC, 2, DC // 2, P, 2, D), np.float32)
        for f in range(FPC):
            k = core * FPC + f
            tsc = (taps[:, k] * TAP_SCALE).astype(np.float64)
            blk = np.where(idx >= 0, tsc[np.clip(idx, 0, SL - 1)], 0.0)
            tw[:, :, :, :, f * P:(f + 1) * P] = blk
            for sd, W in ((0, Ws[k]), (1, Wd[k])):
                for cp in range(DC // 2):
                    for ko in range(2):
                        c = 2 * cp + ko
                        wt[f, sd, cp, :, ko, :] = (
                            W[:, c * P:(c + 1) * P].T * W_SCALE)
        tw_cores.append(tw.astype(FP8NP))
        wt_cores.append(wt.astype(FP8NP))
    return tw_cores, wt_cores


def kernel(x, V, sigma, M_u, M_phi_plus, M_phi_minus, rn1_w, rn2_w, fc1_w, fc2_w):
    x = np.ascontiguousarray(x, np.float32)
    if "p1" not in _cache:
        _cache["p1"] = _SpmdRunner(_build_phase1(), shared=("xb",), volatile=("xb",))
    if "p2" not in _cache:
        _cache["p2"] = _SpmdRunner(
            _build_phase2(), shared=("mut", "fc1", "fc2"),
            volatile=("xr", "x1r"))

    rn1 = np.ascontiguousarray(rn1_w, np.float32)
    rn2 = np.ascontiguousarray(rn2_w, np.float32)
    tw_cores, wt_cores = _host_prep(V, sigma, M_u, M_phi_plus, M_phi_minus, rn1)
    xb = x.astype(BF16NP)

    in_maps1 = [
        {"xb": xb, "tw": tw_cores[c], "wt": wt_cores[c]}
        for c in range(NCORES)
    ]
    r1 = _cache["p1"]
    sp_cat = r1.run_prepped(r1.prep(in_maps1))[0]
    if "reduce" not in _cache:
        import jax
        import jax.numpy as jnp
        from jax.sharding import NamedSharding, PartitionSpec
        sh = NamedSharding(r1._shardings["xb"].mesh, PartitionSpec())
        _cache["reduce"] = jax.jit(
            lambda spc, xx: xx + spc.reshape(NCORES, B, SL, D)
            .astype(jnp.float32).sum(0),
            out_shardings=sh,
        )
    x1 = np.asarray(_cache["reduce"](sp_cat, np.asarray(x)))

    # phase 2 inputs; rn1 folds into mut's contraction axis, rn2 into fc1's
    mut = np.zeros((KU, DC, P, D), np.float32)
    for t in range(KU):
        for c in range(DC):
            mut[t, c] = (M_u[t] * rn1[None, :])[:, c * P:(c + 1) * P].T
    mut = mut.astype(BF16NP)
    # fc1 layout (JC, DC, P, 2, P): [..., 0, :] = y half column block jc,
    # [..., 1, :] = gate half column block jc (fp8 hi plane only; the y-side
    # hi/lo split on-device compensates the activation quantization)
    f1s = np.ascontiguousarray(fc1_w, np.float32) * rn2[:, None] * MLP_SCALE
    hi8 = f1s.astype(FP8NP)
    fc1p = np.ascontiguousarray(
        np.transpose(hi8.reshape(DC, P, 2, JC, P), (3, 0, 1, 2, 4)))
    f2s = np.ascontiguousarray(fc2_w, np.float32).reshape(JC, P, D) * MLP_SCALE
    fc2 = np.ascontiguousarray(f2s.astype(FP8NP))

    x_rows = x.reshape(B * SL, D)
    x1_rows = x1.reshape(B * SL, D)
    in_maps2 = []
    for c in range(NCORES):
        r0 = c * RPC
        xr = np.zeros((RPC + 2, D), np.float32)
        xr[2:] = x_rows[r0:r0 + RPC]
        if r0 % SL != 0:
            xr[0:2] = x_rows[r0 - 2:r0]
        in_maps2.append({
            "xr": xr.astype(BF16NP),
            "x1r": np.ascontiguousarray(
                x1_rows[r0:r0 + RPC]).astype(np.float16),
            "mut": mut, "fc1": fc1p, "fc2": fc2,
        })
    res2 = _cache["p2"](in_maps2)
    out = np.concatenate(
        [res2[c]["o"] for c in range(NCORES)], axis=0
    ).astype(np.float32).reshape(B, SL, D)
    return out



# revision 45
# speedup vs baseline: 1.3099x; 1.0030x over previous
"""Trainium2 Bass kernel for the STU (spectral transform unit) dense-transformer block.

Algorithm (validated against the jax reference in fp64 numpy):
  The FFT causal conv is a block-Toeplitz matmul. The two sign branches share
  work through the parity identity T^- = D T D (D = diag((-1)^s)): with the
  even/odd-input half-convs C_e[s] = sum_{s' even} phi[s-s'] u[s'] and C_o
  (odd s'), both branches are linear in (C_e, C_o):
    spectral[even s] = C_e Ws + C_o Wd,   spectral[odd s] = C_e Wd + C_o Ws,
    Ws = W+ + W-,  Wd = W+ - W-.
  Each half-conv reads half the input rows, so conv FLOPs halve vs computing
  U+ and U- separately; the +- reconstruction is free (it moves into which
  projection weight multiplies which psum). Projection psums pack the
  same-parity rows of two consecutive 128-row blocks (conv tap columns are
  emitted parity-major so proj lhsT slices stay contiguous). sigma^(1/4)
  folds into the taps, rn1/rn2 into downstream contraction weights. The
  KU=3 autoregressive taps are shifted-u^T projections with M_u; MLP is a
  standard gated MLP.

Sharding (8 cores, no cross-core communication, host-side reduce between two
uniform SPMD programs):
  Phase 1: filter-parallel. Core c computes C_e/C_o + fused projection for
           its 2 of the 16 filters over the full (B, SL): partial spectral.
  Host:    x1 = x + sum_c partial_c
  Phase 2: row-parallel. Core c owns 512 of the 4096 (b, s) rows: adds the AR
           term and computes the gated MLP + residual for its rows.

Precision: conv+projection in fp8 DoubleRow (noise ~3e-3 relative); AR in
bf16 (shifted fp8 u^T windows would need odd Ldweights byte offsets); the
MLP runs fp8 weights-hi-only with activation-side compensation
(h = (yh + yl) @ wh, yl = y - fp8(y); same for fc2 with g hi/lo), dropping
only the static w - fp8(w) perturbation. Measured end-to-end error:
1.35e-2 scale-relative (deterministic) vs the 2e-2 harness gate.

Schedule notes (all targets are the InstructionCostModel timeline):
 - DMA transfers are emitted in compute-need order; mut/fc2 stream in fine
   chunks so their consumers start on the first chunk.
 - PSUM->SBUF drains alternate DVE/Act (only they read PSUM); Pool (gpsimd)
   takes SBUF-side rmsnorm scales and fc1's gt-lo split; rmsnorm chains
   round-robin their square/scale ops across DVE/Act/Pool.
 - Phase-1 software-pipelines convs ahead of projections (conv psum pool 4
   bufs, one superblock in flight); b0 runs superblocks ascending, b1
   descending, so b1's conv-heavy big blocks absorb drain backlog; the
   final projection splits its F1/256 chains across two psum tiles so the
   last drain+DMA chain is short.
 - Both phases warm the PE p-state with dummy fp8 matmuls while first
   inputs stream in; work pools are deep enough (6-9 bufs) that rmsnorm
   chains for different row blocks overlap instead of serializing.
 - Phase-2's AR runs its last tap m-outer so each row block's x1-add ->
   rmsnorm2 -> transpose chain overlaps the remaining AR matmuls; fc1's
   first two hidden chunks run per-row-block to start during that drain.
"""

import numpy as np
import ml_dtypes

import concourse.bacc as bacc
import concourse.tile as tile
from concourse import mybir
from concourse.bass_utils import run_bass_kernel_spmd  # noqa: F401 (debug path)
from concourse.masks import make_identity


class _SpmdRunner:
    """Cached-jit SPMD executor: trace/compile once, then repeat calls only
    pay input upload + execution (mirrors bass2jax.run_bass_via_pjrt).

    ``shared`` names inputs that are identical on every core: they are fed
    replicated (host uploads one copy) instead of 8x-concatenated."""

    def __init__(self, nc, shared=(), volatile=()):
        import jax
        import concourse.mybir as _mb
        from concourse.bass2jax import (
            install_neuronx_cc_hook, _bass_exec_p, partition_id_tensor,
        )
        from jax.experimental.shard_map import shard_map
        from jax.sharding import Mesh, PartitionSpec

        install_neuronx_cc_hook()
        self.nc = nc
        assert nc.dbg_addr is None
        pid_name = (nc.partition_id_tensor.name
                    if nc.partition_id_tensor is not None else None)
        in_names, out_names, out_avals = [], [], []
        for alloc in nc.m.functions[0].allocations:
            if not isinstance(alloc, mybir.MemoryLocationSet):
                continue
            name = alloc.memorylocations[0].name
            if alloc.kind == "ExternalInput":
                if name != pid_name:
                    in_names.append(name)
            elif alloc.kind == "ExternalOutput":
                out_names.append(name)
                out_avals.append(jax.core.ShapedArray(
                    tuple(alloc.tensor_shape), mybir.dt.np(alloc.dtype)))
        self.in_names, self.out_names, self.out_avals = in_names, out_names, out_avals
        self.shared = frozenset(shared)
        self.volatile = frozenset(volatile)
        self._dev_cache = {}
        n_params = len(in_names)
        all_names = tuple(in_names + out_names)
        if pid_name is not None:
            all_names = all_names + (pid_name,)

        def _body(*args):
            args = list(args)
            if pid_name is not None:
                args.append(partition_id_tensor())
            return tuple(_bass_exec_p.bind(
                *args,
                out_avals=tuple(out_avals),
                in_names=all_names,
                out_names=tuple(out_names),
                lowering_input_output_aliases=(),
                sim_require_finite=True,
                sim_require_nnan=True,
                nc=nc,
            ))

        import jax.numpy as jnp
        from jax.sharding import NamedSharding
        devices = jax.devices()[:NCORES]
        mesh = Mesh(np.asarray(devices), ("core",))
        rep = PartitionSpec()
        core = PartitionSpec("core")
        in_specs = tuple(
            rep if nm in self.shared else core for nm in in_names
        ) + (core,) * len(out_names)
        out_specs = (core,) * len(out_names)
        donate = tuple(range(n_params, n_params + len(out_names)))
        self._fn = jax.jit(
            shard_map(_body, mesh=mesh, in_specs=in_specs, out_specs=out_specs,
                      check_rep=False),
            donate_argnums=donate, keep_unused=True,
        )
        self._zeros_fn = jax.jit(
            lambda: tuple(
                jnp.zeros((NCORES * a.shape[0], *a.shape[1:]), a.dtype)
                for a in out_avals
            ),
            out_shardings=tuple(
                NamedSharding(mesh, core) for _ in out_avals
            ),
        )
        self._shardings = {
            nm: NamedSharding(mesh, rep if nm in self.shared else core)
            for nm in in_names
        }

    def prep(self, in_maps):
        import hashlib
        import jax
        ins = []
        for nm in self.in_names:
            if nm in self.shared:
                arr = np.ascontiguousarray(in_maps[0][nm])
            else:
                arr = np.concatenate(
                    [np.asarray(in_maps[c][nm]) for c in range(NCORES)], axis=0)
            if nm in self.volatile:
                ins.append(arr)
                continue
            key = (nm, hashlib.md5(arr.tobytes()).hexdigest())
            dev = self._dev_cache.get(key)
            if dev is None:
                self._dev_cache.clear() if len(self._dev_cache) > 32 else None
                dev = jax.device_put(arr, self._shardings[nm])
                self._dev_cache[key] = dev
            ins.append(dev)
        return ins

    def run_prepped(self, ins):
        return self._fn(*ins, *self._zeros_fn())

    def __call__(self, in_maps):
        out_arrs = self.run_prepped(self.prep(in_maps))
        return [
            {nm: np.asarray(out_arrs[i]).reshape(NCORES, *self.out_avals[i].shape)[c]
             for i, nm in enumerate(self.out_names)}
            for c in range(NCORES)
        ]

BF16NP = ml_dtypes.bfloat16
FP8NP = ml_dtypes.float8_e4m3
TAP_SCALE = 1024.0
UT_SCALE = 32.0      # psum (TAP_SCALE*U) -> fp8 ut tiles scale factor: 32/1024
W_SCALE = 16.0       # projection weights scaled by 16 for fp8 range
SP_SCALE = UT_SCALE * W_SCALE  # spectral psum carries 32*16 = 512x
MLP_SCALE = 16.0     # fc1 hi/lo fp8 weights carry 16x for fp8 range
F32 = mybir.dt.float32
F32R = mybir.dt.float32r
F16 = mybir.dt.float16
BF = mybir.dt.bfloat16
FP8 = mybir.dt.float8e4

B, SL, D, K, KU = 2, 2048, 768, 16, 3
NFFT, EPS, P, H = 4096, 1e-5, 128, 3072
NB = SL // P            # 16 seq blocks
DC = D // P             # 6 d-chunks
NBR = 2 * K             # 32 conv branches
NCORES = 8
BPC = NBR // NCORES     # 4 branches per core
FPC = K // NCORES       # 2 filters per core (parity-fused conv)
NSB = SL // (2 * P)     # 8 superblocks (256 rows) per batch
RPC = (B * SL) // NCORES  # 512 rows per core
MB = RPC // P           # 4 row blocks per core in phase 2
JC = H // P             # 24 hidden chunks
F1 = 512                # free-dim split of D=768 into 512+256
DR = mybir.MatmulPerfMode.DoubleRow

_cache: dict = {}


def _build_phase1():
    """Parity-fused spectral conv: since T^- = D T D (D = alt signs), the
    even/odd half-convs C_e, C_o of each filter determine both sign branches:
      spectral[even s] = C_e Ws + C_o Wd,  spectral[odd s] = C_e Wd + C_o Ws
    with Ws = W+ + W-, Wd = W+ - W-. The conv FLOPs halve (each half-conv
    reads only half the input rows); the +- reconstruction is absorbed into
    the projection weights at no extra cost. Projection psums pack the
    same-parity rows of two consecutive 128-row blocks (conv output columns
    are emitted parity-major so the proj lhsT stays a contiguous 128-slice).
    Each core owns FPC=2 of the 16 filters."""
    nc = bacc.Bacc("TRN2", target_bir_lowering=False, debug=False, num_devices=NCORES)
    xb = nc.dram_tensor("xb", (B, SL, D), BF, kind="ExternalInput").ap()
    # tw[d0, r, ko, chain, f*128+col]: tap block pair (delta=d0-2*ko), chain
    # 0=even-input 1=odd-input, col parity-major within each filter's 128
    tw = nc.dram_tensor("tw", (NB, P, 2, 2, FPC * P), FP8, kind="ExternalInput").ap()
    # wt[f, sd(0=Ws,1=Wd), cp, r, ko, d_out]
    wt = nc.dram_tensor("wt", (FPC, 2, DC // 2, P, 2, D), FP8, kind="ExternalInput").ap()
    sp = nc.dram_tensor("sp", (B, SL, D), F16, kind="ExternalOutput").ap()

    with tile.TileContext(nc) as tc:
        with (
            tc.tile_pool(name="const", bufs=1) as const_pool,
            tc.tile_pool(name="ubuf", bufs=1) as ubuf_pool,
            tc.tile_pool(name="work", bufs=9) as work,
            tc.tile_pool(name="drain", bufs=4) as drain_pool,
            tc.tile_pool(name="spill", bufs=3) as spill_pool,
            tc.tile_pool(name="psum_u", bufs=4, space="PSUM") as psum_u_pool,
            tc.tile_pool(name="psum_sp", bufs=2, space="PSUM") as psum_sp_pool,
        ):
            eps_sb = const_pool.tile([P, 1], F32)
            nc.vector.memset(eps_sb, float(EPS))
            dummy = const_pool.tile([P, 1], F32, name="dummy")
            nc.scalar.activation(
                dummy, eps_sb, mybir.ActivationFunctionType.Square)
            nc.scalar.activation(
                dummy, dummy, mybir.ActivationFunctionType.Sqrt)
            tw_sb = const_pool.tile([P, NB, 2, 2, FPC * P], FP8)
            wt_sb = const_pool.tile([P, FPC, 2, DC // 2, 2, D], FP8)

            # one persistent fp8 u tile per (b, parity, even/odd-block pair):
            # u_t[b][par][jp][:, ko, :] holds rmsnormed rows
            # 256*(2*jp+ko) + 2r + par of batch b
            u_t = [[[ubuf_pool.tile([P, 2, D], FP8, name=f"u{b}_{par}_{jp}")
                     for jp in range(NB // 4)] for par in range(2)]
                   for b in range(B)]

            def jprep(b, par, blk, dve_sq=False, seng=0):
                """strided x row DMA (one parity class) -> rmsnorm -> fp8.
                (rn1_w is folded into the projection weights host-side.)
                seng: engine for the final scale (0=Pool, 1=DVE, 2=Act) --
                the head jpreps fan out so the Pool queue doesn't serialize
                the first conv's inputs."""
                xt = work.tile([P, D], BF, name="xt")
                r0 = 256 * blk + par
                nc.sync.dma_start(xt, xb[b, r0:r0 + 255:2, :])
                sq = work.tile([P, D], F32, name="sq")
                ms = work.tile([P, 1], F32, name="ms")
                if dve_sq:
                    nc.vector.scalar_tensor_tensor(
                        sq, xt, 1.0, xt, mybir.AluOpType.mult,
                        mybir.AluOpType.mult, accum_out=ms,
                    )
                else:
                    nc.scalar.activation(
                        sq, xt, mybir.ActivationFunctionType.Square,
                        accum_out=ms,
                    )
                nc.scalar.activation(
                    ms, ms, mybir.ActivationFunctionType.Sqrt,
                    bias=eps_sb, scale=1.0 / D,
                )
                nc.vector.reciprocal(ms, ms)
                dst = u_t[b][par][blk // 2][:, blk % 2, :]
                if seng == 1:
                    nc.vector.tensor_scalar_mul(dst, xt, ms)
                elif seng == 2:
                    nc.scalar.activation(
                        dst, xt, mybir.ActivationFunctionType.Copy, scale=ms)
                else:
                    nc.gpsimd.tensor_scalar_mul(dst, xt, ms)

            # PE warmup: dummy matmuls on a zero tile ramp the tensor
            # engine p-state while the first input blocks stream in (memset
            # on gpsimd so the PE isn't gated on the busier DVE queue)
            wz = const_pool.tile([P, 2, 2 * P], FP8, name="wz")
            nc.gpsimd.memset(wz, 0.0)
            wps = psum_u_pool.tile([P, 2 * P], F32, name="psu")
            NW = 34
            for i in range(NW):
                nc.tensor.matmul(wps, lhsT=wz[:, :, 0:P], rhs=wz,
                                 start=i == 0, stop=i == NW - 1, perf_mode=DR)

            # prologue: x rows for conv blocks 0,1 stream before the tap
            # blocks (the taps are only needed once the PE issues Ldweights);
            # scale engines fan out so no single queue serializes readiness
            jprep(0, 0, 0, dve_sq=True, seng=1)
            jprep(0, 1, 0, seng=2)
            nc.sync.dma_start(tw_sb[:, 0], tw[0])
            nc.sync.dma_start(tw_sb[:, 1], tw[1])
            jprep(0, 0, 1, seng=0)
            jprep(0, 1, 1, seng=1)
            nc.sync.dma_start(
                wt_sb[:, :, :, 0, :, :],
                wt[:, :, 0].rearrange("f s p k d -> p f s k d"))
            next_blk = [2, 0]
            jseng = [0]

            def conv_block(b, I, ut_sb):
                """both half-conv chains for seq block I into one psum; drain
                into the superblock ut tile (cols parity-major per filter)."""
                npair = I // 4 + 1
                half = I % 2
                for c in range(DC):
                    ps = psum_u_pool.tile([P, 2 * FPC * P], F32, name="psu")
                    for chain in range(2):
                        dst = ps[:, chain * FPC * P:(chain + 1) * FPC * P]
                        if b == 0 and I < 2:
                            # deltas (I, I-2): the ko=1 half is all-zero taps;
                            # a plain (non-DR) matmul on the first pair-half
                            # depends only on x rows 0..255, so the first
                            # convs start ~2us earlier
                            nc.tensor.matmul(
                                dst,
                                lhsT=u_t[b][chain][0][:, 0, c * P:(c + 1) * P],
                                rhs=tw_sb[:, I, 0, chain, :],
                                start=True, stop=True,
                            )
                            continue
                        for Jp in range(npair):
                            nc.tensor.matmul(
                                dst,
                                lhsT=u_t[b][chain][Jp][:, :, c * P:(c + 1) * P],
                                rhs=tw_sb[:, I - 4 * Jp, :, chain, :],
                                start=(Jp == 0),
                                stop=(Jp == npair - 1),
                                perf_mode=DR,
                            )
                    # psum free dim = (chain, f, par, 64); ut free dim per
                    # chunk = (chain*FPC+f, par, 128=(half,64))
                    pv = ps.rearrange("p (s q j) -> p s q j", s=2 * FPC, q=2)
                    dst = ut_sb[:, c, :, :, half * 64:half * 64 + 64]
                    if c % 2 == 0:
                        nc.vector.tensor_scalar_mul(
                            dst, pv, float(UT_SCALE / TAP_SCALE))
                    else:
                        nc.scalar.activation(
                            dst, pv, mybir.ActivationFunctionType.Copy,
                            scale=float(UT_SCALE / TAP_SCALE),
                        )

            def proj_block(b, Ip, par, ut_sb, last=False):
                """one parity's projection for superblock Ip: 128 same-parity
                rows of blocks (2Ip, 2Ip+1); sd picks Ws for the matching
                parity chain, Wd for the crossed one."""
                psp = psum_sp_pool.tile([P, D], F32, name="psp")
                sp_t = spill_pool.tile([P, D], F16, name="spt")
                r0 = 256 * Ip + par
                if last:
                    # tail only: sequential F1/256 chains so the F1 drain +
                    # DMA overlap the 256 matmuls (elsewhere this loses --
                    # the drain read blocks the tile's second chain)
                    halves = [[(0, F1)], [(F1, D)]]
                else:
                    halves = [[(0, F1), (F1, D)]]
                for grp_i, grp in enumerate(halves):
                    pst = psp
                    if last and grp_i == 1:
                        # separate psum tile: no tile-level WAR against the
                        # F1 chain's drain read
                        pst = psum_sp_pool.tile([P, D], F32, name="psp")
                    i_mm = 0
                    n_mm = 2 * FPC * (DC // 2)
                    for cp in range(DC // 2):
                        for st_i in range(2 * FPC):
                            chain, f = divmod(st_i, FPC)
                            sd = par if chain == 0 else 1 - par
                            lh = ut_sb[:, 2 * cp:2 * cp + 2, st_i, par, :]
                            for d0, d1 in grp:
                                nc.tensor.matmul(
                                    pst[:, d0:d1], lhsT=lh,
                                    rhs=wt_sb[:, f, sd, cp, :, d0:d1],
                                    start=i_mm == 0, stop=i_mm == n_mm - 1,
                                    perf_mode=DR,
                                )
                            i_mm += 1
                    for gi, (d0, d1) in enumerate(grp):
                        if (par + gi + grp_i) % 2 == 0:
                            nc.scalar.activation(
                                sp_t[:, d0:d1], pst[:, d0:d1],
                                mybir.ActivationFunctionType.Copy,
                                scale=float(1.0 / SP_SCALE),
                            )
                        else:
                            nc.vector.tensor_scalar_mul(
                                sp_t[:, d0:d1], pst[:, d0:d1],
                                float(1.0 / SP_SCALE))
                        nc.sync.dma_start(
                            sp[b, r0:r0 + 255:2, d0:d1], sp_t[:, d0:d1])

            # software pipeline: emit convs ahead of projections so the PE
            # covers psum-drain latency; a superblock's proj needs both its
            # conv blocks drained
            from collections import deque
            pend = deque()
            # b=0 ascending, b=1 descending: b1's large-I blocks (conv-heavy)
            # land right after b0's (drain-heavy) first half, keeping the
            # drain engines the bottleneck only briefly
            sched = [(0, Ip) for Ip in range(NSB)]
            sched += [(1, Ip) for Ip in reversed(range(NSB))]
            for si, (b, Ip) in enumerate(sched):
                ut_sb = drain_pool.tile([P, DC, 2 * FPC, 2, P], FP8,
                                        name="ut")
                for half in range(2):
                    I = 2 * Ip + half
                    # pace the DMA queue: tw chunk I+2, remaining wt
                    # chunks, upcoming u row blocks
                    if b == 0 and I + 2 < NB:
                        nc.sync.dma_start(tw_sb[:, I + 2], tw[I + 2])
                    if b == 0 and 1 <= I < 3:
                        cp = I
                        nc.sync.dma_start(
                            wt_sb[:, :, :, cp, :, :],
                            wt[:, :, cp].rearrange(
                                "f s p k d -> p f s k d"))
                    need = min(2 * (((I + 4) // 4)) + 1, NB // 2 - 1)
                    while next_blk[b] <= need:
                        jprep(b, 0, next_blk[b], seng=jseng[0] % 3)
                        jprep(b, 1, next_blk[b], dve_sq=True,
                              seng=(jseng[0] + 1) % 3)
                        jseng[0] += 2
                        next_blk[b] += 1
                    if b == 0 and I >= 8:
                        while next_blk[1] <= min(I - 8, NB // 2 - 1):
                            jprep(1, 0, next_blk[1], seng=jseng[0] % 3)
                            jprep(1, 1, next_blk[1], dve_sq=True,
                                  seng=(jseng[0] + 1) % 3)
                            jseng[0] += 2
                            next_blk[1] += 1
                    conv_block(b, I, ut_sb)
                pend.append((b, Ip, ut_sb))
                depth = 2 if si < 3 else 1
                while len(pend) > depth:
                    pb, pIp, put = pend.popleft()
                    proj_block(pb, pIp, 0, put)
                    proj_block(pb, pIp, 1, put)
            while pend:
                pb, pIp, put = pend.popleft()
                proj_block(pb, pIp, 0, put)
                proj_block(pb, pIp, 1, put, last=True)
    nc.compile()
    return nc


def _build_phase2():
    """Row-parallel AR + gated MLP. The MLP runs fp8 weights-hi-only with
    activation-side compensation: h = (yh + yl) @ wh (yl = y - fp8(y)), so
    the only dropped term is y @ (w - fp8(w)) -- a fixed small weight
    perturbation. Same for fc2 with g hi/lo. Halves both the matmul count
    (vs the hi/lo cross scheme) and the weight DMA. The AR term stays bf16
    (shifted fp8 u^T windows would need odd byte offsets in Ldweights)."""
    nc = bacc.Bacc("TRN2", target_bir_lowering=False, debug=False, num_devices=NCORES)
    xr = nc.dram_tensor("xr", (RPC + 2, D), BF, kind="ExternalInput").ap()
    x1r = nc.dram_tensor("x1r", (RPC, D), F16, kind="ExternalInput").ap()
    mut = nc.dram_tensor("mut", (KU, DC, P, D), BF, kind="ExternalInput").ap()
    fc1 = nc.dram_tensor("fc1", (JC, DC, P, 2, P), FP8, kind="ExternalInput").ap()
    fc2 = nc.dram_tensor("fc2", (JC, P, D), FP8, kind="ExternalInput").ap()
    o = nc.dram_tensor("o", (RPC, D), F16, kind="ExternalOutput").ap()

    with tile.TileContext(nc) as tc:
        with (
            tc.tile_pool(name="const", bufs=1) as const_pool,
            tc.tile_pool(name="persist", bufs=1) as persist,
            tc.tile_pool(name="work", bufs=6) as work,
            tc.tile_pool(name="wstream", bufs=5) as wstream,
            tc.tile_pool(name="psum", bufs=8, space="PSUM") as psum_pool,
        ):
            ident = const_pool.tile([P, P], F32)
            make_identity(nc, ident)
            eps_sb = const_pool.tile([P, 1], F32)
            nc.vector.memset(eps_sb, float(EPS))
            dummy = const_pool.tile([P, 1], F32, name="dummy")
            nc.scalar.activation(
                dummy, eps_sb, mybir.ActivationFunctionType.Square)
            nc.scalar.activation(
                dummy, dummy, mybir.ActivationFunctionType.Sqrt)

            # PE warmup: ramp the p-state while the first x rows stream in
            wz = const_pool.tile([P, 2, 2 * P], FP8, name="wz")
            nc.gpsimd.memset(wz, 0.0)
            wps = psum_pool.tile([P, 2 * P], F32, name="ps")
            for i in range(40):
                nc.tensor.matmul(wps, lhsT=wz[:, :, 0:P], rhs=wz,
                                 start=i == 0, stop=i == 39, perf_mode=DR)

            ut_ext = persist.tile([P, DC, MB, P + 2], BF)
            x1p = persist.tile([P, MB, D], F32)
            xrows = persist.tile([P, MB, D], BF)
            x1rows = persist.tile([P, MB, D], F16)
            yt = persist.tile([P, DC, 2, MB * P], FP8)
            gt = persist.tile([P, JC, 2, MB * P], FP8)
            mut_sb = persist.tile([P, KU, DC, D], BF)
            fc2_sb = persist.tile([P, JC, D], FP8)

            def rmsnorm_to(dst, src, rows, dve_sq=False, seng=0):
                """dst = src / rms(src); the rmsnorm weight is folded into
                the downstream contraction weights host-side."""
                sq = work.tile([P, D], F32, name="sq")
                ms = work.tile([P, 1], F32, name="ms")
                if dve_sq:
                    nc.vector.scalar_tensor_tensor(
                        sq[:rows], src[:rows], 1.0, src[:rows],
                        mybir.AluOpType.mult, mybir.AluOpType.mult,
                        accum_out=ms[:rows],
                    )
                else:
                    nc.scalar.activation(
                        sq[:rows], src[:rows],
                        mybir.ActivationFunctionType.Square,
                        accum_out=ms[:rows],
                    )
                nc.scalar.activation(
                    ms[:rows], ms[:rows], mybir.ActivationFunctionType.Sqrt,
                    bias=eps_sb[:rows], scale=1.0 / D,
                )
                nc.vector.reciprocal(ms[:rows], ms[:rows])
                if seng == 1:
                    nc.vector.tensor_scalar_mul(dst, src[:rows], ms[:rows])
                elif seng == 2:
                    nc.scalar.activation(
                        dst, src[:rows], mybir.ActivationFunctionType.Copy,
                        scale=ms[:rows])
                else:
                    nc.gpsimd.tensor_scalar_mul(dst, src[:rows], ms[:rows])

            # DMA queue front: prefix rows, the 4 x row blocks, mut taps (in
            # per-tap-half chunks so AR starts on the first), then x1 rows;
            # fc1/fc2 stream later in the fws loop
            u_pre = persist.tile([2, D], F32)
            xp = work.tile([P, D], BF, name="xt")[:2]
            nc.sync.dma_start(xp, xr[0:2, :])
            for m in range(MB):
                nc.sync.dma_start(
                    xrows[:, m, :], xr[2 + m * P: 2 + (m + 1) * P, :])
            HC = DC // 2
            for t in range(KU):
                for c in range(DC):
                    nc.sync.dma_start(
                        mut_sb[:, t, c, :],
                        mut[t, c].rearrange("p d -> p d"),
                    )
            for m in range(MB):
                nc.sync.dma_start(x1rows[:, m, :], x1r[m * P:(m + 1) * P, :])

            def psum_copy(dst, src_ps, idx):
                if idx % 2 == 0:
                    nc.vector.tensor_copy(dst, src_ps)
                else:
                    nc.scalar.activation(
                        dst, src_ps, mybir.ActivationFunctionType.Copy
                    )

            # ---- u^T tiles for the AR term (rmsnorm1 + PE transpose);
            # the 2-row prefix runs after the m blocks so it stays off the
            # critical path ----
            for m in range(MB):
                uo = work.tile([P, D], F32, name="uo")
                rmsnorm_to(uo, xrows[:, m, :], P, dve_sq=m % 2 == 0,
                           seng=(1, 2, 0, 0)[m])
                for c in range(DC):
                    pst = psum_pool.tile([P, F1], F32, name="ps")[:, 0:P]
                    nc.tensor.transpose(pst, uo[:, c * P:(c + 1) * P], ident)
                    psum_copy(ut_ext[:, c, m, 2:P + 2], pst, c + 1)
            rmsnorm_to(u_pre, xp, 2)
            for c in range(DC):
                pst2 = psum_pool.tile([P, F1], F32, name="ps")[:, 0:P]
                nc.tensor.transpose(
                    pst2[:, 0:2], u_pre[:, c * P:(c + 1) * P], ident[0:2, 0:2]
                )
                nc.vector.tensor_copy(ut_ext[:, c, 0, 0:2], pst2[:, 0:2])
            for m in range(1, MB):
                for c in range(DC):
                    nc.gpsimd.tensor_copy(
                        ut_ext[:, c, m, 0:2], ut_ext[:, c, m - 1, P:P + 2]
                    )

            # ---- AR term: all 4 row-blocks accumulate per-(tap, d-half) in
            # mut arrival order so the psum groups start on the first chunk ----
            psa = [(psum_pool.tile([P, F1], F32, name="ps"),
                    psum_pool.tile([P, F1], F32, name="ps"))
                   for _ in range(MB)]
            # last tap m-outer: psa[m] stops stagger by ~2us so each row
            # block's x1 add + rmsnorm2 chain overlaps the remaining taps
            groups = [(t, h, m) for t in range(KU - 1) for h in range(2)
                      for m in range(MB)]
            groups += [(KU - 1, h, m) for m in range(MB) for h in range(2)]
            for t, h, m in groups:
                for c in range(h * HC, (h + 1) * HC):
                    st = t == 0 and c == 0
                    fin = t == KU - 1 and c == DC - 1
                    lh = ut_ext[:, c, m, 2 - t:P + 2 - t]
                    nc.tensor.matmul(
                        psa[m][0], lhsT=lh,
                        rhs=mut_sb[:, t, c, 0:F1], start=st, stop=fin,
                    )
                    nc.tensor.matmul(
                        psa[m][1][:, 0:D - F1], lhsT=lh,
                        rhs=mut_sb[:, t, c, F1:D], start=st, stop=fin,
                    )
            for m in range(MB):
                nc.vector.tensor_tensor(
                    x1p[:, m, 0:F1], x1rows[:, m, 0:F1], psa[m][0],
                    mybir.AluOpType.add)
                nc.vector.tensor_tensor(
                    x1p[:, m, F1:D], x1rows[:, m, F1:D],
                    psa[m][1][:, 0:D - F1], mybir.AluOpType.add)

            # fc1 weight chunks + fc2 resident weights, in first-use order
            fws = []
            for jc in range(JC):
                fw = wstream.tile([P, DC, 2, P], FP8, name="fw")
                nc.sync.dma_start(fw, fc1[jc].rearrange("c p k f -> p c k f"))
                fws.append(fw)
                if jc in (10, 14, 18, 22):
                    q4 = JC // 4
                    qi = (jc - 10) // 4
                    nc.sync.dma_start(
                        fc2_sb[:, qi * q4:(qi + 1) * q4, :],
                        fc2[qi * q4:(qi + 1) * q4].rearrange("c p d -> p c d"))

            # ---- y = rmsnorm2(x1) transposed, hi + compensation lo ----
            for m in range(MB):
                yf = work.tile([P, D], F32, name="uo")
                rmsnorm_to(yf, x1p[:, m, :], P, dve_sq=m % 2 == 0,
                           seng=(1, 2, 0, 1)[m])
                for c in range(DC):
                    pst = psum_pool.tile([P, F1], F32, name="ps")[:, 0:P]
                    nc.tensor.transpose(pst, yf[:, c * P:(c + 1) * P], ident)
                    sl = slice(m * P, (m + 1) * P)
                    psum_copy(yt[:, c, 0, sl], pst, 1)
                    nc.vector.scalar_tensor_tensor(
                        yt[:, c, 1, sl], pst, 1.0, yt[:, c, 0, sl],
                        mybir.AluOpType.mult, mybir.AluOpType.subtract,
                    )

            # ---- fc1 + silu gate: (yh + yl) @ wh, 6 DR matmuls per half;
            # the first two jc's run per-row-block so they start while the
            # later row blocks' rmsnorm2/transpose chains still drain ----
            for jc in range(JC):
                ph1 = psum_pool.tile([P, F1], F32, name="ps")
                ph2 = psum_pool.tile([P, F1], F32, name="ps")
                for hh, ph in ((0, ph1), (1, ph2)):
                    if jc < 2:
                        for m in range(MB):
                            msl = slice(m * P, (m + 1) * P)
                            i_mm = 0
                            for q in range(2):
                                for cp in range(DC // 2):
                                    nc.tensor.matmul(
                                        ph[:, msl],
                                        lhsT=fws[jc][:, 2 * cp:2 * cp + 2,
                                                     hh, :],
                                        rhs=yt[:, 2 * cp:2 * cp + 2, q, msl],
                                        start=i_mm == 0, stop=i_mm == DC - 1,
                                        perf_mode=DR,
                                    )
                                    i_mm += 1
                        continue
                    i_mm = 0
                    for q in range(2):
                        for cp in range(DC // 2):
                            nc.tensor.matmul(
                                ph,
                                lhsT=fws[jc][:, 2 * cp:2 * cp + 2, hh, :],
                                rhs=yt[:, 2 * cp:2 * cp + 2, q, :],
                                start=i_mm == 0, stop=i_mm == DC - 1,
                                perf_mode=DR,
                            )
                            i_mm += 1
                sact = work.tile([P, F1], F32, name="sact")
                nc.scalar.activation(sact, ph2,
                                     mybir.ActivationFunctionType.Silu,
                                     scale=float(1.0 / MLP_SCALE))
                g32 = work.tile([P, F1], F32, name="g32")
                nc.vector.scalar_tensor_tensor(
                    g32, ph1, float(1.0 / MLP_SCALE), sact,
                    mybir.AluOpType.mult, mybir.AluOpType.mult,
                )
                nc.scalar.activation(
                    gt[:, jc, 0, :], g32, mybir.ActivationFunctionType.Copy
                )
                nc.vector.scalar_tensor_tensor(
                    gt[:, jc, 1, :], g32, 1.0, gt[:, jc, 0, :],
                    mybir.AluOpType.mult, mybir.AluOpType.subtract,
                )

            # ---- fc2 + residual: (gh + gl) @ f2h, m-outer so each row
            # block's residual add and output DMA overlap later matmuls ----
            for m in range(MB):
                msl = slice(m * P, (m + 1) * P)
                ot = work.tile([P, D], F16, name="ot")
                halves = (((0, F1), (F1, D)) if m < MB - 1 else
                          ((0, 256), (256, F1), (F1, D)))
                for hi, (d0, d1) in enumerate(halves):
                    po = psum_pool.tile([P, F1], F32, name="ps")[:, 0:d1 - d0]
                    i_mm = 0
                    for q in range(2):
                        for jp in range(JC // 2):
                            nc.tensor.matmul(
                                po,
                                lhsT=gt[:, 2 * jp:2 * jp + 2, q, msl],
                                rhs=fc2_sb[:, 2 * jp:2 * jp + 2, d0:d1],
                                start=i_mm == 0, stop=i_mm == JC - 1,
                                perf_mode=DR,
                            )
                            i_mm += 1
                    nc.vector.scalar_tensor_tensor(
                        ot[:, d0:d1], po, float(1.0 / MLP_SCALE),
                        x1p[:, m, d0:d1],
                        mybir.AluOpType.mult, mybir.AluOpType.add)
                    nc.sync.dma_start(o[m * P:(m + 1) * P, d0:d1],
                                      ot[:, d0:d1])
    nc.compile()
    return nc


def _host_prep(V, sigma, M_u, M_phi_plus, M_phi_minus, rn1):
    """Per-core tap blocks + fused projection matrices for the parity-split
    conv. Core c owns filters (2c, 2c+1). rn1_w folds into the projection's
    contraction axis; sigma^0.25 folds into the taps.

    tw[d0, r, ko, chain, f*128 + po*64 + j] =
        taps_f[128*(d0 - 2*ko) + (2j + po) - 2r - chain]   (0 if idx < 0)
    (chain 0 reads even input rows, 1 odd; output cols parity-major po)
    wt[f, sd, cp, i, ko, d] = {Ws,Wd}[f][d, (2cp+ko)*128 + i] * W_SCALE
    """
    phi = np.fft.irfft(V.astype(np.complex128), n=NFFT, axis=0)[:SL]
    s4 = sigma.astype(np.float64) ** 0.25
    taps = (s4[None, :] * phi)                       # (SL, K)
    Ws = (M_phi_plus + M_phi_minus) * rn1[None, None, :]
    Wd = (M_phi_plus - M_phi_minus) * rn1[None, None, :]

    # col -> s_out map (parity-major)
    s_out = np.empty(P, np.int64)
    s_out[:64] = 2 * np.arange(64)
    s_out[64:] = 2 * np.arange(64) + 1

    d0v = np.arange(NB)[:, None, None, None, None]
    rv = np.arange(P)[None, :, None, None, None]
    kov = np.arange(2)[None, None, :, None, None]
    chv = np.arange(2)[None, None, None, :, None]
    colv = s_out[None, None, None, None, :]
    idx = 128 * (d0v - 2 * kov) + colv - 2 * rv - chv   # (NB,P,2,2,P)

    tw_cores = []
    wt_cores = []
    for core in range(NCORES):
        tw = np.zeros((NB, P, 2, 2, FPC * P), np.float32)
        wt = np.zeros((FPC, 2, DC // 2, P, 2, D), np.float32)
        for f in range(FPC):
            k = core * FPC + f
            tsc = (taps[:, k] * TAP_SCALE).astype(np.float64)
            blk = np.where(idx >= 0, tsc[np.clip(idx, 0, SL - 1)], 0.0)
            tw[:, :, :, :, f * P:(f + 1) * P] = blk
            for sd, W in ((0, Ws[k]), (1, Wd[k])):
                for cp in range(DC // 2):
                    for ko in range(2):
                        c = 2 * cp + ko
                        wt[f, sd, cp, :, ko, :] = (
                            W[:, c * P:(c + 1) * P].T * W_SCALE)
        tw_cores.append(tw.astype(FP8NP))
        wt_cores.append(wt.astype(FP8NP))
    return tw_cores, wt_cores


def kernel(x, V, sigma, M_u, M_phi_plus, M_phi_minus, rn1_w, rn2_w, fc1_w, fc2_w):
    x = np.ascontiguousarray(x, np.float32)
    if "p1" not in _cache:
        _cache["p1"] = _SpmdRunner(_build_phase1(), shared=("xb",), volatile=("xb",))
    if "p2" not in _cache:
        _cache["p2"] = _SpmdRunner(
            _build_phase2(), shared=("mut", "fc1", "fc2"),
            volatile=("xr", "x1r"))

    rn1 = np.ascontiguousarray(rn1_w, np.float32)
    rn2 = np.ascontiguousarray(rn2_w, np.float32)
    tw_cores, wt_cores = _host_prep(V, sigma, M_u, M_phi_plus, M_phi_minus, rn1)
    xb = x.astype(BF16NP)

    in_maps1 = [
        {"xb": xb, "tw": tw_cores[c], "wt": wt_cores[c]}
        for c in range(NCORES)
    ]
    r1 = _cache["p1"]
    sp_cat = r1.run_prepped(r1.prep(in_maps1))[0]
    if "reduce" not in _cache:
        import jax
        import jax.numpy as jnp
        from jax.sharding import NamedSharding, PartitionSpec
        sh = NamedSharding(r1._shardings["xb"].mesh, PartitionSpec())
        _cache["reduce"] = jax.jit(
            lambda spc, xx: xx + spc.reshape(NCORES, B, SL, D)
            .astype(jnp.float32).sum(0),
            out_shardings=sh,
        )
    x1 = np.asarray(_cache["reduce"](sp_cat, np.asarray(x)))

    # phase 2 inputs; rn1 folds into mut's contraction axis, rn2 into fc1's
    mut = np.zeros((KU, DC, P, D), np.float32)
    for t in range(KU):
        for c in range(DC):
            mut[t, c] = (M_u[t] * rn1[None, :])[:, c * P:(c + 1) * P].T
    mut = mut.astype(BF16NP)
    # fc1 layout (JC, DC, P, 2, P): [..., 0, :] = y half column block jc,
    # [..., 1, :] = gate half column block jc (fp8 hi plane only; the y-side
    # hi/lo split on-device compensates the activation quantization)
    f1s = np.ascontiguousarray(fc1_w, np.float32) * rn2[:, None] * MLP_SCALE
    hi8 = f1s.astype(FP8NP)
    fc1p = np.ascontiguousarray(
        np.transpose(hi8.reshape(DC, P, 2, JC, P), (3, 0, 1, 2, 4)))
    f2s = np.ascontiguousarray(fc2_w, np.float32).reshape(JC, P, D) * MLP_SCALE
    fc2 = np.ascontiguousarray(f2s.astype(FP8NP))

    x_rows = x.reshape(B * SL, D)
    x1_rows = x1.reshape(B * SL, D)
    in_maps2 = []
    for c in range(NCORES):
        r0 = c * RPC
        xr = np.zeros((RPC + 2, D), np.float32)
        xr[2:] = x_rows[r0:r0 + RPC]
        if r0 % SL != 0:
            xr[0:2] = x_rows[r0 - 2:r0]
        in_maps2.append({
            "xr": xr.astype(BF16NP),
            "x1r": np.ascontiguousarray(
                x1_rows[r0:r0 + RPC]).astype(np.float16),
            "mut": mut, "fc1": fc1p, "fc2": fc2,
        })
    res2 = _cache["p2"](in_maps2)
    out = np.concatenate(
        [res2[c]["o"] for c in range(NCORES)], axis=0
    ).astype(np.float32).reshape(B, SL, D)
    return out

